# revision 1
# baseline (speedup 1.0000x reference)
"""Circulant matmul for TRN2 (final): 3-level CRT split, f32r matmuls.

out[b, r] = sum_c x[b,c] * w[(c-r) mod N]  ==  x @ C.T, C circulant from w.

- Partition-reversed convention makes the circulant band a positive shear:
  band2[p, q] = w3[1 + p + q] (w3 = tile(rev(w), 3)), loaded in 2 DMAs; the
  host passes xtr = x_shard.T with the c axis reversed to match.
- CRT split z^N-1 -> cyc512 + nega512 + nega1024 + nega2048: 88 matmuls of
  [K=128, M=128, N=512] per 128-row block vs 256 dense (2.9x fewer FLOPs).
  All operator bands derive from band2 by shifted adds/subs along the free
  axis (periodicity-reduced), computed once on device.
- float32r matmuls: full PE rate (1 col/cycle, HW-verified), ~15x better
  accuracy than bf16. CRT 1/2-per-level scales fold into ACT PSUM->SBUF
  copies. DVE unfold combines are deferred behind the next block's folds.
- Emission order tuned so the first row block's folds overlap the band
  build and the level-2/3 band derivation happens between the nega and
  cyclic matmul phases of block 0.
"""

import sys

sys.path.insert(0, "/opt/trn_rl_repo")

import numpy as np

N = 4096
B = 8192
N_CORES = 8
B_SHARD = B // N_CORES  # 1024
NB = B_SHARD // 128     # 8 row-tiles per core

_STATE = {}


def _build():
    import concourse.bacc as bacc
    import concourse.mybir as mybir
    import concourse.tile as tile
    import bass_rust

    f32 = mybir.dt.float32
    f32r = mybir.dt.float32r
    ADD = mybir.AluOpType.add
    SUB = mybir.AluOpType.subtract
    mmdt = f32r

    nc = bacc.Bacc("TRN2", target_bir_lowering=False, debug=False)
    xtr_d = nc.declare_dram_parameter("xtr", [N, B_SHARD], f32, isOutput=False)
    w3_d = nc.declare_dram_parameter("w3", [3 * N], f32, isOutput=False)
    out_d = nc.declare_dram_parameter("out", [B_SHARD, N], f32, isOutput=True)

    xtr_t = xtr_d[:].rearrange("(a p) b -> p a b", p=128)  # [128, 32, B_SHARD]

    with tile.TileContext(nc) as tc:
        with (
            tc.tile_pool(name="const", bufs=1) as constp,
            tc.tile_pool(name="xbig", bufs=2) as xbigp,
            tc.tile_pool(name="xplus", bufs=18) as xpp_pool,
            tc.tile_pool(name="xmm", bufs=9) as xmp,
            tc.tile_pool(name="outp", bufs=2) as op,
            tc.tile_pool(name="psum", bufs=1, space="PSUM") as pp,
        ):
            band_mh = constp.tile([128, 3968], mmdt)
            band_pmh = constp.tile([128, 1920], mmdt)
            band_3ph = constp.tile([128, 896], mmdt)
            band_3mh = constp.tile([128, 896], mmdt)

            # PE clock warmup: the HAM gate keeps an idle PE at 1.2 GHz and
            # only releases to 2.4 GHz after ~3.4us of sustained activity.
            # Burn dummy matmuls (never-read results, uninitialized operands)
            # while the band/x DMAs stream so the real matmul stream starts
            # and stays warm.
            warm_in = constp.tile([128, 512], mmdt, name="warm_in")
            warm_f = constp.tile([128, 512], f32, name="warm_f")
            nc.gpsimd.memset(warm_f[:], 0.0)
            nc.vector.tensor_copy(warm_in[:], warm_f[:])

            # ---------- per-block emission helpers ----------
            def emit_nega(bt, xbig):
                """x folds (xpl kept for level 2) + nega-2048 matmuls."""
                s_m = pp.tile([128, 2048], f32, tag="sm", name="sm")
                if bt == 0:
                    # PE clock warmup: HAM keeps an idle PE at 1.2 GHz and
                    # releases to 2.4 GHz only after ~3.4us of sustained
                    # activity. Burn dummy matmuls (results cleared by the
                    # real group's start=True) while the band/x DMAs stream.
                    for _ in range(32):
                        nc.tensor.matmul(
                            s_m[:, 0:512],
                            warm_in[:, 0:128],
                            warm_in[:],
                            start=True,
                            stop=True,
                        )
                xplus = []
                for t in range(16):
                    # original chunk t -> xtr chunk 31-t ; t+16 -> 15-t
                    xpl = xpp_pool.tile([128, 128], f32, tag="xp", name="xp")
                    nc.gpsimd.tensor_tensor(
                        xpl[:], xbig[:, 31 - t, :], xbig[:, 15 - t, :], ADD
                    )
                    xplus.append(xpl)
                    xm = xmp.tile([128, 128], mmdt, tag="xm", name="xm")
                    nc.vector.tensor_tensor(
                        xm[:], xbig[:, 31 - t, :], xbig[:, 15 - t, :], SUB
                    )
                    q0 = (N - 128) - 128 * t  # in [2048, 3968]
                    for j in range(4):
                        u = q0 - 2048 + 512 * j
                        nc.tensor.matmul(
                            s_m[:, 512 * j : 512 * j + 512],
                            xm[:],
                            band_mh[:, u : u + 512],
                            start=(t == 0),
                            stop=(t == 15),
                        )
                return s_m, xplus

            def emit_level23(bt, xplus):
                s_pm = pp.tile([128, 1024], f32, tag="spm", name="spm")
                s_3p = pp.tile([128, 512], f32, tag="s3p", name="s3p")
                s_3m = pp.tile([128, 512], f32, tag="s3m", name="s3m")
                xpp2 = []
                for t in range(8):
                    xq = xpp_pool.tile([128, 128], f32, tag="xq", name="xq", bufs=10)
                    nc.gpsimd.tensor_tensor(
                        xq[:], xplus[t][:], xplus[t + 8][:], ADD
                    )
                    xpp2.append(xq)
                    xpm = xmp.tile([128, 128], mmdt, tag="xpm", name="xpm")
                    nc.vector.tensor_tensor(
                        xpm[:], xplus[t][:], xplus[t + 8][:], SUB
                    )
                    q0pm = (2048 - 128) - 128 * t  # in [1024, 1920]
                    for j in range(2):
                        u = q0pm - 1024 + 512 * j
                        nc.tensor.matmul(
                            s_pm[:, 512 * j : 512 * j + 512],
                            xpm[:],
                            band_pmh[:, u : u + 512],
                            start=(t == 0),
                            stop=(t == 7),
                        )
                for t in range(4):
                    x3p = xmp.tile([128, 128], mmdt, tag="x3p", name="x3p")
                    nc.vector.tensor_tensor(
                        x3p[:], xpp2[t][:], xpp2[t + 4][:], ADD
                    )
                    x3m = xmp.tile([128, 128], mmdt, tag="x3m", name="x3m")
                    nc.vector.tensor_tensor(
                        x3m[:], xpp2[t][:], xpp2[t + 4][:], SUB
                    )
                    q03p = (512 - 128) - 128 * t   # in [0, 384]
                    q03m = (1024 - 128) - 128 * t  # in [512, 896]
                    nc.tensor.matmul(
                        s_3p[:],
                        x3p[:],
                        band_3ph[:, q03p : q03p + 512],
                        start=(t == 0),
                        stop=(t == 3),
                    )
                    nc.tensor.matmul(
                        s_3m[:],
                        x3m[:],
                        band_3mh[:, q03m - 512 : q03m - 512 + 512],
                        start=(t == 0),
                        stop=(t == 3),
                    )
                return s_pm, s_3p, s_3m

            def emit_copies(s_m, s_pm, s_3p, s_3m):
                # PSUM -> SBUF on ACT with CRT scales folded in; cm first
                # (the next block's first matmuls reuse s_m's banks).
                cm = op.tile([128, 2048], f32, tag="cm", name="cm")
                nc.scalar.mul(cm[:], s_m[:], 0.5)
                c3p = op.tile([128, 512], f32, tag="c3p", name="c3p")
                nc.scalar.mul(c3p[:], s_3p[:], 0.125)
                c3m = op.tile([128, 512], f32, tag="c3m", name="c3m")
                nc.scalar.mul(c3m[:], s_3m[:], 0.125)
                cpm = op.tile([128, 1024], f32, tag="cpm", name="cpm")
                nc.scalar.mul(cpm[:], s_pm[:], 0.25)
                return cm, c3p, c3m, cpm

            def make_unfold(b0, cm, c3p, c3m, cpm):
                def unfold():
                    cpp = op.tile([128, 1024], f32, tag="cpp", name="cpp")
                    nc.vector.tensor_tensor(cpp[:, 0:512], c3p[:], c3m[:], ADD)
                    nc.vector.tensor_tensor(cpp[:, 512:1024], c3p[:], c3m[:], SUB)
                    u1 = op.tile([128, 1024], f32, tag="u1", name="u1")
                    nc.vector.tensor_tensor(u1[:], cpp[:], cpm[:], ADD)
                    u2 = op.tile([128, 1024], f32, tag="u2", name="u2")
                    nc.vector.tensor_tensor(u2[:], cpp[:], cpm[:], SUB)
                    # out[:,   0:1024] = u1 + cm[:, 0:1024]
                    # out[:,1024:2048] = u2 + cm[:, 1024:2048]
                    # out[:,2048:3072] = u1 - cm[:, 0:1024]
                    # out[:,3072:4096] = u2 - cm[:, 1024:2048]
                    for seg, (usrc, moff, alu) in enumerate(
                        ((u1, 0, ADD), (u2, 1024, ADD), (u1, 0, SUB), (u2, 1024, SUB))
                    ):
                        o = op.tile([128, 1024], f32, tag="o", name="o", bufs=4)
                        nc.vector.tensor_tensor(
                            o[:], usrc[:], cm[:, moff : moff + 1024], alu
                        )
                        nc.sync.dma_start(
                            out_d[b0 : b0 + 128, 1024 * seg : 1024 * seg + 1024],
                            o[:],
                        )

                return unfold

            # ---------- band construction + software-pipelined block 0 ----
            with tc.tile_pool(name="scratch", bufs=1) as scr:
                # band2[p, q] = w3[1 + p + q], period N in q -> width 6016.
                W = 6016
                band_wf = scr.tile([128, W], f32)
                srcA = bass_rust.AP(
                    tensor=w3_d[:].tensor,
                    offset=1 + 1920,
                    ap=[[1, 128], [1, W - 1920]],
                )
                nc.sync.dma_start(band_wf[:, 1920:W], srcA)
                srcB = bass_rust.AP(
                    tensor=w3_d[:].tensor, offset=1, ap=[[1, 128], [1, 1920]]
                )
                nc.sync.dma_start(band_wf[:, 0:1920], srcB)

                # block 0 x tiles, quarters in consumption order
                xbig0 = xbigp.tile([128, 32, 128], f32, tag="xbig", name="xbig0")
                nc.sync.dma_start(xbig0[:, 24:32, :], xtr_t[:, 24:32, 0:128])
                nc.sync.dma_start(xbig0[:, 8:16, :], xtr_t[:, 8:16, 0:128])
                nc.sync.dma_start(xbig0[:, 16:24, :], xtr_t[:, 16:24, 0:128])
                nc.sync.dma_start(xbig0[:, 0:8, :], xtr_t[:, 0:8, 0:128])

                # band_mh[:, u] = band2[2048+u] - band2[u]  (band2 period 4096)
                nc.vector.tensor_tensor(
                    band_mh[:, 1920:3968],
                    band_wf[:, 3968 : 3968 + 2048],
                    band_wf[:, 1920 : 1920 + 2048],
                    SUB,
                )
                nc.vector.tensor_tensor(
                    band_mh[:, 0:1920],
                    band_wf[:, 2048 : 2048 + 1920],
                    band_wf[:, 0:1920],
                    SUB,
                )

                # block 0 nega phase overlaps the rest of the band build
                s_m0, xplus0 = emit_nega(0, xbig0)

                # band_p[q] = band2[q] + band2[2048+q]; period 2048
                band_p = scr.tile([128, 2048], f32)
                nc.vector.tensor_tensor(
                    band_p[:], band_wf[:, 0:2048], band_wf[:, 2048:4096], ADD
                )
                # band_pmh[u] = band_p[(1024+u) % 2048] - band_p[u % 2048]
                nc.vector.tensor_tensor(
                    band_pmh[:, 0:1024], band_p[:, 1024:2048], band_p[:, 0:1024], SUB
                )
                nc.vector.tensor_tensor(
                    band_pmh[:, 1024:1920],
                    band_p[:, 0:896],
                    band_p[:, 1024 : 1024 + 896],
                    SUB,
                )
                # band_pp[q] = band_p[q] + band_p[1024+q]; period 1024
                band_pp = scr.tile([128, 1024], f32)
                nc.vector.tensor_tensor(
                    band_pp[:], band_p[:, 0:1024], band_p[:, 1024:2048], ADD
                )
                # band_3ph[q] = band_pp[q % 1024] + band_pp[(q+512) % 1024]
                nc.vector.tensor_tensor(
                    band_3ph[:, 0:512], band_pp[:, 0:512], band_pp[:, 512:1024], ADD
                )
                nc.vector.tensor_tensor(
                    band_3ph[:, 512:896], band_pp[:, 512:896], band_pp[:, 0:384], ADD
                )
                # band_3mh[u] = band_pp[(512+u) % 1024] - band_pp[u % 1024]
                nc.vector.tensor_tensor(
                    band_3mh[:, 0:512], band_pp[:, 512:1024], band_pp[:, 0:512], SUB
                )
                nc.vector.tensor_tensor(
                    band_3mh[:, 512:896], band_pp[:, 0:384], band_pp[:, 512:896], SUB
                )

            def emit_folds_only(xbig):
                xplus = []
                for t in range(16):
                    xpl = xpp_pool.tile([128, 128], f32, tag="xp", name="xp")
                    nc.gpsimd.tensor_tensor(
                        xpl[:], xbig[:, 31 - t, :], xbig[:, 15 - t, :], ADD
                    )
                    xplus.append(xpl)
                return xplus

            def emit_nega_mms(xplus_src, xbig):
                s_m = pp.tile([128, 2048], f32, tag="sm", name="sm")
                for t in range(16):
                    xm = xmp.tile([128, 128], mmdt, tag="xm", name="xm")
                    nc.vector.tensor_tensor(
                        xm[:], xbig[:, 31 - t, :], xbig[:, 15 - t, :], SUB
                    )
                    q0 = (N - 128) - 128 * t
                    for j in range(4):
                        u = q0 - 2048 + 512 * j
                        nc.tensor.matmul(
                            s_m[:, 512 * j : 512 * j + 512],
                            xm[:],
                            band_mh[:, u : u + 512],
                            start=(t == 0),
                            stop=(t == 15),
                        )
                return s_m

            # ---------- main pipeline ----------
            s_pm0, s_3p0, s_3m0 = emit_level23(0, xplus0)
            pending = make_unfold(0, *emit_copies(s_m0, s_pm0, s_3p0, s_3m0))

            for bt in range(1, NB - 1):
                b0 = 128 * bt
                xbig = xbigp.tile([128, 32, 128], f32, tag="xbig", name="xbig")
                nc.sync.dma_start(xbig[:], xtr_t[:, :, b0 : b0 + 128])
                s_m, xplus = emit_nega(bt, xbig)
                s_pm, s_3p, s_3m = emit_level23(bt, xplus)
                copies = emit_copies(s_m, s_pm, s_3p, s_3m)
                pending()
                pending = make_unfold(b0, *copies)

            # last block: level-2/3 first, nega last, so the tail after the
            # final matmul is just cm + the 4 output combines.
            b0 = 128 * (NB - 1)
            xbig = xbigp.tile([128, 32, 128], f32, tag="xbig", name="xbig")
            nc.sync.dma_start(xbig[:], xtr_t[:, :, b0 : b0 + 128])
            xplus = emit_folds_only(xbig)
            s_pm, s_3p, s_3m = emit_level23(NB - 1, xplus)
            c3p = op.tile([128, 512], f32, tag="c3p", name="c3p")
            nc.scalar.mul(c3p[:], s_3p[:], 0.125)
            c3m = op.tile([128, 512], f32, tag="c3m", name="c3m")
            nc.scalar.mul(c3m[:], s_3m[:], 0.125)
            cpm = op.tile([128, 1024], f32, tag="cpm", name="cpm")
            nc.scalar.mul(cpm[:], s_pm[:], 0.25)
            s_m = emit_nega_mms(xplus, xbig)
            pending()
            # u1/u2 computed while the nega matmuls run
            cpp = op.tile([128, 1024], f32, tag="cpp", name="cpp")
            nc.vector.tensor_tensor(cpp[:, 0:512], c3p[:], c3m[:], ADD)
            nc.vector.tensor_tensor(cpp[:, 512:1024], c3p[:], c3m[:], SUB)
            u1 = op.tile([128, 1024], f32, tag="u1", name="u1")
            nc.vector.tensor_tensor(u1[:], cpp[:], cpm[:], ADD)
            u2 = op.tile([128, 1024], f32, tag="u2", name="u2")
            nc.vector.tensor_tensor(u2[:], cpp[:], cpm[:], SUB)
            # cm in PSUM-bank quarters; finals at half width, emitted in
            # bank-completion order so the tail pipelines with the last MMs
            cm = op.tile([128, 2048], f32, tag="cm", name="cm")
            combos = {0: (u1, 0, ADD), 1: (u2, 1024, ADD), 2: (u1, 0, SUB), 3: (u2, 1024, SUB)}
            os_ = {}
            for seg in range(4):
                os_[seg] = op.tile([128, 1024], f32, tag="o", name="o", bufs=4)
            for q in range(4):
                c0 = 512 * q
                nc.scalar.mul(cm[:, c0 : c0 + 512], s_m[:, c0 : c0 + 512], 0.5)
                half = q % 2          # halves within each 1024 cm block
                blk = q // 2          # cm block 0 -> segs 0,2 ; block 1 -> segs 1,3
                for seg in (blk, blk + 2):
                    usrc, moff, alu = combos[seg]
                    h0 = moff + 512 * half
                    o = os_[seg]
                    nc.vector.tensor_tensor(
                        o[:, 512 * half : 512 * half + 512],
                        usrc[:, 512 * half : 512 * half + 512],
                        cm[:, h0 : h0 + 512],
                        alu,
                    )
                    nc.sync.dma_start(
                        out_d[
                            b0 : b0 + 128,
                            1024 * seg + 512 * half : 1024 * seg + 512 * half + 512,
                        ],
                        o[:, 512 * half : 512 * half + 512],
                    )

    nc.compile()
    return nc


def _get_nc():
    if "nc" not in _STATE:
        _STATE["nc"] = _build()
    return _STATE["nc"]


def _prep_inputs(x, w):
    x = np.ascontiguousarray(x, dtype=np.float32)
    w = np.ascontiguousarray(w, dtype=np.float32)
    wrev = np.roll(w[::-1], 1)
    w3 = np.tile(wrev, 3)
    in_maps = []
    for i in range(N_CORES):
        xtr = np.ascontiguousarray(x[i * B_SHARD : (i + 1) * B_SHARD, ::-1].T)
        in_maps.append({"xtr": xtr, "w3": w3})
    return in_maps


def kernel(x, w, _trace=False):
    from concourse.bass_utils import run_bass_kernel_spmd

    nc = _get_nc()
    in_maps = _prep_inputs(x, w)
    res = run_bass_kernel_spmd(nc, in_maps, list(range(N_CORES)), trace=_trace)
    out = np.concatenate([res.results[i]["out"] for i in range(N_CORES)], axis=0)
    if _trace:
        _STATE["last_result"] = res
    return out



# revision 3
# speedup vs baseline: 1.0020x; 1.0020x over previous
"""Circulant matmul for TRN2 v3: CRT + Gauss 3-mult negacyclic splits with
depth-2 Karatsuba on two of the three nega2048 trees, bf16 matmuls,
host-precomputed x-folds and pre-scaled band operators.

out[b, r] = sum_c x[b,c] w[(c-r) mod N] = cyclic_conv(x_row, v), v = roll(rev(w),1).

CRT: z^4096-1 -> leaves nega2048 (A), nega1024 (B), nega512 (C), cyc512 (D).
nega-n via Gauss 3-mult on its [[T,-U],[U,T]] structure:
  P = T(a0+a1), Q = (T+U)a1, R = (U-T)a0, y = [P-Q, P+R].
For A, the P and Q Toeplitz-1024 products are themselves Karatsuba'd into
3 Toeplitz-512 products each (p = T0(v0+v1), top = p + (V-T0)v1,
bot = p + (W-T0)v0); R stays dense -- engine balance: deeper splits save PE
but overload DVE/Pool with combines. PE: 30208 cyc/128-row block.

Host sends bf16: x CRT-leaf chunks [128, 8, 32, 128] (8 MB/core, per-block
contiguous) + pre-scaled band blob (3.1 MB). All CRT/Gauss scales folded
into bands. ACT evacuates shared Kara leaves, DVE does PSUM combines,
Pool does folds fB/fC and all four final output combines.
"""

import sys

sys.path.insert(0, "/opt/trn_rl_repo")

import numpy as np
import ml_dtypes

BF16 = ml_dtypes.bfloat16
N = 4096
B = 8192
N_CORES = 8
B_SHARD = B // N_CORES  # 1024
NB = B_SHARD // 128     # 8 row-tiles per core

# band blob layout: name -> (m, offset); width = 2m-128; consumption order
_BANDS = {
    "QT0": (512, 0), "QV": (512, 896), "QW": (512, 1792),
    "PT0": (512, 2688), "PV": (512, 3584), "PW": (512, 4480),
    "AR": (1024, 5376),
    "D": (512, 7296), "BQ": (512, 8192), "BP": (512, 9088),
    "CQ": (256, 9984), "CP": (256, 10368), "BR": (512, 10752),
    "CR": (256, 11648),
}
WTOT = 12032

_STATE = {}


# ---------------------------------------------------------------- host math
def _sub_symbol(t, m, shift, h):
    c = m - 1
    return t[c + shift - (h - 1) : c + shift + h].copy()


def _band_from_symbol(t, m):
    u = np.arange(2 * m - 128)[None, :]
    p = np.arange(128)[:, None]
    return t[u - p + 127]


def _make_bands(w):
    v = np.roll(w[::-1].astype(np.float64), 1)
    vm = v[:2048] - v[2048:]
    vp = v[:2048] + v[2048:]
    vpm = vp[:1024] - vp[1024:]
    vpp = vp[:1024] + vp[1024:]
    vppm = vpp[:512] - vpp[512:]
    vppp = vpp[:512] + vpp[512:]

    def skew_sym(b):
        n = len(b)
        t = np.empty(2 * n - 1)
        t[n - 1 :] = b
        t[: n - 1] = -b[1:]
        return t

    def cyc_sym(b):
        n = len(b)
        t = np.empty(2 * n - 1)
        t[n - 1 :] = b
        t[: n - 1] = b[1:]
        return t

    blob = np.zeros((128, WTOT), np.float64)

    def put(name, sym):
        m, off = _BANDS[name]
        blob[:, off : off + 2 * m - 128] = _band_from_symbol(sym, m)

    # A = nega2048: T1024-level Gauss operators (scale 0.5 folded in)
    full = skew_sym(vm)
    tT = _sub_symbol(full, 2048, 0, 1024)
    tU = _sub_symbol(full, 2048, 1024, 1024)
    sP = 0.5 * tT
    sQ = 0.5 * (tT + tU)
    sR = 0.5 * (tU - tT)
    for nm, sym in (("P", sP), ("Q", sQ)):
        t0 = _sub_symbol(sym, 1024, 0, 512)
        tV = _sub_symbol(sym, 1024, -512, 512)
        tW = _sub_symbol(sym, 1024, 512, 512)
        put(nm + "T0", t0)
        put(nm + "V", tV - t0)
        put(nm + "W", tW - t0)
    put("AR", sR)

    for nm, bsrc, scale in [("B", vpm, 0.25), ("C", vppm, 0.125)]:
        n = len(bsrc)
        m = n // 2
        fl = skew_sym(bsrc)
        t1 = _sub_symbol(fl, n, 0, m)
        t2 = _sub_symbol(fl, n, m, m)
        put(nm + "P", scale * t1)
        put(nm + "Q", scale * (t1 + t2))
        put(nm + "R", scale * (t2 - t1))
    put("D", 0.125 * cyc_sym(vppp))
    return blob.astype(BF16)


def _fold_x(x_shard):
    """[1024, 4096] f32 -> [128, 8, 32, 128] bf16 chunk-partitioned CRT
    leaves, per-block contiguous."""
    xs = x_shard.astype(np.float32)
    xm = xs[:, :2048] - xs[:, 2048:]
    xp = xs[:, :2048] + xs[:, 2048:]
    xpm = xp[:, :1024] - xp[:, 1024:]
    xpp = xp[:, :1024] + xp[:, 1024:]
    xppm = xpp[:, :512] - xpp[:, 512:]
    xppp = xpp[:, :512] + xpp[:, 512:]
    L = np.concatenate([xm, xpm, xppm, xppp], axis=1)  # [1024, 4096]
    a = L.reshape(1024, 32, 128).transpose(2, 1, 0)  # [128, 32, 1024]
    a = a.reshape(128, 32, 8, 128).transpose(0, 2, 1, 3)
    return np.ascontiguousarray(a.astype(BF16))


# ---------------------------------------------------------------- device
def _build():
    import concourse.bacc as bacc
    import concourse.mybir as mybir
    import concourse.tile as tile

    f32 = mybir.dt.float32
    bf16 = mybir.dt.bfloat16
    ADD = mybir.AluOpType.add
    SUB = mybir.AluOpType.subtract

    nc = bacc.Bacc("TRN2", target_bir_lowering=False, debug=False)
    xl_d = nc.declare_dram_parameter("xl", [128, 32 * B_SHARD], bf16, isOutput=False)
    bands_d = nc.declare_dram_parameter("bands", [128, WTOT], bf16, isOutput=False)
    out_d = nc.declare_dram_parameter("out", [B_SHARD, N], f32, isOutput=True)

    xl_t = xl_d[:].rearrange("p (k a b) -> p k a b", k=NB, a=32)  # [128, 8, 32, 128]

    with tile.TileContext(nc) as tc:
        with (
            tc.tile_pool(name="const", bufs=1) as constp,
            tc.tile_pool(name="xl", bufs=2) as xlp,
            tc.tile_pool(name="fold", bufs=3) as fp,
            tc.tile_pool(name="yn", bufs=3) as ynp,
            tc.tile_pool(name="mid", bufs=3) as midp,
            tc.tile_pool(name="outp", bufs=3) as op,
            tc.tile_pool(name="ps", bufs=1, space="PSUM") as pp,
        ):
            band = constp.tile([128, WTOT], bf16)
            warm = constp.tile([128, 512], bf16, name="warm")
            nc.gpsimd.memset(warm[:], 0.0)

            def bsl(name, lo, width):
                off = _BANDS[name][1]
                return band[:, off + lo : off + lo + width]

            # first-block DMAs in consumption order; xl(1) last
            xl0 = xlp.tile([128, 32, 128], bf16, tag="xl", name="xl0")
            nc.sync.dma_start(xl0[:, 8:16, :], xl_t[:, 0, 8:16, :])
            nc.sync.dma_start(band[:, 0:896], bands_d[:, 0:896])        # QT0 band
            nc.sync.dma_start(xl0[:, 0:8, :], xl_t[:, 0, 0:8, :])
            nc.sync.dma_start(band[:, 896:2688], bands_d[:, 896:2688])  # QV/QW
            nc.sync.dma_start(band[:, 2688:5376], bands_d[:, 2688:5376])  # P bands
            nc.sync.dma_start(band[:, 5376:7296], bands_d[:, 5376:7296])  # AR band
            nc.sync.dma_start(xl0[:, 16:32, :], xl_t[:, 0, 16:32, :])
            nc.sync.dma_start(band[:, 7296:WTOT], bands_d[:, 7296:WTOT])  # D/B/C
            xl1 = xlp.tile([128, 32, 128], bf16, tag="xl", name="xl1")
            nc.sync.dma_start(xl1[:], xl_t[:, 1, :, :])

            def t1024(psum, bname, src):
                for j in range(2):
                    for t in range(8):
                        nc.tensor.matmul(
                            psum[:, 512 * j : 512 * j + 512],
                            src(t),
                            bsl(bname, (7 - t) * 128 + 512 * j, 512),
                            start=(t == 0),
                            stop=(t == 7),
                        )

            def t512(psum, bname, src):
                for t in range(4):
                    nc.tensor.matmul(
                        psum[:],
                        src(t),
                        bsl(bname, (3 - t) * 128, 512),
                        start=(t == 0),
                        stop=(t == 3),
                    )

            def t256(psum, bname, src):
                for t in range(2):
                    nc.tensor.matmul(
                        psum[:],
                        src(t),
                        bsl(bname, (1 - t) * 128, 256),
                        start=(t == 0),
                        stop=(t == 1),
                    )

            def emit_folds(xl):
                vQ01 = fp.tile([128, 4, 128], bf16, tag="vQ01", name="vQ01")
                nc.vector.tensor_tensor(vQ01[:], xl[:, 8:12, :], xl[:, 12:16, :], ADD)
                fA = fp.tile([128, 8, 128], bf16, tag="fA", name="fA")
                nc.vector.tensor_tensor(fA[:], xl[:, 0:8, :], xl[:, 8:16, :], ADD)
                vP01 = fp.tile([128, 4, 128], bf16, tag="vP01", name="vP01")
                nc.vector.tensor_tensor(vP01[:], fA[:, 0:4, :], fA[:, 4:8, :], ADD)
                fC = fp.tile([128, 2, 128], bf16, tag="fC", name="fC")
                nc.gpsimd.tensor_tensor(fC[:], xl[:, 24:26, :], xl[:, 26:28, :], ADD)
                fB = fp.tile([128, 4, 128], bf16, tag="fB", name="fB")
                nc.gpsimd.tensor_tensor(fB[:], xl[:, 16:20, :], xl[:, 20:24, :], ADD)
                return vQ01, fA, vP01, fB, fC

            def emit_tree(bt, tag, bpfx, vp01, v0src, v1src, warmup=False):
                """Depth-2 Karatsuba Toeplitz-1024 tree -> tX [128,1024] bf16."""
                TL = pp.tile([128, 1536], f32, tag=tag, name=tag + "L")
                if warmup:
                    for _ in range(4):
                        nc.tensor.matmul(
                            TL[:, 1024:1536], warm[:, 0:128], warm[:],
                            start=True, stop=True,
                        )
                t512(TL[:, 0:512], bpfx + "T0", lambda t: vp01[:, t, :])
                t512(TL[:, 512:1024], bpfx + "V", v1src)
                sXp = midp.tile([128, 512], bf16, tag="s" + bpfx, name="s" + bpfx)
                nc.scalar.copy(sXp[:], TL[:, 0:512])
                tX = midp.tile([128, 1024], bf16, tag="t" + bpfx, name="t" + bpfx)
                nc.vector.tensor_tensor(tX[:, 0:512], sXp[:], TL[:, 512:1024], ADD)
                t512(TL[:, 1024:1536], bpfx + "W", v0src)
                nc.vector.tensor_tensor(tX[:, 512:1024], sXp[:], TL[:, 1024:1536], ADD)
                return tX

            def emit_C(xl, fC):
                TCg = pp.tile([128, 768], f32, tag="TC", name="TC")
                t256(TCg[:, 0:256], "CQ", lambda t: xl[:, 26 + t, :])
                t256(TCg[:, 256:512], "CP", lambda t: fC[:, t, :])
                sCP = midp.tile([128, 256], bf16, tag="sCP", name="sCP")
                nc.scalar.copy(sCP[:], TCg[:, 256:512])
                ynC = ynp.tile([128, 512], bf16, tag="ynC", name="ynC")
                nc.vector.tensor_tensor(ynC[:, 0:256], sCP[:], TCg[:, 0:256], SUB)
                t256(TCg[:, 512:768], "CR", lambda t: xl[:, 24 + t, :])
                nc.vector.tensor_tensor(ynC[:, 256:512], sCP[:], TCg[:, 512:768], ADD)
                return ynC

            def emit_B(xl, fB):
                T2g = pp.tile([128, 1536], f32, tag="T2", name="T2B")
                t512(T2g[:, 0:512], "BQ", lambda t: xl[:, 20 + t, :])
                t512(T2g[:, 512:1024], "BP", lambda t: fB[:, t, :])
                sBP = midp.tile([128, 512], bf16, tag="sBP", name="sBP")
                nc.scalar.copy(sBP[:], T2g[:, 512:1024])
                ynB = ynp.tile([128, 1024], bf16, tag="ynB", name="ynB")
                nc.vector.tensor_tensor(ynB[:, 0:512], sBP[:], T2g[:, 0:512], SUB)
                t512(T2g[:, 1024:1536], "BR", lambda t: xl[:, 16 + t, :])
                nc.vector.tensor_tensor(ynB[:, 512:1024], sBP[:], T2g[:, 1024:1536], ADD)
                return ynB

            def emit_D(psum, xl):
                # D shares the R generation's T1 tile (banks are disjoint);
                # ACT evacuates so cpp is SBUF-only and nothing D-related
                # gates the next block's T1 reuse
                t512(psum, "D", lambda t: xl[:, 28 + t, :])
                sD = midp.tile([128, 512], bf16, tag="sD", name="sD")
                nc.scalar.copy(sD[:], psum)
                return sD

            def emit_unfold_mid(TD, ynB, ynC):
                cpp = midp.tile([128, 1024], bf16, tag="cpp", name="cpp")
                nc.vector.tensor_tensor(cpp[:, 0:512], TD[:], ynC[:], ADD)
                nc.vector.tensor_tensor(cpp[:, 512:1024], TD[:], ynC[:], SUB)
                u1 = midp.tile([128, 1024], bf16, tag="u1", name="u1")
                nc.vector.tensor_tensor(u1[:], cpp[:], ynB[:], ADD)
                u2 = midp.tile([128, 1024], bf16, tag="u2", name="u2")
                nc.vector.tensor_tensor(u2[:], cpp[:], ynB[:], SUB)
                return u1, u2

            def emit_finals(b0, u1, u2, ynA, dve_share=False):
                # Pool by default; DVE takes half in the drain-critical blocks,
                # which also get per-1024 DMAs for finer tail interleave
                if dve_share:
                    for seg, usrc, alu, eng in (
                        (0, u1, ADD, nc.vector), (1, u2, ADD, nc.gpsimd),
                        (2, u1, SUB, nc.vector), (3, u2, SUB, nc.gpsimd),
                    ):
                        o = op.tile([128, 1024], f32, tag="o", name="o", bufs=4)
                        eng.tensor_tensor(
                            o[:], usrc[:], ynA[:, 1024 * (seg % 2) : 1024 * (seg % 2) + 1024], alu
                        )
                        nc.sync.dma_start(
                            out_d[b0 : b0 + 128, 1024 * seg : 1024 * seg + 1024], o[:]
                        )
                    return
                oL = op.tile([128, 2048], f32, tag="oL", name="oL")
                nc.gpsimd.tensor_tensor(oL[:, 0:1024], u1[:], ynA[:, 0:1024], ADD)
                nc.gpsimd.tensor_tensor(oL[:, 1024:2048], u2[:], ynA[:, 1024:2048], ADD)
                nc.sync.dma_start(out_d[b0 : b0 + 128, 0:2048], oL[:])
                oR = op.tile([128, 2048], f32, tag="oR", name="oR")
                nc.gpsimd.tensor_tensor(oR[:, 0:1024], u1[:], ynA[:, 0:1024], SUB)
                nc.gpsimd.tensor_tensor(oR[:, 1024:2048], u2[:], ynA[:, 1024:2048], SUB)
                nc.sync.dma_start(out_d[b0 : b0 + 128, 2048:4096], oR[:])

            pending = None
            xls = {0: xl0, 1: xl1}
            folds_cur = emit_folds(xl0)
            for bt in range(NB):
                b0 = 128 * bt
                xl = xls[bt]
                vQ01, fA, vP01, fB, fC = folds_cur
                last = bt == NB - 1
                if not last:
                    if pending is not None:
                        pending()
                    tQ = emit_tree(
                        bt, "T1", "Q", vQ01,
                        lambda t: xl[:, 8 + t, :], lambda t: xl[:, 12 + t, :],
                        warmup=(bt == 0),
                    )
                    tP = emit_tree(
                        bt, "T2", "P", vP01,
                        lambda t: fA[:, t, :], lambda t: fA[:, 4 + t, :],
                    )
                    ynA = ynp.tile([128, 2048], bf16, tag="ynA", name="ynA")
                    nc.vector.tensor_tensor(ynA[:, 0:1024], tP[:], tQ[:], SUB)
                    TRp = pp.tile([128, 1536], f32, tag="T1", name="T1RD")
                    t1024(TRp[:, 0:1024], "AR", lambda t: xl[:, t, :])
                    for h in (0, 1):
                        nc.vector.tensor_tensor(
                            ynA[:, 1024 + 512 * h : 1536 + 512 * h],
                            tP[:, 512 * h : 512 * h + 512],
                            TRp[:, 512 * h : 512 * h + 512],
                            ADD,
                        )
                    sD = emit_D(TRp[:, 1024:1536], xl)
                    # next block's input + folds right behind the hi-halves in
                    # the DVE queue so block bt+1's first group is never gated
                    if bt + 2 < NB:
                        xls[bt + 2] = xlp.tile([128, 32, 128], bf16, tag="xl", name="xl")
                        nc.sync.dma_start(xls[bt + 2][:], xl_t[:, bt + 2, :, :])
                    folds_cur = emit_folds(xls[bt + 1])
                    ynB = emit_B(xl, fB)
                    ynC = emit_C(xl, fC)
                    cpp = midp.tile([128, 1024], bf16, tag="cpp", name="cpp")
                    nc.vector.tensor_tensor(cpp[:, 0:512], sD[:], ynC[:], ADD)
                    nc.vector.tensor_tensor(cpp[:, 512:1024], sD[:], ynC[:], SUB)
                    u1 = midp.tile([128, 1024], bf16, tag="u1", name="u1")
                    nc.vector.tensor_tensor(u1[:], cpp[:], ynB[:], ADD)
                    u2 = midp.tile([128, 1024], bf16, tag="u2", name="u2")
                    nc.vector.tensor_tensor(u2[:], cpp[:], ynB[:], SUB)
                    dve_share = bt == NB - 2
                    pending = (
                        lambda b0=b0, u1=u1, u2=u2, ynA=ynA, d=dve_share: emit_finals(
                            b0, u1, u2, ynA, dve_share=d
                        )
                    )
                else:
                    # last block: C/B/D first, then Q/P trees, R last; finals
                    # for the y_lo half stream during R, y_hi half-granular
                    if pending is not None:
                        pending()
                        pending = None
                    ynC = emit_C(xl, fC)
                    ynB = emit_B(xl, fB)
                    TDg = pp.tile([128, 1536], f32, tag="T1", name="T1D")
                    sD = emit_D(TDg[:, 0:512], xl)
                    cpp = midp.tile([128, 1024], bf16, tag="cpp", name="cpp")
                    nc.vector.tensor_tensor(cpp[:, 0:512], sD[:], ynC[:], ADD)
                    nc.vector.tensor_tensor(cpp[:, 512:1024], sD[:], ynC[:], SUB)
                    u1 = midp.tile([128, 1024], bf16, tag="u1", name="u1")
                    nc.vector.tensor_tensor(u1[:], cpp[:], ynB[:], ADD)
                    u2 = midp.tile([128, 1024], bf16, tag="u2", name="u2")
                    nc.vector.tensor_tensor(u2[:], cpp[:], ynB[:], SUB)
                    tQ = emit_tree(
                        bt, "T1", "Q", vQ01,
                        lambda t: xl[:, 8 + t, :], lambda t: xl[:, 12 + t, :],
                    )
                    tP = emit_tree(
                        bt, "T2", "P", vP01,
                        lambda t: fA[:, t, :], lambda t: fA[:, 4 + t, :],
                    )
                    ynA = ynp.tile([128, 2048], bf16, tag="ynA", name="ynA")
                    nc.vector.tensor_tensor(ynA[:, 0:1024], tP[:], tQ[:], SUB)
                    for seg, alu, eng in ((0, ADD, "v"), (2, SUB, "p")):
                        o = op.tile([128, 1024], f32, tag="o", name="o", bufs=4)
                        if eng == "v":
                            nc.vector.tensor_tensor(o[:], u1[:], ynA[:, 0:1024], alu)
                        else:
                            nc.gpsimd.tensor_tensor(o[:], u1[:], ynA[:, 0:1024], alu)
                        nc.sync.dma_start(
                            out_d[b0 : b0 + 128, 1024 * seg : 1024 * seg + 1024], o[:]
                        )
                    TRp = pp.tile([128, 1536], f32, tag="T2", name="T2R")
                    t1024(TRp[:, 0:1024], "AR", lambda t: xl[:, t, :])
                    for h in (0, 1):
                        nc.vector.tensor_tensor(
                            ynA[:, 1024 + 512 * h : 1536 + 512 * h],
                            tP[:, 512 * h : 512 * h + 512],
                            TRp[:, 512 * h : 512 * h + 512],
                            ADD,
                        )
                        for seg, alu, eng in ((1, ADD, "v"), (3, SUB, "p")):
                            o = op.tile([128, 512], f32, tag="oh", name="oh", bufs=4)
                            args = (
                                o[:],
                                u2[:, 512 * h : 512 * h + 512],
                                ynA[:, 1024 + 512 * h : 1536 + 512 * h],
                                alu,
                            )
                            if eng == "v":
                                nc.vector.tensor_tensor(*args)
                            else:
                                nc.gpsimd.tensor_tensor(*args)
                            nc.sync.dma_start(
                                out_d[
                                    b0 : b0 + 128,
                                    1024 * seg + 512 * h : 1024 * seg + 512 * h + 512,
                                ],
                                o[:],
                            )
            if pending is not None:
                pending()

    nc.compile()
    return nc


def _get_nc():
    if "nc" not in _STATE:
        _STATE["nc"] = _build()
    return _STATE["nc"]


def _prep_inputs(x, w):
    x = np.ascontiguousarray(x, dtype=np.float32)
    w = np.asarray(w, dtype=np.float32)
    bands = _make_bands(w)
    in_maps = []
    for i in range(N_CORES):
        xl = _fold_x(x[i * B_SHARD : (i + 1) * B_SHARD])
        in_maps.append({"xl": xl.reshape(128, 32 * B_SHARD), "bands": bands})
    return in_maps


def kernel(x, w, _trace=False):
    from concourse.bass_utils import run_bass_kernel_spmd

    nc = _get_nc()
    in_maps = _prep_inputs(x, w)
    res = run_bass_kernel_spmd(nc, in_maps, list(range(N_CORES)), trace=_trace)
    out = np.concatenate([res.results[i]["out"] for i in range(N_CORES)], axis=0)
    if _trace:
        _STATE["last_result"] = res
    return out


# revision 5
# speedup vs baseline: 1.0085x; 1.0064x over previous
"""Circulant matmul for TRN2 v3: CRT + Gauss 3-mult negacyclic splits with
depth-2 Karatsuba on two of the three nega2048 trees, bf16 matmuls,
host-precomputed x-folds and pre-scaled band operators.

out[b, r] = sum_c x[b,c] w[(c-r) mod N] = cyclic_conv(x_row, v), v = roll(rev(w),1).

CRT: z^4096-1 -> leaves nega2048 (A), nega1024 (B), nega512 (C), cyc512 (D).
nega-n via Gauss 3-mult on its [[T,-U],[U,T]] structure:
  P = T(a0+a1), Q = (T+U)a1, R = (U-T)a0, y = [P-Q, P+R].
For A, the P and Q Toeplitz-1024 products are themselves Karatsuba'd into
3 Toeplitz-512 products each (p = T0(v0+v1), top = p + (V-T0)v1,
bot = p + (W-T0)v0); R stays dense -- engine balance: deeper splits save PE
but overload DVE/Pool with combines. PE: 30208 cyc/128-row block.

Host sends bf16: x CRT-leaf chunks [128, 8, 32, 128] (8 MB/core, per-block
contiguous) + pre-scaled band blob (3.1 MB). All CRT/Gauss scales folded
into bands. ACT evacuates shared Kara leaves + cyc512, DVE does PSUM
combines, Pool does folds fB/fC and most final output combines. PSUM map:
T1 (Q-leaves -> R||cyc512), T2 (P-leaves -> B-leaves), TC (nega512) --
ordered so no combine chain ever gates the next block's first matmul
group; per-block PE order Q, P, R, D, B, C. The last two blocks split
finals across DVE/Pool at fine granularity so the tail output DMAs
overlap the final A-tree matmul stream.
"""

import sys

sys.path.insert(0, "/opt/trn_rl_repo")

import numpy as np
import ml_dtypes

BF16 = ml_dtypes.bfloat16
N = 4096
B = 8192
N_CORES = 8
B_SHARD = B // N_CORES  # 1024
NB = B_SHARD // 128     # 8 row-tiles per core

# band blob layout: name -> (m, offset); width = 2m-128; consumption order
_BANDS = {
    "QT0": (512, 0), "QV": (512, 896), "QW": (512, 1792),
    "PT0": (512, 2688), "PV": (512, 3584), "PW": (512, 4480),
    "AR": (1024, 5376),
    "D": (512, 7296), "BQ": (512, 8192), "BP": (512, 9088),
    "CQ": (256, 9984), "CP": (256, 10368), "BR": (512, 10752),
    "CR": (256, 11648),
}
WTOT = 12032

_STATE = {}


# ---------------------------------------------------------------- host math
def _sub_symbol(t, m, shift, h):
    c = m - 1
    return t[c + shift - (h - 1) : c + shift + h].copy()


def _band_from_symbol(t, m):
    u = np.arange(2 * m - 128)[None, :]
    p = np.arange(128)[:, None]
    return t[u - p + 127]


def _make_bands(w):
    v = np.roll(w[::-1].astype(np.float64), 1)
    vm = v[:2048] - v[2048:]
    vp = v[:2048] + v[2048:]
    vpm = vp[:1024] - vp[1024:]
    vpp = vp[:1024] + vp[1024:]
    vppm = vpp[:512] - vpp[512:]
    vppp = vpp[:512] + vpp[512:]

    def skew_sym(b):
        n = len(b)
        t = np.empty(2 * n - 1)
        t[n - 1 :] = b
        t[: n - 1] = -b[1:]
        return t

    def cyc_sym(b):
        n = len(b)
        t = np.empty(2 * n - 1)
        t[n - 1 :] = b
        t[: n - 1] = b[1:]
        return t

    blob = np.zeros((128, WTOT), np.float64)

    def put(name, sym):
        m, off = _BANDS[name]
        blob[:, off : off + 2 * m - 128] = _band_from_symbol(sym, m)

    # A = nega2048: T1024-level Gauss operators (scale 0.5 folded in)
    full = skew_sym(vm)
    tT = _sub_symbol(full, 2048, 0, 1024)
    tU = _sub_symbol(full, 2048, 1024, 1024)
    sP = 0.5 * tT
    sQ = 0.5 * (tT + tU)
    sR = 0.5 * (tU - tT)
    for nm, sym in (("P", sP), ("Q", sQ)):
        t0 = _sub_symbol(sym, 1024, 0, 512)
        tV = _sub_symbol(sym, 1024, -512, 512)
        tW = _sub_symbol(sym, 1024, 512, 512)
        put(nm + "T0", t0)
        put(nm + "V", tV - t0)
        put(nm + "W", tW - t0)
    put("AR", sR)

    for nm, bsrc, scale in [("B", vpm, 0.25), ("C", vppm, 0.125)]:
        n = len(bsrc)
        m = n // 2
        fl = skew_sym(bsrc)
        t1 = _sub_symbol(fl, n, 0, m)
        t2 = _sub_symbol(fl, n, m, m)
        put(nm + "P", scale * t1)
        put(nm + "Q", scale * (t1 + t2))
        put(nm + "R", scale * (t2 - t1))
    put("D", 0.125 * cyc_sym(vppp))
    return blob.astype(BF16)


def _fold_x(x_shard):
    """[1024, 4096] f32 -> [128, 8, 32, 128] bf16 chunk-partitioned CRT
    leaves, per-block contiguous."""
    xs = x_shard.astype(np.float32)
    xm = xs[:, :2048] - xs[:, 2048:]
    xp = xs[:, :2048] + xs[:, 2048:]
    xpm = xp[:, :1024] - xp[:, 1024:]
    xpp = xp[:, :1024] + xp[:, 1024:]
    xppm = xpp[:, :512] - xpp[:, 512:]
    xppp = xpp[:, :512] + xpp[:, 512:]
    L = np.concatenate([xm, xpm, xppm, xppp], axis=1)  # [1024, 4096]
    a = L.reshape(1024, 32, 128).transpose(2, 1, 0)  # [128, 32, 1024]
    a = a.reshape(128, 32, 8, 128).transpose(0, 2, 1, 3)
    return np.ascontiguousarray(a.astype(BF16))


# ---------------------------------------------------------------- device
def _build():
    import concourse.bacc as bacc
    import concourse.mybir as mybir
    import concourse.tile as tile

    f32 = mybir.dt.float32
    bf16 = mybir.dt.bfloat16
    ADD = mybir.AluOpType.add
    SUB = mybir.AluOpType.subtract

    nc = bacc.Bacc("TRN2", target_bir_lowering=False, debug=False)
    xl_d = nc.declare_dram_parameter("xl", [128, 32 * B_SHARD], bf16, isOutput=False)
    bands_d = nc.declare_dram_parameter("bands", [128, WTOT], bf16, isOutput=False)
    out_d = nc.declare_dram_parameter("out", [B_SHARD, N], f32, isOutput=True)

    xl_t = xl_d[:].rearrange("p (k a b) -> p k a b", k=NB, a=32)  # [128, 8, 32, 128]

    with tile.TileContext(nc) as tc:
        with (
            tc.tile_pool(name="const", bufs=1) as constp,
            tc.tile_pool(name="xl", bufs=2) as xlp,
            tc.tile_pool(name="fold", bufs=3) as fp,
            tc.tile_pool(name="yn", bufs=3) as ynp,
            tc.tile_pool(name="mid", bufs=3) as midp,
            tc.tile_pool(name="outp", bufs=3) as op,
            tc.tile_pool(name="ps", bufs=1, space="PSUM") as pp,
        ):
            band = constp.tile([128, WTOT], bf16)
            warm = constp.tile([128, 512], bf16, name="warm")
            nc.gpsimd.memset(warm[:], 0.0)

            def bsl(name, lo, width):
                off = _BANDS[name][1]
                return band[:, off + lo : off + lo + width]

            # first-block DMAs in consumption order; xl(1) last
            xl0 = xlp.tile([128, 32, 128], bf16, tag="xl", name="xl0")
            nc.sync.dma_start(xl0[:, 8:16, :], xl_t[:, 0, 8:16, :])
            nc.sync.dma_start(band[:, 0:896], bands_d[:, 0:896])        # QT0 band
            nc.sync.dma_start(xl0[:, 0:8, :], xl_t[:, 0, 0:8, :])
            nc.sync.dma_start(band[:, 896:2688], bands_d[:, 896:2688])  # QV/QW
            nc.sync.dma_start(band[:, 2688:5376], bands_d[:, 2688:5376])  # P bands
            nc.sync.dma_start(band[:, 5376:7296], bands_d[:, 5376:7296])  # AR band
            nc.sync.dma_start(xl0[:, 16:32, :], xl_t[:, 0, 16:32, :])
            nc.sync.dma_start(band[:, 7296:WTOT], bands_d[:, 7296:WTOT])  # D/B/C
            xl1 = xlp.tile([128, 32, 128], bf16, tag="xl", name="xl1")
            nc.sync.dma_start(xl1[:], xl_t[:, 1, :, :])

            def t1024(psum, bname, src):
                for j in range(2):
                    for t in range(8):
                        nc.tensor.matmul(
                            psum[:, 512 * j : 512 * j + 512],
                            src(t),
                            bsl(bname, (7 - t) * 128 + 512 * j, 512),
                            start=(t == 0),
                            stop=(t == 7),
                        )

            def t512(psum, bname, src):
                for t in range(4):
                    nc.tensor.matmul(
                        psum[:],
                        src(t),
                        bsl(bname, (3 - t) * 128, 512),
                        start=(t == 0),
                        stop=(t == 3),
                    )

            def t256(psum, bname, src):
                for t in range(2):
                    nc.tensor.matmul(
                        psum[:],
                        src(t),
                        bsl(bname, (1 - t) * 128, 256),
                        start=(t == 0),
                        stop=(t == 1),
                    )

            def emit_folds(xl):
                vQ01 = fp.tile([128, 4, 128], bf16, tag="vQ01", name="vQ01")
                nc.vector.tensor_tensor(vQ01[:], xl[:, 8:12, :], xl[:, 12:16, :], ADD)
                fA = fp.tile([128, 8, 128], bf16, tag="fA", name="fA")
                nc.vector.tensor_tensor(fA[:], xl[:, 0:8, :], xl[:, 8:16, :], ADD)
                vP01 = fp.tile([128, 4, 128], bf16, tag="vP01", name="vP01")
                nc.vector.tensor_tensor(vP01[:], fA[:, 0:4, :], fA[:, 4:8, :], ADD)
                fC = fp.tile([128, 2, 128], bf16, tag="fC", name="fC")
                nc.gpsimd.tensor_tensor(fC[:], xl[:, 24:26, :], xl[:, 26:28, :], ADD)
                fB = fp.tile([128, 4, 128], bf16, tag="fB", name="fB")
                nc.gpsimd.tensor_tensor(fB[:], xl[:, 16:20, :], xl[:, 20:24, :], ADD)
                return vQ01, fA, vP01, fB, fC

            def emit_tree(bt, tag, bpfx, vp01, v0src, v1src, warmup=False):
                """Depth-2 Karatsuba Toeplitz-1024 tree -> tX [128,1024] bf16."""
                TL = pp.tile([128, 1536], f32, tag=tag, name=tag + "L")
                if warmup:
                    for _ in range(4):
                        nc.tensor.matmul(
                            TL[:, 1024:1536], warm[:, 0:128], warm[:],
                            start=True, stop=True,
                        )
                t512(TL[:, 0:512], bpfx + "T0", lambda t: vp01[:, t, :])
                t512(TL[:, 512:1024], bpfx + "V", v1src)
                sXp = midp.tile([128, 512], bf16, tag="s" + bpfx, name="s" + bpfx)
                nc.scalar.copy(sXp[:], TL[:, 0:512])
                tX = midp.tile([128, 1024], bf16, tag="t" + bpfx, name="t" + bpfx)
                nc.vector.tensor_tensor(tX[:, 0:512], sXp[:], TL[:, 512:1024], ADD)
                t512(TL[:, 1024:1536], bpfx + "W", v0src)
                nc.vector.tensor_tensor(tX[:, 512:1024], sXp[:], TL[:, 1024:1536], ADD)
                return tX

            def emit_C(xl, fC):
                TCg = pp.tile([128, 768], f32, tag="TC", name="TC")
                t256(TCg[:, 0:256], "CQ", lambda t: xl[:, 26 + t, :])
                t256(TCg[:, 256:512], "CP", lambda t: fC[:, t, :])
                sCP = midp.tile([128, 256], bf16, tag="sCP", name="sCP")
                nc.scalar.copy(sCP[:], TCg[:, 256:512])
                ynC = ynp.tile([128, 512], bf16, tag="ynC", name="ynC")
                nc.vector.tensor_tensor(ynC[:, 0:256], sCP[:], TCg[:, 0:256], SUB)
                t256(TCg[:, 512:768], "CR", lambda t: xl[:, 24 + t, :])
                nc.vector.tensor_tensor(ynC[:, 256:512], sCP[:], TCg[:, 512:768], ADD)
                return ynC

            def emit_B(xl, fB):
                T2g = pp.tile([128, 1536], f32, tag="T2", name="T2B")
                t512(T2g[:, 0:512], "BQ", lambda t: xl[:, 20 + t, :])
                t512(T2g[:, 512:1024], "BP", lambda t: fB[:, t, :])
                sBP = midp.tile([128, 512], bf16, tag="sBP", name="sBP")
                nc.scalar.copy(sBP[:], T2g[:, 512:1024])
                ynB = ynp.tile([128, 1024], bf16, tag="ynB", name="ynB")
                nc.vector.tensor_tensor(ynB[:, 0:512], sBP[:], T2g[:, 0:512], SUB)
                t512(T2g[:, 1024:1536], "BR", lambda t: xl[:, 16 + t, :])
                nc.vector.tensor_tensor(ynB[:, 512:1024], sBP[:], T2g[:, 1024:1536], ADD)
                return ynB

            def emit_D(psum, xl):
                # D shares the R generation's T1 tile (banks are disjoint);
                # ACT evacuates so cpp is SBUF-only and nothing D-related
                # gates the next block's T1 reuse
                t512(psum, "D", lambda t: xl[:, 28 + t, :])
                sD = midp.tile([128, 512], bf16, tag="sD", name="sD")
                nc.scalar.copy(sD[:], psum)
                return sD

            def emit_unfold_mid(TD, ynB, ynC):
                cpp = midp.tile([128, 1024], bf16, tag="cpp", name="cpp")
                nc.vector.tensor_tensor(cpp[:, 0:512], TD[:], ynC[:], ADD)
                nc.vector.tensor_tensor(cpp[:, 512:1024], TD[:], ynC[:], SUB)
                u1 = midp.tile([128, 1024], bf16, tag="u1", name="u1")
                nc.vector.tensor_tensor(u1[:], cpp[:], ynB[:], ADD)
                u2 = midp.tile([128, 1024], bf16, tag="u2", name="u2")
                nc.vector.tensor_tensor(u2[:], cpp[:], ynB[:], SUB)
                return u1, u2

            def emit_finals(b0, u1, u2, ynA, dve_share=False):
                # Pool by default; DVE takes half in the drain-critical blocks,
                # which also get per-1024 DMAs for finer tail interleave
                if dve_share:
                    for seg, usrc, alu, eng in (
                        (0, u1, ADD, nc.gpsimd), (1, u2, ADD, nc.gpsimd),
                        (2, u1, SUB, nc.gpsimd), (3, u2, SUB, nc.gpsimd),
                    ):
                        o = op.tile([128, 1024], f32, tag="o", name="o", bufs=4)
                        eng.tensor_tensor(
                            o[:], usrc[:], ynA[:, 1024 * (seg % 2) : 1024 * (seg % 2) + 1024], alu
                        )
                        nc.sync.dma_start(
                            out_d[b0 : b0 + 128, 1024 * seg : 1024 * seg + 1024], o[:]
                        )
                    return
                oL = op.tile([128, 2048], f32, tag="oL", name="oL")
                nc.gpsimd.tensor_tensor(oL[:, 0:1024], u1[:], ynA[:, 0:1024], ADD)
                nc.gpsimd.tensor_tensor(oL[:, 1024:2048], u2[:], ynA[:, 1024:2048], ADD)
                nc.sync.dma_start(out_d[b0 : b0 + 128, 0:2048], oL[:])
                oR = op.tile([128, 2048], f32, tag="oR", name="oR")
                nc.gpsimd.tensor_tensor(oR[:, 0:1024], u1[:], ynA[:, 0:1024], SUB)
                nc.gpsimd.tensor_tensor(oR[:, 1024:2048], u2[:], ynA[:, 1024:2048], SUB)
                nc.sync.dma_start(out_d[b0 : b0 + 128, 2048:4096], oR[:])

            pending = None
            xls = {0: xl0, 1: xl1}
            folds_cur = emit_folds(xl0)
            for bt in range(NB):
                b0 = 128 * bt
                xl = xls[bt]
                vQ01, fA, vP01, fB, fC = folds_cur
                last = bt == NB - 1
                if not last:
                    if pending is not None:
                        pending()
                    tQ = emit_tree(
                        bt, "T1", "Q", vQ01,
                        lambda t: xl[:, 8 + t, :], lambda t: xl[:, 12 + t, :],
                        warmup=(bt == 0),
                    )
                    tP = emit_tree(
                        bt, "T2", "P", vP01,
                        lambda t: fA[:, t, :], lambda t: fA[:, 4 + t, :],
                    )
                    ynA = ynp.tile([128, 2048], bf16, tag="ynA", name="ynA")
                    nc.vector.tensor_tensor(ynA[:, 0:1024], tP[:], tQ[:], SUB)
                    TRp = pp.tile([128, 1536], f32, tag="T1", name="T1RD")
                    t1024(TRp[:, 0:1024], "AR", lambda t: xl[:, t, :])
                    for h in (0, 1):
                        nc.vector.tensor_tensor(
                            ynA[:, 1024 + 512 * h : 1536 + 512 * h],
                            tP[:, 512 * h : 512 * h + 512],
                            TRp[:, 512 * h : 512 * h + 512],
                            ADD,
                        )
                    sD = emit_D(TRp[:, 1024:1536], xl)
                    # next block's input + folds right behind the hi-halves in
                    # the DVE queue so block bt+1's first group is never gated
                    if bt + 2 < NB:
                        xls[bt + 2] = xlp.tile([128, 32, 128], bf16, tag="xl", name="xl")
                        nc.sync.dma_start(xls[bt + 2][:], xl_t[:, bt + 2, :, :])
                    folds_cur = emit_folds(xls[bt + 1])
                    ynB = emit_B(xl, fB)
                    ynC = emit_C(xl, fC)
                    cpp = midp.tile([128, 1024], bf16, tag="cpp", name="cpp")
                    nc.vector.tensor_tensor(cpp[:, 0:512], sD[:], ynC[:], ADD)
                    nc.vector.tensor_tensor(cpp[:, 512:1024], sD[:], ynC[:], SUB)
                    u1 = midp.tile([128, 1024], bf16, tag="u1", name="u1")
                    nc.vector.tensor_tensor(u1[:], cpp[:], ynB[:], ADD)
                    u2 = midp.tile([128, 1024], bf16, tag="u2", name="u2")
                    nc.vector.tensor_tensor(u2[:], cpp[:], ynB[:], SUB)
                    dve_share = bt >= NB - 3
                    pending = (
                        lambda b0=b0, u1=u1, u2=u2, ynA=ynA, d=dve_share: emit_finals(
                            b0, u1, u2, ynA, dve_share=d
                        )
                    )
                else:
                    # last block: C/B/D first, then Q/P trees, R last; finals
                    # for the y_lo half stream during R, y_hi half-granular
                    if pending is not None:
                        pending()
                        pending = None
                    ynC = emit_C(xl, fC)
                    ynB = emit_B(xl, fB)
                    TDg = pp.tile([128, 1536], f32, tag="T1", name="T1D")
                    sD = emit_D(TDg[:, 0:512], xl)
                    cpp = midp.tile([128, 1024], bf16, tag="cpp", name="cpp")
                    nc.vector.tensor_tensor(cpp[:, 0:512], sD[:], ynC[:], ADD)
                    nc.vector.tensor_tensor(cpp[:, 512:1024], sD[:], ynC[:], SUB)
                    u1 = midp.tile([128, 1024], bf16, tag="u1", name="u1")
                    nc.vector.tensor_tensor(u1[:], cpp[:], ynB[:], ADD)
                    u2 = midp.tile([128, 1024], bf16, tag="u2", name="u2")
                    nc.vector.tensor_tensor(u2[:], cpp[:], ynB[:], SUB)
                    tQ = emit_tree(
                        bt, "T1", "Q", vQ01,
                        lambda t: xl[:, 8 + t, :], lambda t: xl[:, 12 + t, :],
                    )
                    tP = emit_tree(
                        bt, "T2", "P", vP01,
                        lambda t: fA[:, t, :], lambda t: fA[:, 4 + t, :],
                    )
                    ynA = ynp.tile([128, 2048], bf16, tag="ynA", name="ynA")
                    nc.vector.tensor_tensor(ynA[:, 0:1024], tP[:], tQ[:], SUB)
                    for seg, alu, eng in ((0, ADD, "v"), (2, SUB, "p")):
                        o = op.tile([128, 1024], f32, tag="o", name="o", bufs=4)
                        if eng == "v":
                            nc.vector.tensor_tensor(o[:], u1[:], ynA[:, 0:1024], alu)
                        else:
                            nc.gpsimd.tensor_tensor(o[:], u1[:], ynA[:, 0:1024], alu)
                        nc.sync.dma_start(
                            out_d[b0 : b0 + 128, 1024 * seg : 1024 * seg + 1024], o[:]
                        )
                    TRp = pp.tile([128, 1536], f32, tag="T2", name="T2R")
                    t1024(TRp[:, 0:1024], "AR", lambda t: xl[:, t, :])
                    for h in (0, 1):
                        nc.vector.tensor_tensor(
                            ynA[:, 1024 + 512 * h : 1536 + 512 * h],
                            tP[:, 512 * h : 512 * h + 512],
                            TRp[:, 512 * h : 512 * h + 512],
                            ADD,
                        )
                        for seg, alu, eng in ((1, ADD, "v"), (3, SUB, "p")):
                            o = op.tile([128, 512], f32, tag="oh", name="oh", bufs=4)
                            args = (
                                o[:],
                                u2[:, 512 * h : 512 * h + 512],
                                ynA[:, 1024 + 512 * h : 1536 + 512 * h],
                                alu,
                            )
                            if eng == "v":
                                nc.vector.tensor_tensor(*args)
                            else:
                                nc.gpsimd.tensor_tensor(*args)
                            nc.sync.dma_start(
                                out_d[
                                    b0 : b0 + 128,
                                    1024 * seg + 512 * h : 1024 * seg + 512 * h + 512,
                                ],
                                o[:],
                            )
            if pending is not None:
                pending()

    nc.compile()
    return nc


def _get_nc():
    if "nc" not in _STATE:
        _STATE["nc"] = _build()
    return _STATE["nc"]


def _prep_inputs(x, w):
    x = np.ascontiguousarray(x, dtype=np.float32)
    w = np.asarray(w, dtype=np.float32)
    bands = _make_bands(w)
    in_maps = []
    for i in range(N_CORES):
        xl = _fold_x(x[i * B_SHARD : (i + 1) * B_SHARD])
        in_maps.append({"xl": xl.reshape(128, 32 * B_SHARD), "bands": bands})
    return in_maps


def kernel(x, w, _trace=False):
    from concourse.bass_utils import run_bass_kernel_spmd

    nc = _get_nc()
    in_maps = _prep_inputs(x, w)
    res = run_bass_kernel_spmd(nc, in_maps, list(range(N_CORES)), trace=_trace)
    out = np.concatenate([res.results[i]["out"] for i in range(N_CORES)], axis=0)
    if _trace:
        _STATE["last_result"] = res
    return out


# revision 6
# speedup vs baseline: 1.0209x; 1.0123x over previous
"""Circulant matmul for TRN2 v3: CRT + Gauss 3-mult negacyclic splits with
depth-2 Karatsuba on two of the three nega2048 trees, bf16 matmuls,
host-precomputed x-folds and pre-scaled band operators.

out[b, r] = sum_c x[b,c] w[(c-r) mod N] = cyclic_conv(x_row, v), v = roll(rev(w),1).

CRT: z^4096-1 -> leaves nega2048 (A), nega1024 (B), nega512 (C), cyc512 (D).
nega-n via Gauss 3-mult on its [[T,-U],[U,T]] structure:
  P = T(a0+a1), Q = (T+U)a1, R = (U-T)a0, y = [P-Q, P+R].
For A, the P and Q Toeplitz-1024 products are themselves Karatsuba'd into
3 Toeplitz-512 products each (p = T0(v0+v1), top = p + (V-T0)v1,
bot = p + (W-T0)v0); R stays dense -- engine balance: deeper splits save PE
but overload DVE/Pool with combines. PE: 30208 cyc/128-row block.

Host sends bf16: x CRT-leaf chunks [128, 8, 32, 128] (8 MB/core, per-block
contiguous) + pre-scaled band blob (3.1 MB). All CRT/Gauss scales folded
into bands. ACT evacuates shared Kara leaves + cyc512, DVE does PSUM
combines, Pool does folds fB/fC and most final output combines. PSUM map:
T1 (Q-leaves -> R||cyc512), T2 (P-leaves -> B-leaves), TC (nega512) --
ordered so no combine chain ever gates the next block's first matmul
group; per-block PE order Q, P, R, D, B, C. The last two blocks split
finals across DVE/Pool at fine granularity so the tail output DMAs
overlap the final A-tree matmul stream.
"""

import sys

sys.path.insert(0, "/opt/trn_rl_repo")

import numpy as np
import ml_dtypes

BF16 = ml_dtypes.bfloat16
N = 4096
B = 8192
N_CORES = 8
B_SHARD = B // N_CORES  # 1024
NB = B_SHARD // 128     # 8 row-tiles per core

# band blob layout: name -> (m, offset); width = 2m-128; consumption order
_BANDS = {
    "QT0": (512, 0), "QV": (512, 896), "QW": (512, 1792),
    "PT0": (512, 2688), "PV": (512, 3584), "PW": (512, 4480),
    "AR": (1024, 5376),
    "D": (512, 7296), "BQ": (512, 8192), "BP": (512, 9088),
    "CQ": (256, 9984), "CP": (256, 10368), "BR": (512, 10752),
    "CR": (256, 11648),
}
WTOT = 12032

_STATE = {}


# ---------------------------------------------------------------- host math
def _sub_symbol(t, m, shift, h):
    c = m - 1
    return t[c + shift - (h - 1) : c + shift + h].copy()


def _band_from_symbol(t, m):
    u = np.arange(2 * m - 128)[None, :]
    p = np.arange(128)[:, None]
    return t[u - p + 127]


def _make_bands(w):
    v = np.roll(w[::-1].astype(np.float64), 1)
    vm = v[:2048] - v[2048:]
    vp = v[:2048] + v[2048:]
    vpm = vp[:1024] - vp[1024:]
    vpp = vp[:1024] + vp[1024:]
    vppm = vpp[:512] - vpp[512:]
    vppp = vpp[:512] + vpp[512:]

    def skew_sym(b):
        n = len(b)
        t = np.empty(2 * n - 1)
        t[n - 1 :] = b
        t[: n - 1] = -b[1:]
        return t

    def cyc_sym(b):
        n = len(b)
        t = np.empty(2 * n - 1)
        t[n - 1 :] = b
        t[: n - 1] = b[1:]
        return t

    blob = np.zeros((128, WTOT), np.float64)

    def put(name, sym):
        m, off = _BANDS[name]
        blob[:, off : off + 2 * m - 128] = _band_from_symbol(sym, m)

    # A = nega2048: T1024-level Gauss operators (scale 0.5 folded in)
    full = skew_sym(vm)
    tT = _sub_symbol(full, 2048, 0, 1024)
    tU = _sub_symbol(full, 2048, 1024, 1024)
    sP = 0.5 * tT
    sQ = 0.5 * (tT + tU)
    sR = 0.5 * (tU - tT)
    for nm, sym in (("P", sP), ("Q", sQ)):
        t0 = _sub_symbol(sym, 1024, 0, 512)
        tV = _sub_symbol(sym, 1024, -512, 512)
        tW = _sub_symbol(sym, 1024, 512, 512)
        put(nm + "T0", t0)
        put(nm + "V", tV - t0)
        put(nm + "W", tW - t0)
    put("AR", sR)

    for nm, bsrc, scale in [("B", vpm, 0.25), ("C", vppm, 0.125)]:
        n = len(bsrc)
        m = n // 2
        fl = skew_sym(bsrc)
        t1 = _sub_symbol(fl, n, 0, m)
        t2 = _sub_symbol(fl, n, m, m)
        put(nm + "P", scale * t1)
        put(nm + "Q", scale * (t1 + t2))
        put(nm + "R", scale * (t2 - t1))
    put("D", 0.125 * cyc_sym(vppp))
    return blob.astype(BF16)


def _fold_x(x_shard):
    """[1024, 4096] f32 -> [128, 8, 32, 128] bf16 chunk-partitioned CRT
    leaves, per-block contiguous."""
    xs = x_shard.astype(np.float32)
    xm = xs[:, :2048] - xs[:, 2048:]
    xp = xs[:, :2048] + xs[:, 2048:]
    xpm = xp[:, :1024] - xp[:, 1024:]
    xpp = xp[:, :1024] + xp[:, 1024:]
    xppm = xpp[:, :512] - xpp[:, 512:]
    xppp = xpp[:, :512] + xpp[:, 512:]
    L = np.concatenate([xm, xpm, xppm, xppp], axis=1)  # [1024, 4096]
    a = L.reshape(1024, 32, 128).transpose(2, 1, 0)  # [128, 32, 1024]
    a = a.reshape(128, 32, 8, 128).transpose(0, 2, 1, 3)
    return np.ascontiguousarray(a.astype(BF16))


# ---------------------------------------------------------------- device
def _build():
    import concourse.bacc as bacc
    import concourse.mybir as mybir
    import concourse.tile as tile

    f32 = mybir.dt.float32
    bf16 = mybir.dt.bfloat16
    ADD = mybir.AluOpType.add
    SUB = mybir.AluOpType.subtract

    nc = bacc.Bacc("TRN2", target_bir_lowering=False, debug=False)
    xl_d = nc.declare_dram_parameter("xl", [128, 32 * B_SHARD], bf16, isOutput=False)
    bands_d = nc.declare_dram_parameter("bands", [128, WTOT], bf16, isOutput=False)
    out_d = nc.declare_dram_parameter("out", [B_SHARD, N], bf16, isOutput=True)

    xl_t = xl_d[:].rearrange("p (k a b) -> p k a b", k=NB, a=32)  # [128, 8, 32, 128]

    with tile.TileContext(nc) as tc:
        with (
            tc.tile_pool(name="const", bufs=1) as constp,
            tc.tile_pool(name="xl", bufs=2) as xlp,
            tc.tile_pool(name="fold", bufs=3) as fp,
            tc.tile_pool(name="yn", bufs=3) as ynp,
            tc.tile_pool(name="mid", bufs=3) as midp,
            tc.tile_pool(name="outp", bufs=3) as op,
            tc.tile_pool(name="ps", bufs=1, space="PSUM") as pp,
        ):
            band = constp.tile([128, WTOT], bf16)
            warm = constp.tile([128, 512], bf16, name="warm")
            nc.gpsimd.memset(warm[:], 0.0)

            def bsl(name, lo, width):
                off = _BANDS[name][1]
                return band[:, off + lo : off + lo + width]

            # first-block DMAs in consumption order; xl(1) last
            xl0 = xlp.tile([128, 32, 128], bf16, tag="xl", name="xl0")
            nc.sync.dma_start(xl0[:, 8:16, :], xl_t[:, 0, 8:16, :])
            nc.sync.dma_start(band[:, 0:896], bands_d[:, 0:896])        # QT0 band
            nc.sync.dma_start(xl0[:, 0:8, :], xl_t[:, 0, 0:8, :])
            nc.sync.dma_start(band[:, 896:2688], bands_d[:, 896:2688])  # QV/QW
            nc.sync.dma_start(band[:, 2688:5376], bands_d[:, 2688:5376])  # P bands
            nc.sync.dma_start(band[:, 5376:7296], bands_d[:, 5376:7296])  # AR band
            nc.sync.dma_start(xl0[:, 16:32, :], xl_t[:, 0, 16:32, :])
            nc.sync.dma_start(band[:, 7296:WTOT], bands_d[:, 7296:WTOT])  # D/B/C
            xl1 = xlp.tile([128, 32, 128], bf16, tag="xl", name="xl1")
            nc.sync.dma_start(xl1[:], xl_t[:, 1, :, :])

            def t1024(psum, bname, src):
                for j in range(2):
                    for t in range(8):
                        nc.tensor.matmul(
                            psum[:, 512 * j : 512 * j + 512],
                            src(t),
                            bsl(bname, (7 - t) * 128 + 512 * j, 512),
                            start=(t == 0),
                            stop=(t == 7),
                        )

            def t512(psum, bname, src):
                for t in range(4):
                    nc.tensor.matmul(
                        psum[:],
                        src(t),
                        bsl(bname, (3 - t) * 128, 512),
                        start=(t == 0),
                        stop=(t == 3),
                    )

            def t256(psum, bname, src):
                for t in range(2):
                    nc.tensor.matmul(
                        psum[:],
                        src(t),
                        bsl(bname, (1 - t) * 128, 256),
                        start=(t == 0),
                        stop=(t == 1),
                    )

            def emit_folds(xl):
                vQ01 = fp.tile([128, 4, 128], bf16, tag="vQ01", name="vQ01")
                nc.vector.tensor_tensor(vQ01[:], xl[:, 8:12, :], xl[:, 12:16, :], ADD)
                fA = fp.tile([128, 8, 128], bf16, tag="fA", name="fA")
                nc.vector.tensor_tensor(fA[:], xl[:, 0:8, :], xl[:, 8:16, :], ADD)
                vP01 = fp.tile([128, 4, 128], bf16, tag="vP01", name="vP01")
                nc.vector.tensor_tensor(vP01[:], fA[:, 0:4, :], fA[:, 4:8, :], ADD)
                fC = fp.tile([128, 2, 128], bf16, tag="fC", name="fC")
                nc.gpsimd.tensor_tensor(fC[:], xl[:, 24:26, :], xl[:, 26:28, :], ADD)
                fB = fp.tile([128, 4, 128], bf16, tag="fB", name="fB")
                nc.gpsimd.tensor_tensor(fB[:], xl[:, 16:20, :], xl[:, 20:24, :], ADD)
                return vQ01, fA, vP01, fB, fC

            def emit_tree(bt, tag, bpfx, vp01, v0src, v1src, warmup=False):
                """Depth-2 Karatsuba Toeplitz-1024 tree -> tX [128,1024] bf16."""
                TL = pp.tile([128, 1536], f32, tag=tag, name=tag + "L")
                if warmup:
                    for _ in range(4):
                        nc.tensor.matmul(
                            TL[:, 1024:1536], warm[:, 0:128], warm[:],
                            start=True, stop=True,
                        )
                t512(TL[:, 0:512], bpfx + "T0", lambda t: vp01[:, t, :])
                t512(TL[:, 512:1024], bpfx + "V", v1src)
                sXp = midp.tile([128, 512], bf16, tag="s" + bpfx, name="s" + bpfx)
                nc.scalar.copy(sXp[:], TL[:, 0:512])
                tX = midp.tile([128, 1024], bf16, tag="t" + bpfx, name="t" + bpfx)
                nc.vector.tensor_tensor(tX[:, 0:512], sXp[:], TL[:, 512:1024], ADD)
                t512(TL[:, 1024:1536], bpfx + "W", v0src)
                nc.vector.tensor_tensor(tX[:, 512:1024], sXp[:], TL[:, 1024:1536], ADD)
                return tX

            def emit_C(xl, fC):
                TCg = pp.tile([128, 768], f32, tag="TC", name="TC")
                t256(TCg[:, 0:256], "CQ", lambda t: xl[:, 26 + t, :])
                t256(TCg[:, 256:512], "CP", lambda t: fC[:, t, :])
                sCP = midp.tile([128, 256], bf16, tag="sCP", name="sCP")
                nc.scalar.copy(sCP[:], TCg[:, 256:512])
                ynC = ynp.tile([128, 512], bf16, tag="ynC", name="ynC")
                nc.vector.tensor_tensor(ynC[:, 0:256], sCP[:], TCg[:, 0:256], SUB)
                t256(TCg[:, 512:768], "CR", lambda t: xl[:, 24 + t, :])
                nc.vector.tensor_tensor(ynC[:, 256:512], sCP[:], TCg[:, 512:768], ADD)
                return ynC

            def emit_B(xl, fB):
                T2g = pp.tile([128, 1536], f32, tag="T2", name="T2B")
                t512(T2g[:, 0:512], "BQ", lambda t: xl[:, 20 + t, :])
                t512(T2g[:, 512:1024], "BP", lambda t: fB[:, t, :])
                sBP = midp.tile([128, 512], bf16, tag="sBP", name="sBP")
                nc.scalar.copy(sBP[:], T2g[:, 512:1024])
                ynB = ynp.tile([128, 1024], bf16, tag="ynB", name="ynB")
                nc.vector.tensor_tensor(ynB[:, 0:512], sBP[:], T2g[:, 0:512], SUB)
                t512(T2g[:, 1024:1536], "BR", lambda t: xl[:, 16 + t, :])
                nc.vector.tensor_tensor(ynB[:, 512:1024], sBP[:], T2g[:, 1024:1536], ADD)
                return ynB

            def emit_D(psum, xl):
                # D shares the R generation's T1 tile (banks are disjoint);
                # ACT evacuates so cpp is SBUF-only and nothing D-related
                # gates the next block's T1 reuse
                t512(psum, "D", lambda t: xl[:, 28 + t, :])
                sD = midp.tile([128, 512], bf16, tag="sD", name="sD")
                nc.scalar.copy(sD[:], psum)
                return sD

            def emit_unfold_mid(TD, ynB, ynC):
                cpp = midp.tile([128, 1024], bf16, tag="cpp", name="cpp")
                nc.vector.tensor_tensor(cpp[:, 0:512], TD[:], ynC[:], ADD)
                nc.vector.tensor_tensor(cpp[:, 512:1024], TD[:], ynC[:], SUB)
                u1 = midp.tile([128, 1024], bf16, tag="u1", name="u1")
                nc.vector.tensor_tensor(u1[:], cpp[:], ynB[:], ADD)
                u2 = midp.tile([128, 1024], bf16, tag="u2", name="u2")
                nc.vector.tensor_tensor(u2[:], cpp[:], ynB[:], SUB)
                return u1, u2

            def emit_finals(b0, u1, u2, ynA, dve_share=False):
                # Pool by default; DVE takes half in the drain-critical blocks,
                # which also get per-1024 DMAs for finer tail interleave
                if dve_share:
                    for seg, usrc, alu, eng in (
                        (0, u1, ADD, nc.gpsimd), (1, u2, ADD, nc.gpsimd),
                        (2, u1, SUB, nc.gpsimd), (3, u2, SUB, nc.gpsimd),
                    ):
                        o = op.tile([128, 1024], bf16, tag="o", name="o", bufs=4)
                        eng.tensor_tensor(
                            o[:], usrc[:], ynA[:, 1024 * (seg % 2) : 1024 * (seg % 2) + 1024], alu
                        )
                        nc.sync.dma_start(
                            out_d[b0 : b0 + 128, 1024 * seg : 1024 * seg + 1024], o[:]
                        )
                    return
                oL = op.tile([128, 2048], bf16, tag="oL", name="oL")
                nc.gpsimd.tensor_tensor(oL[:, 0:1024], u1[:], ynA[:, 0:1024], ADD)
                nc.gpsimd.tensor_tensor(oL[:, 1024:2048], u2[:], ynA[:, 1024:2048], ADD)
                nc.sync.dma_start(out_d[b0 : b0 + 128, 0:2048], oL[:])
                oR = op.tile([128, 2048], bf16, tag="oR", name="oR")
                nc.gpsimd.tensor_tensor(oR[:, 0:1024], u1[:], ynA[:, 0:1024], SUB)
                nc.gpsimd.tensor_tensor(oR[:, 1024:2048], u2[:], ynA[:, 1024:2048], SUB)
                nc.sync.dma_start(out_d[b0 : b0 + 128, 2048:4096], oR[:])

            pending = None
            xls = {0: xl0, 1: xl1}
            folds_cur = emit_folds(xl0)
            for bt in range(NB):
                b0 = 128 * bt
                xl = xls[bt]
                vQ01, fA, vP01, fB, fC = folds_cur
                last = bt == NB - 1
                if not last:
                    if pending is not None:
                        pending()
                    tQ = emit_tree(
                        bt, "T1", "Q", vQ01,
                        lambda t: xl[:, 8 + t, :], lambda t: xl[:, 12 + t, :],
                        warmup=(bt == 0),
                    )
                    tP = emit_tree(
                        bt, "T2", "P", vP01,
                        lambda t: fA[:, t, :], lambda t: fA[:, 4 + t, :],
                    )
                    ynA = ynp.tile([128, 2048], bf16, tag="ynA", name="ynA")
                    nc.vector.tensor_tensor(ynA[:, 0:1024], tP[:], tQ[:], SUB)
                    TRp = pp.tile([128, 1536], f32, tag="T1", name="T1RD")
                    t1024(TRp[:, 0:1024], "AR", lambda t: xl[:, t, :])
                    for h in (0, 1):
                        nc.vector.tensor_tensor(
                            ynA[:, 1024 + 512 * h : 1536 + 512 * h],
                            tP[:, 512 * h : 512 * h + 512],
                            TRp[:, 512 * h : 512 * h + 512],
                            ADD,
                        )
                    sD = emit_D(TRp[:, 1024:1536], xl)
                    # next block's input + folds right behind the hi-halves in
                    # the DVE queue so block bt+1's first group is never gated
                    if bt + 2 < NB:
                        xls[bt + 2] = xlp.tile([128, 32, 128], bf16, tag="xl", name="xl")
                        nc.sync.dma_start(xls[bt + 2][:], xl_t[:, bt + 2, :, :])
                    folds_cur = emit_folds(xls[bt + 1])
                    ynB = emit_B(xl, fB)
                    ynC = emit_C(xl, fC)
                    cpp = midp.tile([128, 1024], bf16, tag="cpp", name="cpp")
                    nc.vector.tensor_tensor(cpp[:, 0:512], sD[:], ynC[:], ADD)
                    nc.vector.tensor_tensor(cpp[:, 512:1024], sD[:], ynC[:], SUB)
                    u1 = midp.tile([128, 1024], bf16, tag="u1", name="u1")
                    nc.vector.tensor_tensor(u1[:], cpp[:], ynB[:], ADD)
                    u2 = midp.tile([128, 1024], bf16, tag="u2", name="u2")
                    nc.vector.tensor_tensor(u2[:], cpp[:], ynB[:], SUB)
                    dve_share = bt >= NB - 3
                    pending = (
                        lambda b0=b0, u1=u1, u2=u2, ynA=ynA, d=dve_share: emit_finals(
                            b0, u1, u2, ynA, dve_share=d
                        )
                    )
                else:
                    # last block: C/B/D first, then Q/P trees, R last; finals
                    # for the y_lo half stream during R, y_hi half-granular
                    if pending is not None:
                        pending()
                        pending = None
                    ynC = emit_C(xl, fC)
                    ynB = emit_B(xl, fB)
                    TDg = pp.tile([128, 1536], f32, tag="T1", name="T1D")
                    sD = emit_D(TDg[:, 0:512], xl)
                    cpp = midp.tile([128, 1024], bf16, tag="cpp", name="cpp")
                    nc.vector.tensor_tensor(cpp[:, 0:512], sD[:], ynC[:], ADD)
                    nc.vector.tensor_tensor(cpp[:, 512:1024], sD[:], ynC[:], SUB)
                    u1 = midp.tile([128, 1024], bf16, tag="u1", name="u1")
                    nc.vector.tensor_tensor(u1[:], cpp[:], ynB[:], ADD)
                    u2 = midp.tile([128, 1024], bf16, tag="u2", name="u2")
                    nc.vector.tensor_tensor(u2[:], cpp[:], ynB[:], SUB)
                    tQ = emit_tree(
                        bt, "T1", "Q", vQ01,
                        lambda t: xl[:, 8 + t, :], lambda t: xl[:, 12 + t, :],
                    )
                    tP = emit_tree(
                        bt, "T2", "P", vP01,
                        lambda t: fA[:, t, :], lambda t: fA[:, 4 + t, :],
                    )
                    ynA = ynp.tile([128, 2048], bf16, tag="ynA", name="ynA")
                    nc.vector.tensor_tensor(ynA[:, 0:1024], tP[:], tQ[:], SUB)
                    for seg, alu, eng in ((0, ADD, "v"), (2, SUB, "p")):
                        o = op.tile([128, 1024], bf16, tag="o", name="o", bufs=4)
                        if eng == "v":
                            nc.vector.tensor_tensor(o[:], u1[:], ynA[:, 0:1024], alu)
                        else:
                            nc.gpsimd.tensor_tensor(o[:], u1[:], ynA[:, 0:1024], alu)
                        nc.sync.dma_start(
                            out_d[b0 : b0 + 128, 1024 * seg : 1024 * seg + 1024], o[:]
                        )
                    TRp = pp.tile([128, 1536], f32, tag="T2", name="T2R")
                    t1024(TRp[:, 0:1024], "AR", lambda t: xl[:, t, :])
                    for h in (0, 1):
                        nc.vector.tensor_tensor(
                            ynA[:, 1024 + 512 * h : 1536 + 512 * h],
                            tP[:, 512 * h : 512 * h + 512],
                            TRp[:, 512 * h : 512 * h + 512],
                            ADD,
                        )
                        for seg, alu, eng in ((1, ADD, "v"), (3, SUB, "p")):
                            o = op.tile([128, 512], bf16, tag="oh", name="oh", bufs=4)
                            args = (
                                o[:],
                                u2[:, 512 * h : 512 * h + 512],
                                ynA[:, 1024 + 512 * h : 1536 + 512 * h],
                                alu,
                            )
                            if eng == "v":
                                nc.vector.tensor_tensor(*args)
                            else:
                                nc.gpsimd.tensor_tensor(*args)
                            nc.sync.dma_start(
                                out_d[
                                    b0 : b0 + 128,
                                    1024 * seg + 512 * h : 1024 * seg + 512 * h + 512,
                                ],
                                o[:],
                            )
            if pending is not None:
                pending()

    nc.compile()
    return nc


def _get_nc():
    if "nc" not in _STATE:
        _STATE["nc"] = _build()
    return _STATE["nc"]


def _prep_inputs(x, w):
    x = np.ascontiguousarray(x, dtype=np.float32)
    w = np.asarray(w, dtype=np.float32)
    bands = _make_bands(w)
    in_maps = []
    for i in range(N_CORES):
        xl = _fold_x(x[i * B_SHARD : (i + 1) * B_SHARD])
        in_maps.append({"xl": xl.reshape(128, 32 * B_SHARD), "bands": bands})
    return in_maps


def kernel(x, w, _trace=False):
    from concourse.bass_utils import run_bass_kernel_spmd

    nc = _get_nc()
    in_maps = _prep_inputs(x, w)
    res = run_bass_kernel_spmd(nc, in_maps, list(range(N_CORES)), trace=_trace)
    out = np.concatenate(
        [np.asarray(res.results[i]["out"]).astype(np.float32) for i in range(N_CORES)],
        axis=0,
    )
    if _trace:
        _STATE["last_result"] = res
    return out


# revision 7
# speedup vs baseline: 1.0222x; 1.0012x over previous
"""Circulant matmul for TRN2 v3: CRT + Gauss 3-mult negacyclic splits with
depth-2 Karatsuba on two of the three nega2048 trees, bf16 matmuls,
host-precomputed x-folds and pre-scaled band operators.

out[b, r] = sum_c x[b,c] w[(c-r) mod N] = cyclic_conv(x_row, v), v = roll(rev(w),1).

CRT: z^4096-1 -> leaves nega2048 (A), nega1024 (B), nega512 (C), cyc512 (D).
nega-n via Gauss 3-mult on its [[T,-U],[U,T]] structure:
  P = T(a0+a1), Q = (T+U)a1, R = (U-T)a0, y = [P-Q, P+R].
For A, the P and Q Toeplitz-1024 products are themselves Karatsuba'd into
3 Toeplitz-512 products each (p = T0(v0+v1), top = p + (V-T0)v1,
bot = p + (W-T0)v0); R stays dense -- engine balance: deeper splits save PE
but overload DVE/Pool with combines. PE: 30208 cyc/128-row block.

Host sends bf16: x CRT-leaf chunks [128, 8, 32, 128] (8 MB/core, per-block
contiguous) + pre-scaled band blob (3.1 MB). All CRT/Gauss scales folded
into bands. ACT evacuates shared Kara leaves + cyc512, DVE does PSUM
combines, Pool does folds fB/fC and most final output combines. PSUM map:
T1 (Q-leaves -> R||cyc512), T2 (P-leaves -> B-leaves), TC (nega512) --
ordered so no combine chain ever gates the next block's first matmul
group; per-block PE order Q, P, R, D, B, C. The last two blocks split
finals across DVE/Pool at fine granularity so the tail output DMAs
overlap the final A-tree matmul stream.
"""

import sys

sys.path.insert(0, "/opt/trn_rl_repo")

import numpy as np
import ml_dtypes

BF16 = ml_dtypes.bfloat16
N = 4096
B = 8192
N_CORES = 8
B_SHARD = B // N_CORES  # 1024
NB = B_SHARD // 128     # 8 row-tiles per core

# band blob layout: name -> (m, offset); width = 2m-128; consumption order
_BANDS = {
    "QT0": (512, 0), "QV": (512, 896), "QW": (512, 1792),
    "PT0": (512, 2688), "PV": (512, 3584), "PW": (512, 4480),
    "AR": (1024, 5376),
    "D": (512, 7296), "BQ": (512, 8192), "BP": (512, 9088),
    "CQ": (256, 9984), "CP": (256, 10368), "BR": (512, 10752),
    "CR": (256, 11648),
}
WTOT = 12032

_STATE = {}


# ---------------------------------------------------------------- host math
def _sub_symbol(t, m, shift, h):
    c = m - 1
    return t[c + shift - (h - 1) : c + shift + h].copy()


def _band_from_symbol(t, m):
    u = np.arange(2 * m - 128)[None, :]
    p = np.arange(128)[:, None]
    return t[u - p + 127]


def _make_bands(w):
    v = np.roll(w[::-1].astype(np.float64), 1)
    vm = v[:2048] - v[2048:]
    vp = v[:2048] + v[2048:]
    vpm = vp[:1024] - vp[1024:]
    vpp = vp[:1024] + vp[1024:]
    vppm = vpp[:512] - vpp[512:]
    vppp = vpp[:512] + vpp[512:]

    def skew_sym(b):
        n = len(b)
        t = np.empty(2 * n - 1)
        t[n - 1 :] = b
        t[: n - 1] = -b[1:]
        return t

    def cyc_sym(b):
        n = len(b)
        t = np.empty(2 * n - 1)
        t[n - 1 :] = b
        t[: n - 1] = b[1:]
        return t

    blob = np.zeros((128, WTOT), np.float64)

    def put(name, sym):
        m, off = _BANDS[name]
        blob[:, off : off + 2 * m - 128] = _band_from_symbol(sym, m)

    # A = nega2048: T1024-level Gauss operators (scale 0.5 folded in)
    full = skew_sym(vm)
    tT = _sub_symbol(full, 2048, 0, 1024)
    tU = _sub_symbol(full, 2048, 1024, 1024)
    sP = 0.5 * tT
    sQ = 0.5 * (tT + tU)
    sR = 0.5 * (tU - tT)
    for nm, sym in (("P", sP), ("Q", sQ)):
        t0 = _sub_symbol(sym, 1024, 0, 512)
        tV = _sub_symbol(sym, 1024, -512, 512)
        tW = _sub_symbol(sym, 1024, 512, 512)
        put(nm + "T0", t0)
        put(nm + "V", tV - t0)
        put(nm + "W", tW - t0)
    put("AR", sR)

    for nm, bsrc, scale in [("B", vpm, 0.25), ("C", vppm, 0.125)]:
        n = len(bsrc)
        m = n // 2
        fl = skew_sym(bsrc)
        t1 = _sub_symbol(fl, n, 0, m)
        t2 = _sub_symbol(fl, n, m, m)
        put(nm + "P", scale * t1)
        put(nm + "Q", scale * (t1 + t2))
        put(nm + "R", scale * (t2 - t1))
    put("D", 0.125 * cyc_sym(vppp))
    return blob.astype(BF16)


def _fold_x(x_shard):
    """[1024, 4096] f32 -> [128, 8, 32, 128] bf16 chunk-partitioned CRT
    leaves, per-block contiguous."""
    xs = x_shard.astype(np.float32)
    xm = xs[:, :2048] - xs[:, 2048:]
    xp = xs[:, :2048] + xs[:, 2048:]
    xpm = xp[:, :1024] - xp[:, 1024:]
    xpp = xp[:, :1024] + xp[:, 1024:]
    xppm = xpp[:, :512] - xpp[:, 512:]
    xppp = xpp[:, :512] + xpp[:, 512:]
    L = np.concatenate([xm, xpm, xppm, xppp], axis=1)  # [1024, 4096]
    a = L.reshape(1024, 32, 128).transpose(2, 1, 0)  # [128, 32, 1024]
    a = a.reshape(128, 32, 8, 128).transpose(0, 2, 1, 3)
    return np.ascontiguousarray(a.astype(BF16))


# ---------------------------------------------------------------- device
def _build():
    import concourse.bacc as bacc
    import concourse.mybir as mybir
    import concourse.tile as tile

    f32 = mybir.dt.float32
    bf16 = mybir.dt.bfloat16
    ADD = mybir.AluOpType.add
    SUB = mybir.AluOpType.subtract

    nc = bacc.Bacc("TRN2", target_bir_lowering=False, debug=False)
    xl_d = nc.declare_dram_parameter("xl", [128, 32 * B_SHARD], bf16, isOutput=False)
    bands_d = nc.declare_dram_parameter("bands", [128, WTOT], bf16, isOutput=False)
    out_d = nc.declare_dram_parameter("out", [B_SHARD, N], bf16, isOutput=True)

    xl_t = xl_d[:].rearrange("p (k a b) -> p k a b", k=NB, a=32)  # [128, 8, 32, 128]

    with tile.TileContext(nc) as tc:
        with (
            tc.tile_pool(name="const", bufs=1) as constp,
            tc.tile_pool(name="xl", bufs=2) as xlp,
            tc.tile_pool(name="fold", bufs=3) as fp,
            tc.tile_pool(name="yn", bufs=4) as ynp,
            tc.tile_pool(name="mid", bufs=4) as midp,
            tc.tile_pool(name="outp", bufs=3) as op,
            tc.tile_pool(name="ps", bufs=1, space="PSUM") as pp,
        ):
            band = constp.tile([128, WTOT], bf16)
            warm = constp.tile([128, 512], bf16, name="warm")
            nc.gpsimd.memset(warm[:], 0.0)

            def bsl(name, lo, width):
                off = _BANDS[name][1]
                return band[:, off + lo : off + lo + width]

            # first-block DMAs in consumption order; xl(1) last
            xl0 = xlp.tile([128, 32, 128], bf16, tag="xl", name="xl0")
            nc.sync.dma_start(xl0[:, 8:16, :], xl_t[:, 0, 8:16, :])
            nc.sync.dma_start(band[:, 0:896], bands_d[:, 0:896])        # QT0 band
            nc.sync.dma_start(xl0[:, 0:8, :], xl_t[:, 0, 0:8, :])
            nc.sync.dma_start(band[:, 896:2688], bands_d[:, 896:2688])  # QV/QW
            nc.sync.dma_start(band[:, 2688:5376], bands_d[:, 2688:5376])  # P bands
            nc.sync.dma_start(band[:, 5376:7296], bands_d[:, 5376:7296])  # AR band
            nc.sync.dma_start(xl0[:, 16:32, :], xl_t[:, 0, 16:32, :])
            nc.sync.dma_start(band[:, 7296:WTOT], bands_d[:, 7296:WTOT])  # D/B/C
            xl1 = xlp.tile([128, 32, 128], bf16, tag="xl", name="xl1")
            nc.sync.dma_start(xl1[:], xl_t[:, 1, :, :])

            def t1024(psum, bname, src):
                for j in range(2):
                    for t in range(8):
                        nc.tensor.matmul(
                            psum[:, 512 * j : 512 * j + 512],
                            src(t),
                            bsl(bname, (7 - t) * 128 + 512 * j, 512),
                            start=(t == 0),
                            stop=(t == 7),
                        )

            def t512(psum, bname, src):
                for t in range(4):
                    nc.tensor.matmul(
                        psum[:],
                        src(t),
                        bsl(bname, (3 - t) * 128, 512),
                        start=(t == 0),
                        stop=(t == 3),
                    )

            def t256(psum, bname, src):
                for t in range(2):
                    nc.tensor.matmul(
                        psum[:],
                        src(t),
                        bsl(bname, (1 - t) * 128, 256),
                        start=(t == 0),
                        stop=(t == 1),
                    )

            def emit_folds(xl):
                vQ01 = fp.tile([128, 4, 128], bf16, tag="vQ01", name="vQ01")
                nc.vector.tensor_tensor(vQ01[:], xl[:, 8:12, :], xl[:, 12:16, :], ADD)
                fA = fp.tile([128, 8, 128], bf16, tag="fA", name="fA")
                nc.vector.tensor_tensor(fA[:], xl[:, 0:8, :], xl[:, 8:16, :], ADD)
                vP01 = fp.tile([128, 4, 128], bf16, tag="vP01", name="vP01")
                nc.vector.tensor_tensor(vP01[:], fA[:, 0:4, :], fA[:, 4:8, :], ADD)
                fC = fp.tile([128, 2, 128], bf16, tag="fC", name="fC")
                nc.gpsimd.tensor_tensor(fC[:], xl[:, 24:26, :], xl[:, 26:28, :], ADD)
                fB = fp.tile([128, 4, 128], bf16, tag="fB", name="fB")
                nc.gpsimd.tensor_tensor(fB[:], xl[:, 16:20, :], xl[:, 20:24, :], ADD)
                return vQ01, fA, vP01, fB, fC

            def emit_tree(bt, tag, bpfx, vp01, v0src, v1src, warmup=False):
                """Depth-2 Karatsuba Toeplitz-1024 tree -> tX [128,1024] bf16."""
                TL = pp.tile([128, 1536], f32, tag=tag, name=tag + "L")
                if warmup:
                    for _ in range(4):
                        nc.tensor.matmul(
                            TL[:, 1024:1536], warm[:, 0:128], warm[:],
                            start=True, stop=True,
                        )
                t512(TL[:, 0:512], bpfx + "T0", lambda t: vp01[:, t, :])
                t512(TL[:, 512:1024], bpfx + "V", v1src)
                sXp = midp.tile([128, 512], bf16, tag="s" + bpfx, name="s" + bpfx)
                nc.scalar.copy(sXp[:], TL[:, 0:512])
                tX = midp.tile([128, 1024], bf16, tag="t" + bpfx, name="t" + bpfx)
                nc.vector.tensor_tensor(tX[:, 0:512], sXp[:], TL[:, 512:1024], ADD)
                t512(TL[:, 1024:1536], bpfx + "W", v0src)
                nc.vector.tensor_tensor(tX[:, 512:1024], sXp[:], TL[:, 1024:1536], ADD)
                return tX

            def emit_C(xl, fC):
                TCg = pp.tile([128, 768], f32, tag="TC", name="TC")
                t256(TCg[:, 0:256], "CQ", lambda t: xl[:, 26 + t, :])
                t256(TCg[:, 256:512], "CP", lambda t: fC[:, t, :])
                sCP = midp.tile([128, 256], bf16, tag="sCP", name="sCP")
                nc.scalar.copy(sCP[:], TCg[:, 256:512])
                ynC = ynp.tile([128, 512], bf16, tag="ynC", name="ynC")
                nc.vector.tensor_tensor(ynC[:, 0:256], sCP[:], TCg[:, 0:256], SUB)
                t256(TCg[:, 512:768], "CR", lambda t: xl[:, 24 + t, :])
                nc.vector.tensor_tensor(ynC[:, 256:512], sCP[:], TCg[:, 512:768], ADD)
                return ynC

            def emit_B(xl, fB):
                T2g = pp.tile([128, 1536], f32, tag="T2", name="T2B")
                t512(T2g[:, 0:512], "BQ", lambda t: xl[:, 20 + t, :])
                t512(T2g[:, 512:1024], "BP", lambda t: fB[:, t, :])
                sBP = midp.tile([128, 512], bf16, tag="sBP", name="sBP")
                nc.scalar.copy(sBP[:], T2g[:, 512:1024])
                ynB = ynp.tile([128, 1024], bf16, tag="ynB", name="ynB")
                nc.vector.tensor_tensor(ynB[:, 0:512], sBP[:], T2g[:, 0:512], SUB)
                t512(T2g[:, 1024:1536], "BR", lambda t: xl[:, 16 + t, :])
                nc.vector.tensor_tensor(ynB[:, 512:1024], sBP[:], T2g[:, 1024:1536], ADD)
                return ynB

            def emit_D(psum, xl):
                # D shares the R generation's T1 tile (banks are disjoint);
                # ACT evacuates so cpp is SBUF-only and nothing D-related
                # gates the next block's T1 reuse
                t512(psum, "D", lambda t: xl[:, 28 + t, :])
                sD = midp.tile([128, 512], bf16, tag="sD", name="sD")
                nc.scalar.copy(sD[:], psum)
                return sD

            def emit_unfold_mid(TD, ynB, ynC):
                cpp = midp.tile([128, 1024], bf16, tag="cpp", name="cpp")
                nc.vector.tensor_tensor(cpp[:, 0:512], TD[:], ynC[:], ADD)
                nc.vector.tensor_tensor(cpp[:, 512:1024], TD[:], ynC[:], SUB)
                u1 = midp.tile([128, 1024], bf16, tag="u1", name="u1")
                nc.vector.tensor_tensor(u1[:], cpp[:], ynB[:], ADD)
                u2 = midp.tile([128, 1024], bf16, tag="u2", name="u2")
                nc.vector.tensor_tensor(u2[:], cpp[:], ynB[:], SUB)
                return u1, u2

            def emit_finals(b0, u1, u2, ynA, dve_share=False):
                # Pool by default; DVE takes half in the drain-critical blocks,
                # which also get per-1024 DMAs for finer tail interleave
                if dve_share:
                    for seg, usrc, alu, eng in (
                        (0, u1, ADD, nc.gpsimd), (1, u2, ADD, nc.gpsimd),
                        (2, u1, SUB, nc.gpsimd), (3, u2, SUB, nc.gpsimd),
                    ):
                        o = op.tile([128, 1024], bf16, tag="o", name="o", bufs=4)
                        eng.tensor_tensor(
                            o[:], usrc[:], ynA[:, 1024 * (seg % 2) : 1024 * (seg % 2) + 1024], alu
                        )
                        nc.sync.dma_start(
                            out_d[b0 : b0 + 128, 1024 * seg : 1024 * seg + 1024], o[:]
                        )
                    return
                oL = op.tile([128, 2048], bf16, tag="oL", name="oL")
                nc.gpsimd.tensor_tensor(oL[:, 0:1024], u1[:], ynA[:, 0:1024], ADD)
                nc.gpsimd.tensor_tensor(oL[:, 1024:2048], u2[:], ynA[:, 1024:2048], ADD)
                nc.sync.dma_start(out_d[b0 : b0 + 128, 0:2048], oL[:])
                oR = op.tile([128, 2048], bf16, tag="oR", name="oR")
                nc.gpsimd.tensor_tensor(oR[:, 0:1024], u1[:], ynA[:, 0:1024], SUB)
                nc.gpsimd.tensor_tensor(oR[:, 1024:2048], u2[:], ynA[:, 1024:2048], SUB)
                nc.sync.dma_start(out_d[b0 : b0 + 128, 2048:4096], oR[:])

            pending = None
            xls = {0: xl0, 1: xl1}
            folds_cur = emit_folds(xl0)
            for bt in range(NB):
                b0 = 128 * bt
                xl = xls[bt]
                vQ01, fA, vP01, fB, fC = folds_cur
                last = bt == NB - 1
                if not last:
                    if pending is not None:
                        pending()
                    tQ = emit_tree(
                        bt, "T1", "Q", vQ01,
                        lambda t: xl[:, 8 + t, :], lambda t: xl[:, 12 + t, :],
                        warmup=(bt == 0),
                    )
                    tP = emit_tree(
                        bt, "T2", "P", vP01,
                        lambda t: fA[:, t, :], lambda t: fA[:, 4 + t, :],
                    )
                    ynA = ynp.tile([128, 2048], bf16, tag="ynA", name="ynA")
                    nc.vector.tensor_tensor(ynA[:, 0:1024], tP[:], tQ[:], SUB)
                    TRp = pp.tile([128, 1536], f32, tag="T1", name="T1RD")
                    t1024(TRp[:, 0:1024], "AR", lambda t: xl[:, t, :])
                    for h in (0, 1):
                        nc.vector.tensor_tensor(
                            ynA[:, 1024 + 512 * h : 1536 + 512 * h],
                            tP[:, 512 * h : 512 * h + 512],
                            TRp[:, 512 * h : 512 * h + 512],
                            ADD,
                        )
                    sD = emit_D(TRp[:, 1024:1536], xl)
                    # next block's input + folds right behind the hi-halves in
                    # the DVE queue so block bt+1's first group is never gated
                    if bt + 2 < NB:
                        xls[bt + 2] = xlp.tile([128, 32, 128], bf16, tag="xl", name="xl")
                        nc.sync.dma_start(xls[bt + 2][:], xl_t[:, bt + 2, :, :])
                    folds_cur = emit_folds(xls[bt + 1])
                    ynB = emit_B(xl, fB)
                    ynC = emit_C(xl, fC)
                    cpp = midp.tile([128, 1024], bf16, tag="cpp", name="cpp")
                    nc.vector.tensor_tensor(cpp[:, 0:512], sD[:], ynC[:], ADD)
                    nc.vector.tensor_tensor(cpp[:, 512:1024], sD[:], ynC[:], SUB)
                    u1 = midp.tile([128, 1024], bf16, tag="u1", name="u1")
                    nc.vector.tensor_tensor(u1[:], cpp[:], ynB[:], ADD)
                    u2 = midp.tile([128, 1024], bf16, tag="u2", name="u2")
                    nc.vector.tensor_tensor(u2[:], cpp[:], ynB[:], SUB)
                    dve_share = bt >= NB - 3
                    pending = (
                        lambda b0=b0, u1=u1, u2=u2, ynA=ynA, d=dve_share: emit_finals(
                            b0, u1, u2, ynA, dve_share=d
                        )
                    )
                else:
                    # last block: C/B/D first, then Q/P trees, R last; finals
                    # for the y_lo half stream during R, y_hi half-granular
                    if pending is not None:
                        pending()
                        pending = None
                    ynC = emit_C(xl, fC)
                    ynB = emit_B(xl, fB)
                    TDg = pp.tile([128, 1536], f32, tag="T1", name="T1D")
                    sD = emit_D(TDg[:, 0:512], xl)
                    cpp = midp.tile([128, 1024], bf16, tag="cpp", name="cpp")
                    nc.vector.tensor_tensor(cpp[:, 0:512], sD[:], ynC[:], ADD)
                    nc.vector.tensor_tensor(cpp[:, 512:1024], sD[:], ynC[:], SUB)
                    u1 = midp.tile([128, 1024], bf16, tag="u1", name="u1")
                    nc.vector.tensor_tensor(u1[:], cpp[:], ynB[:], ADD)
                    u2 = midp.tile([128, 1024], bf16, tag="u2", name="u2")
                    nc.vector.tensor_tensor(u2[:], cpp[:], ynB[:], SUB)
                    tQ = emit_tree(
                        bt, "T1", "Q", vQ01,
                        lambda t: xl[:, 8 + t, :], lambda t: xl[:, 12 + t, :],
                    )
                    tP = emit_tree(
                        bt, "T2", "P", vP01,
                        lambda t: fA[:, t, :], lambda t: fA[:, 4 + t, :],
                    )
                    ynA = ynp.tile([128, 2048], bf16, tag="ynA", name="ynA")
                    nc.vector.tensor_tensor(ynA[:, 0:1024], tP[:], tQ[:], SUB)
                    for seg, alu, eng in ((0, ADD, "v"), (2, SUB, "p")):
                        o = op.tile([128, 1024], bf16, tag="o", name="o", bufs=4)
                        if eng == "v":
                            nc.vector.tensor_tensor(o[:], u1[:], ynA[:, 0:1024], alu)
                        else:
                            nc.gpsimd.tensor_tensor(o[:], u1[:], ynA[:, 0:1024], alu)
                        nc.sync.dma_start(
                            out_d[b0 : b0 + 128, 1024 * seg : 1024 * seg + 1024], o[:]
                        )
                    TRp = pp.tile([128, 1536], f32, tag="T2", name="T2R")
                    t1024(TRp[:, 0:1024], "AR", lambda t: xl[:, t, :])
                    for h in (0, 1):
                        nc.vector.tensor_tensor(
                            ynA[:, 1024 + 512 * h : 1536 + 512 * h],
                            tP[:, 512 * h : 512 * h + 512],
                            TRp[:, 512 * h : 512 * h + 512],
                            ADD,
                        )
                        for seg, alu, eng in ((1, ADD, "v"), (3, SUB, "p")):
                            o = op.tile([128, 512], bf16, tag="oh", name="oh", bufs=4)
                            args = (
                                o[:],
                                u2[:, 512 * h : 512 * h + 512],
                                ynA[:, 1024 + 512 * h : 1536 + 512 * h],
                                alu,
                            )
                            if eng == "v":
                                nc.vector.tensor_tensor(*args)
                            else:
                                nc.gpsimd.tensor_tensor(*args)
                            nc.sync.dma_start(
                                out_d[
                                    b0 : b0 + 128,
                                    1024 * seg + 512 * h : 1024 * seg + 512 * h + 512,
                                ],
                                o[:],
                            )
            if pending is not None:
                pending()

    nc.compile()
    return nc


def _get_nc():
    if "nc" not in _STATE:
        _STATE["nc"] = _build()
    return _STATE["nc"]


def _prep_inputs(x, w):
    x = np.ascontiguousarray(x, dtype=np.float32)
    w = np.asarray(w, dtype=np.float32)
    bands = _make_bands(w)
    in_maps = []
    for i in range(N_CORES):
        xl = _fold_x(x[i * B_SHARD : (i + 1) * B_SHARD])
        in_maps.append({"xl": xl.reshape(128, 32 * B_SHARD), "bands": bands})
    return in_maps


def kernel(x, w, _trace=False):
    from concourse.bass_utils import run_bass_kernel_spmd

    nc = _get_nc()
    in_maps = _prep_inputs(x, w)
    res = run_bass_kernel_spmd(nc, in_maps, list(range(N_CORES)), trace=_trace)
    out = np.concatenate(
        [np.asarray(res.results[i]["out"]).astype(np.float32) for i in range(N_CORES)],
        axis=0,
    )
    if _trace:
        _STATE["last_result"] = res
    return out


# revision 8
# speedup vs baseline: 1.0248x; 1.0026x over previous
"""Circulant matmul for TRN2 v3: CRT + Gauss 3-mult negacyclic splits with
depth-2 Karatsuba on two of the three nega2048 trees, bf16 matmuls,
host-precomputed x-folds and pre-scaled band operators.

out[b, r] = sum_c x[b,c] w[(c-r) mod N] = cyclic_conv(x_row, v), v = roll(rev(w),1).

CRT: z^4096-1 -> leaves nega2048 (A), nega1024 (B), nega512 (C), cyc512 (D).
nega-n via Gauss 3-mult on its [[T,-U],[U,T]] structure:
  P = T(a0+a1), Q = (T+U)a1, R = (U-T)a0, y = [P-Q, P+R].
For A, the P and Q Toeplitz-1024 products are themselves Karatsuba'd into
3 Toeplitz-512 products each (p = T0(v0+v1), top = p + (V-T0)v1,
bot = p + (W-T0)v0); R stays dense -- engine balance: deeper splits save PE
but overload DVE/Pool with combines. PE: 30208 cyc/128-row block.

Host sends bf16: x CRT-leaf chunks [128, 8, 32, 128] (8 MB/core, per-block
contiguous) + pre-scaled band blob (3.1 MB). All CRT/Gauss scales folded
into bands. ACT evacuates shared Kara leaves + cyc512, DVE does PSUM
combines, Pool does folds fB/fC and most final output combines. PSUM map:
T1 (Q-leaves -> R||cyc512), T2 (P-leaves -> B-leaves), TC (nega512) --
ordered so no combine chain ever gates the next block's first matmul
group; per-block PE order Q, P, R, D, B, C. The last two blocks split
finals across DVE/Pool at fine granularity so the tail output DMAs
overlap the final A-tree matmul stream.
"""

import sys

sys.path.insert(0, "/opt/trn_rl_repo")

import numpy as np
import ml_dtypes

BF16 = ml_dtypes.bfloat16
N = 4096
B = 8192
N_CORES = 8
B_SHARD = B // N_CORES  # 1024
NB = B_SHARD // 128     # 8 row-tiles per core

# band blob layout: name -> (m, offset); width = 2m-128; consumption order
_BANDS = {
    "QT0": (512, 0), "QV": (512, 896), "QW": (512, 1792),
    "PT0": (512, 2688), "PV": (512, 3584), "PW": (512, 4480),
    "AR": (1024, 5376),
    "D": (512, 7296), "BQ": (512, 8192), "BP": (512, 9088),
    "CQ": (256, 9984), "CP": (256, 10368), "BR": (512, 10752),
    "CR": (256, 11648),
}
WTOT = 12032

_STATE = {}


# ---------------------------------------------------------------- host math
def _sub_symbol(t, m, shift, h):
    c = m - 1
    return t[c + shift - (h - 1) : c + shift + h].copy()


def _band_from_symbol(t, m):
    u = np.arange(2 * m - 128)[None, :]
    p = np.arange(128)[:, None]
    return t[u - p + 127]


def _make_bands(w):
    v = np.roll(w[::-1].astype(np.float64), 1)
    vm = v[:2048] - v[2048:]
    vp = v[:2048] + v[2048:]
    vpm = vp[:1024] - vp[1024:]
    vpp = vp[:1024] + vp[1024:]
    vppm = vpp[:512] - vpp[512:]
    vppp = vpp[:512] + vpp[512:]

    def skew_sym(b):
        n = len(b)
        t = np.empty(2 * n - 1)
        t[n - 1 :] = b
        t[: n - 1] = -b[1:]
        return t

    def cyc_sym(b):
        n = len(b)
        t = np.empty(2 * n - 1)
        t[n - 1 :] = b
        t[: n - 1] = b[1:]
        return t

    blob = np.zeros((128, WTOT), np.float64)

    def put(name, sym):
        m, off = _BANDS[name]
        blob[:, off : off + 2 * m - 128] = _band_from_symbol(sym, m)

    # A = nega2048: T1024-level Gauss operators (scale 0.5 folded in)
    full = skew_sym(vm)
    tT = _sub_symbol(full, 2048, 0, 1024)
    tU = _sub_symbol(full, 2048, 1024, 1024)
    sP = 0.5 * tT
    sQ = 0.5 * (tT + tU)
    sR = 0.5 * (tU - tT)
    for nm, sym in (("P", sP), ("Q", sQ)):
        t0 = _sub_symbol(sym, 1024, 0, 512)
        tV = _sub_symbol(sym, 1024, -512, 512)
        tW = _sub_symbol(sym, 1024, 512, 512)
        put(nm + "T0", t0)
        put(nm + "V", tV - t0)
        put(nm + "W", tW - t0)
    put("AR", sR)

    for nm, bsrc, scale in [("B", vpm, 0.25), ("C", vppm, 0.125)]:
        n = len(bsrc)
        m = n // 2
        fl = skew_sym(bsrc)
        t1 = _sub_symbol(fl, n, 0, m)
        t2 = _sub_symbol(fl, n, m, m)
        put(nm + "P", scale * t1)
        put(nm + "Q", scale * (t1 + t2))
        put(nm + "R", scale * (t2 - t1))
    put("D", 0.125 * cyc_sym(vppp))
    return blob.astype(BF16)


def _fold_x(x_shard):
    """[1024, 4096] f32 -> [128, 8, 32, 128] bf16 chunk-partitioned CRT
    leaves, per-block contiguous."""
    xs = x_shard.astype(np.float32)
    xm = xs[:, :2048] - xs[:, 2048:]
    xp = xs[:, :2048] + xs[:, 2048:]
    xpm = xp[:, :1024] - xp[:, 1024:]
    xpp = xp[:, :1024] + xp[:, 1024:]
    xppm = xpp[:, :512] - xpp[:, 512:]
    xppp = xpp[:, :512] + xpp[:, 512:]
    L = np.concatenate([xm, xpm, xppm, xppp], axis=1)  # [1024, 4096]
    a = L.reshape(1024, 32, 128).transpose(2, 1, 0)  # [128, 32, 1024]
    a = a.reshape(128, 32, 8, 128).transpose(0, 2, 1, 3)
    return np.ascontiguousarray(a.astype(BF16))


# ---------------------------------------------------------------- device
def _build():
    import concourse.bacc as bacc
    import concourse.mybir as mybir
    import concourse.tile as tile

    f32 = mybir.dt.float32
    bf16 = mybir.dt.bfloat16
    ADD = mybir.AluOpType.add
    SUB = mybir.AluOpType.subtract

    nc = bacc.Bacc("TRN2", target_bir_lowering=False, debug=False)
    xl_d = nc.declare_dram_parameter("xl", [128, 32 * B_SHARD], bf16, isOutput=False)
    bands_d = nc.declare_dram_parameter("bands", [128, WTOT], bf16, isOutput=False)
    out_d = nc.declare_dram_parameter("out", [B_SHARD, N], bf16, isOutput=True)

    xl_t = xl_d[:].rearrange("p (k a b) -> p k a b", k=NB, a=32)  # [128, 8, 32, 128]

    with tile.TileContext(nc) as tc:
        with (
            tc.tile_pool(name="const", bufs=1) as constp,
            tc.tile_pool(name="xl", bufs=2) as xlp,
            tc.tile_pool(name="fold", bufs=3) as fp,
            tc.tile_pool(name="yn", bufs=4) as ynp,
            tc.tile_pool(name="mid", bufs=4) as midp,
            tc.tile_pool(name="outp", bufs=3) as op,
            tc.tile_pool(name="ps", bufs=1, space="PSUM") as pp,
        ):
            band = constp.tile([128, WTOT], bf16)
            warm = constp.tile([128, 512], bf16, name="warm")
            nc.gpsimd.memset(warm[:], 0.0)

            def bsl(name, lo, width):
                off = _BANDS[name][1]
                return band[:, off + lo : off + lo + width]

            # first-block DMAs in consumption order; xl(1) last
            xl0 = xlp.tile([128, 32, 128], bf16, tag="xl", name="xl0")
            nc.sync.dma_start(xl0[:, 8:16, :], xl_t[:, 0, 8:16, :])
            nc.sync.dma_start(band[:, 0:896], bands_d[:, 0:896])        # QT0 band
            nc.sync.dma_start(xl0[:, 0:8, :], xl_t[:, 0, 0:8, :])
            nc.sync.dma_start(band[:, 896:2688], bands_d[:, 896:2688])  # QV/QW
            nc.sync.dma_start(band[:, 2688:5376], bands_d[:, 2688:5376])  # P bands
            nc.sync.dma_start(band[:, 5376:7296], bands_d[:, 5376:7296])  # AR band
            nc.sync.dma_start(xl0[:, 16:32, :], xl_t[:, 0, 16:32, :])
            nc.sync.dma_start(band[:, 7296:WTOT], bands_d[:, 7296:WTOT])  # D/B/C
            xl1 = xlp.tile([128, 32, 128], bf16, tag="xl", name="xl1")
            nc.sync.dma_start(xl1[:], xl_t[:, 1, :, :])

            def t1024(psum, bname, src):
                for j in range(2):
                    for t in range(8):
                        nc.tensor.matmul(
                            psum[:, 512 * j : 512 * j + 512],
                            src(t),
                            bsl(bname, (7 - t) * 128 + 512 * j, 512),
                            start=(t == 0),
                            stop=(t == 7),
                        )

            def t512(psum, bname, src):
                for t in range(4):
                    nc.tensor.matmul(
                        psum[:],
                        src(t),
                        bsl(bname, (3 - t) * 128, 512),
                        start=(t == 0),
                        stop=(t == 3),
                    )

            def t256(psum, bname, src):
                for t in range(2):
                    nc.tensor.matmul(
                        psum[:],
                        src(t),
                        bsl(bname, (1 - t) * 128, 256),
                        start=(t == 0),
                        stop=(t == 1),
                    )

            def emit_folds(xl):
                vQ01 = fp.tile([128, 4, 128], bf16, tag="vQ01", name="vQ01")
                nc.vector.tensor_tensor(vQ01[:], xl[:, 8:12, :], xl[:, 12:16, :], ADD)
                fA = fp.tile([128, 8, 128], bf16, tag="fA", name="fA")
                nc.vector.tensor_tensor(fA[:], xl[:, 0:8, :], xl[:, 8:16, :], ADD)
                vP01 = fp.tile([128, 4, 128], bf16, tag="vP01", name="vP01")
                nc.vector.tensor_tensor(vP01[:], fA[:, 0:4, :], fA[:, 4:8, :], ADD)
                fC = fp.tile([128, 2, 128], bf16, tag="fC", name="fC")
                nc.gpsimd.tensor_tensor(fC[:], xl[:, 24:26, :], xl[:, 26:28, :], ADD)
                fB = fp.tile([128, 4, 128], bf16, tag="fB", name="fB")
                nc.gpsimd.tensor_tensor(fB[:], xl[:, 16:20, :], xl[:, 20:24, :], ADD)
                return vQ01, fA, vP01, fB, fC

            def emit_tree(bt, tag, bpfx, vp01, v0src, v1src, warmup=False):
                """Depth-2 Karatsuba Toeplitz-1024 tree -> tX [128,1024] bf16."""
                TL = pp.tile([128, 1536], f32, tag=tag, name=tag + "L")
                if warmup:
                    for _ in range(4):
                        nc.tensor.matmul(
                            TL[:, 1024:1536], warm[:, 0:128], warm[:],
                            start=True, stop=True,
                        )
                t512(TL[:, 0:512], bpfx + "T0", lambda t: vp01[:, t, :])
                t512(TL[:, 512:1024], bpfx + "V", v1src)
                sXp = midp.tile([128, 512], bf16, tag="s" + bpfx, name="s" + bpfx)
                nc.scalar.copy(sXp[:], TL[:, 0:512])
                sXv = midp.tile([128, 1024], bf16, tag="sv" + bpfx, name="sv" + bpfx)
                nc.scalar.copy(sXv[:, 0:512], TL[:, 512:1024])
                tX = midp.tile([128, 1024], bf16, tag="t" + bpfx, name="t" + bpfx)
                nc.vector.tensor_tensor(tX[:, 0:512], sXp[:], sXv[:, 0:512], ADD)
                t512(TL[:, 1024:1536], bpfx + "W", v0src)
                nc.scalar.copy(sXv[:, 512:1024], TL[:, 1024:1536])
                nc.vector.tensor_tensor(tX[:, 512:1024], sXp[:], sXv[:, 512:1024], ADD)
                return tX

            def emit_C(xl, fC):
                TCg = pp.tile([128, 768], f32, tag="TC", name="TC")
                t256(TCg[:, 0:256], "CQ", lambda t: xl[:, 26 + t, :])
                t256(TCg[:, 256:512], "CP", lambda t: fC[:, t, :])
                sCP = midp.tile([128, 256], bf16, tag="sCP", name="sCP")
                nc.scalar.copy(sCP[:], TCg[:, 256:512])
                ynC = ynp.tile([128, 512], bf16, tag="ynC", name="ynC")
                nc.vector.tensor_tensor(ynC[:, 0:256], sCP[:], TCg[:, 0:256], SUB)
                t256(TCg[:, 512:768], "CR", lambda t: xl[:, 24 + t, :])
                nc.vector.tensor_tensor(ynC[:, 256:512], sCP[:], TCg[:, 512:768], ADD)
                return ynC

            def emit_B(xl, fB):
                T2g = pp.tile([128, 1536], f32, tag="T2", name="T2B")
                t512(T2g[:, 0:512], "BQ", lambda t: xl[:, 20 + t, :])
                t512(T2g[:, 512:1024], "BP", lambda t: fB[:, t, :])
                sBP = midp.tile([128, 512], bf16, tag="sBP", name="sBP")
                nc.scalar.copy(sBP[:], T2g[:, 512:1024])
                ynB = ynp.tile([128, 1024], bf16, tag="ynB", name="ynB")
                nc.vector.tensor_tensor(ynB[:, 0:512], sBP[:], T2g[:, 0:512], SUB)
                t512(T2g[:, 1024:1536], "BR", lambda t: xl[:, 16 + t, :])
                nc.vector.tensor_tensor(ynB[:, 512:1024], sBP[:], T2g[:, 1024:1536], ADD)
                return ynB

            def emit_D(psum, xl):
                # D shares the R generation's T1 tile (banks are disjoint);
                # ACT evacuates so cpp is SBUF-only and nothing D-related
                # gates the next block's T1 reuse
                t512(psum, "D", lambda t: xl[:, 28 + t, :])
                sD = midp.tile([128, 512], bf16, tag="sD", name="sD")
                nc.scalar.copy(sD[:], psum)
                return sD

            def emit_unfold_mid(TD, ynB, ynC):
                cpp = midp.tile([128, 1024], bf16, tag="cpp", name="cpp")
                nc.vector.tensor_tensor(cpp[:, 0:512], TD[:], ynC[:], ADD)
                nc.vector.tensor_tensor(cpp[:, 512:1024], TD[:], ynC[:], SUB)
                u1 = midp.tile([128, 1024], bf16, tag="u1", name="u1")
                nc.vector.tensor_tensor(u1[:], cpp[:], ynB[:], ADD)
                u2 = midp.tile([128, 1024], bf16, tag="u2", name="u2")
                nc.vector.tensor_tensor(u2[:], cpp[:], ynB[:], SUB)
                return u1, u2

            def emit_finals(b0, u1, u2, ynA, dve_share=False):
                # Pool by default; DVE takes half in the drain-critical blocks,
                # which also get per-1024 DMAs for finer tail interleave
                if dve_share:
                    for seg, usrc, alu, eng in (
                        (0, u1, ADD, nc.gpsimd), (1, u2, ADD, nc.gpsimd),
                        (2, u1, SUB, nc.gpsimd), (3, u2, SUB, nc.gpsimd),
                    ):
                        o = op.tile([128, 1024], bf16, tag="o", name="o", bufs=4)
                        eng.tensor_tensor(
                            o[:], usrc[:], ynA[:, 1024 * (seg % 2) : 1024 * (seg % 2) + 1024], alu
                        )
                        nc.sync.dma_start(
                            out_d[b0 : b0 + 128, 1024 * seg : 1024 * seg + 1024], o[:]
                        )
                    return
                oL = op.tile([128, 2048], bf16, tag="oL", name="oL")
                nc.gpsimd.tensor_tensor(oL[:, 0:1024], u1[:], ynA[:, 0:1024], ADD)
                nc.gpsimd.tensor_tensor(oL[:, 1024:2048], u2[:], ynA[:, 1024:2048], ADD)
                nc.sync.dma_start(out_d[b0 : b0 + 128, 0:2048], oL[:])
                oR = op.tile([128, 2048], bf16, tag="oR", name="oR")
                nc.gpsimd.tensor_tensor(oR[:, 0:1024], u1[:], ynA[:, 0:1024], SUB)
                nc.gpsimd.tensor_tensor(oR[:, 1024:2048], u2[:], ynA[:, 1024:2048], SUB)
                nc.sync.dma_start(out_d[b0 : b0 + 128, 2048:4096], oR[:])

            pending = None
            xls = {0: xl0, 1: xl1}
            folds_cur = emit_folds(xl0)
            for bt in range(NB):
                b0 = 128 * bt
                xl = xls[bt]
                vQ01, fA, vP01, fB, fC = folds_cur
                last = bt == NB - 1
                if not last:
                    if pending is not None:
                        pending()
                    tQ = emit_tree(
                        bt, "T1", "Q", vQ01,
                        lambda t: xl[:, 8 + t, :], lambda t: xl[:, 12 + t, :],
                        warmup=(bt == 0),
                    )
                    tP = emit_tree(
                        bt, "T2", "P", vP01,
                        lambda t: fA[:, t, :], lambda t: fA[:, 4 + t, :],
                    )
                    ynA = ynp.tile([128, 2048], bf16, tag="ynA", name="ynA")
                    nc.vector.tensor_tensor(ynA[:, 0:1024], tP[:], tQ[:], SUB)
                    TRp = pp.tile([128, 1536], f32, tag="T1", name="T1RD")
                    t1024(TRp[:, 0:1024], "AR", lambda t: xl[:, t, :])
                    sR = midp.tile([128, 1024], bf16, tag="sRR", name="sRR")
                    for h in (0, 1):
                        nc.scalar.copy(
                            sR[:, 512 * h : 512 * h + 512],
                            TRp[:, 512 * h : 512 * h + 512],
                        )
                        nc.vector.tensor_tensor(
                            ynA[:, 1024 + 512 * h : 1536 + 512 * h],
                            tP[:, 512 * h : 512 * h + 512],
                            sR[:, 512 * h : 512 * h + 512],
                            ADD,
                        )
                    sD = emit_D(TRp[:, 1024:1536], xl)
                    # next block's input + folds right behind the hi-halves in
                    # the DVE queue so block bt+1's first group is never gated
                    if bt + 2 < NB:
                        xls[bt + 2] = xlp.tile([128, 32, 128], bf16, tag="xl", name="xl")
                        nc.sync.dma_start(xls[bt + 2][:], xl_t[:, bt + 2, :, :])
                    folds_cur = emit_folds(xls[bt + 1])
                    ynB = emit_B(xl, fB)
                    ynC = emit_C(xl, fC)
                    cpp = midp.tile([128, 1024], bf16, tag="cpp", name="cpp")
                    nc.vector.tensor_tensor(cpp[:, 0:512], sD[:], ynC[:], ADD)
                    nc.vector.tensor_tensor(cpp[:, 512:1024], sD[:], ynC[:], SUB)
                    u1 = midp.tile([128, 1024], bf16, tag="u1", name="u1")
                    nc.vector.tensor_tensor(u1[:], cpp[:], ynB[:], ADD)
                    u2 = midp.tile([128, 1024], bf16, tag="u2", name="u2")
                    nc.vector.tensor_tensor(u2[:], cpp[:], ynB[:], SUB)
                    dve_share = bt >= NB - 3
                    pending = (
                        lambda b0=b0, u1=u1, u2=u2, ynA=ynA, d=dve_share: emit_finals(
                            b0, u1, u2, ynA, dve_share=d
                        )
                    )
                else:
                    # last block: C/B/D first, then Q/P trees, R last; finals
                    # for the y_lo half stream during R, y_hi half-granular
                    if pending is not None:
                        pending()
                        pending = None
                    ynC = emit_C(xl, fC)
                    ynB = emit_B(xl, fB)
                    TDg = pp.tile([128, 1536], f32, tag="T1", name="T1D")
                    sD = emit_D(TDg[:, 0:512], xl)
                    cpp = midp.tile([128, 1024], bf16, tag="cpp", name="cpp")
                    nc.vector.tensor_tensor(cpp[:, 0:512], sD[:], ynC[:], ADD)
                    nc.vector.tensor_tensor(cpp[:, 512:1024], sD[:], ynC[:], SUB)
                    u1 = midp.tile([128, 1024], bf16, tag="u1", name="u1")
                    nc.vector.tensor_tensor(u1[:], cpp[:], ynB[:], ADD)
                    u2 = midp.tile([128, 1024], bf16, tag="u2", name="u2")
                    nc.vector.tensor_tensor(u2[:], cpp[:], ynB[:], SUB)
                    tQ = emit_tree(
                        bt, "T1", "Q", vQ01,
                        lambda t: xl[:, 8 + t, :], lambda t: xl[:, 12 + t, :],
                    )
                    tP = emit_tree(
                        bt, "T2", "P", vP01,
                        lambda t: fA[:, t, :], lambda t: fA[:, 4 + t, :],
                    )
                    ynA = ynp.tile([128, 2048], bf16, tag="ynA", name="ynA")
                    nc.vector.tensor_tensor(ynA[:, 0:1024], tP[:], tQ[:], SUB)
                    for seg, alu, eng in ((0, ADD, "v"), (2, SUB, "p")):
                        o = op.tile([128, 1024], bf16, tag="o", name="o", bufs=4)
                        if eng == "v":
                            nc.vector.tensor_tensor(o[:], u1[:], ynA[:, 0:1024], alu)
                        else:
                            nc.gpsimd.tensor_tensor(o[:], u1[:], ynA[:, 0:1024], alu)
                        nc.sync.dma_start(
                            out_d[b0 : b0 + 128, 1024 * seg : 1024 * seg + 1024], o[:]
                        )
                    TRp = pp.tile([128, 1536], f32, tag="T2", name="T2R")
                    t1024(TRp[:, 0:1024], "AR", lambda t: xl[:, t, :])
                    for h in (0, 1):
                        nc.vector.tensor_tensor(
                            ynA[:, 1024 + 512 * h : 1536 + 512 * h],
                            tP[:, 512 * h : 512 * h + 512],
                            TRp[:, 512 * h : 512 * h + 512],
                            ADD,
                        )
                        for seg, alu, eng in ((1, ADD, "v"), (3, SUB, "p")):
                            o = op.tile([128, 512], bf16, tag="oh", name="oh", bufs=4)
                            args = (
                                o[:],
                                u2[:, 512 * h : 512 * h + 512],
                                ynA[:, 1024 + 512 * h : 1536 + 512 * h],
                                alu,
                            )
                            if eng == "v":
                                nc.vector.tensor_tensor(*args)
                            else:
                                nc.gpsimd.tensor_tensor(*args)
                            nc.sync.dma_start(
                                out_d[
                                    b0 : b0 + 128,
                                    1024 * seg + 512 * h : 1024 * seg + 512 * h + 512,
                                ],
                                o[:],
                            )
            if pending is not None:
                pending()

    nc.compile()
    return nc


def _get_nc():
    if "nc" not in _STATE:
        _STATE["nc"] = _build()
    return _STATE["nc"]


def _prep_inputs(x, w):
    x = np.ascontiguousarray(x, dtype=np.float32)
    w = np.asarray(w, dtype=np.float32)
    bands = _make_bands(w)
    in_maps = []
    for i in range(N_CORES):
        xl = _fold_x(x[i * B_SHARD : (i + 1) * B_SHARD])
        in_maps.append({"xl": xl.reshape(128, 32 * B_SHARD), "bands": bands})
    return in_maps


def kernel(x, w, _trace=False):
    from concourse.bass_utils import run_bass_kernel_spmd

    nc = _get_nc()
    in_maps = _prep_inputs(x, w)
    res = run_bass_kernel_spmd(nc, in_maps, list(range(N_CORES)), trace=_trace)
    out = np.concatenate(
        [np.asarray(res.results[i]["out"]).astype(np.float32) for i in range(N_CORES)],
        axis=0,
    )
    if _trace:
        _STATE["last_result"] = res
    return out


# revision 9
# speedup vs baseline: 1.0365x; 1.0115x over previous
"""Circulant matmul for TRN2 v3: CRT + Gauss 3-mult negacyclic splits with
depth-2 Karatsuba on two of the three nega2048 trees, bf16 matmuls,
host-precomputed x-folds and pre-scaled band operators.

out[b, r] = sum_c x[b,c] w[(c-r) mod N] = cyclic_conv(x_row, v), v = roll(rev(w),1).

CRT: z^4096-1 -> leaves nega2048 (A), nega1024 (B), nega512 (C), cyc512 (D).
nega-n via Gauss 3-mult on its [[T,-U],[U,T]] structure:
  P = T(a0+a1), Q = (T+U)a1, R = (U-T)a0, y = [P-Q, P+R].
For A, the P and Q Toeplitz-1024 products are themselves Karatsuba'd into
3 Toeplitz-512 products each (p = T0(v0+v1), top = p + (V-T0)v1,
bot = p + (W-T0)v0); R stays dense -- engine balance: deeper splits save PE
but overload DVE/Pool with combines. PE: 30208 cyc/128-row block.

Host sends bf16: x CRT-leaf chunks [128, 8, 32, 128] (8 MB/core, per-block
contiguous) + pre-scaled band blob (3.1 MB). All CRT/Gauss scales folded
into bands. ACT evacuates shared Kara leaves + cyc512, DVE does PSUM
combines, Pool does folds fB/fC and most final output combines. PSUM map:
T1 (Q-leaves -> R||cyc512), T2 (P-leaves -> B-leaves), TC (nega512) --
ordered so no combine chain ever gates the next block's first matmul
group; per-block PE order Q, P, R, D, B, C. The last two blocks split
finals across DVE/Pool at fine granularity so the tail output DMAs
overlap the final A-tree matmul stream.
"""

import sys

sys.path.insert(0, "/opt/trn_rl_repo")

import numpy as np
import ml_dtypes

BF16 = ml_dtypes.bfloat16
N = 4096
B = 8192
N_CORES = 8
B_SHARD = B // N_CORES  # 1024
NB = B_SHARD // 128     # 8 row-tiles per core

# band blob layout: name -> (m, offset); width = 2m-128; consumption order
_BANDS = {
    "QT0": (512, 0), "QV": (512, 896), "QW": (512, 1792),
    "PT0": (512, 2688), "PV": (512, 3584), "PW": (512, 4480),
    "AR": (1024, 5376),
    "D": (512, 7296), "BQ": (512, 8192), "BP": (512, 9088),
    "CQ": (256, 9984), "CP": (256, 10368), "BR": (512, 10752),
    "CR": (256, 11648),
}
WTOT = 12032

_STATE = {}


# ---------------------------------------------------------------- host math
def _sub_symbol(t, m, shift, h):
    c = m - 1
    return t[c + shift - (h - 1) : c + shift + h].copy()


def _band_from_symbol(t, m):
    u = np.arange(2 * m - 128)[None, :]
    p = np.arange(128)[:, None]
    return t[u - p + 127]


def _make_bands(w):
    v = np.roll(w[::-1].astype(np.float64), 1)
    vm = v[:2048] - v[2048:]
    vp = v[:2048] + v[2048:]
    vpm = vp[:1024] - vp[1024:]
    vpp = vp[:1024] + vp[1024:]
    vppm = vpp[:512] - vpp[512:]
    vppp = vpp[:512] + vpp[512:]

    def skew_sym(b):
        n = len(b)
        t = np.empty(2 * n - 1)
        t[n - 1 :] = b
        t[: n - 1] = -b[1:]
        return t

    def cyc_sym(b):
        n = len(b)
        t = np.empty(2 * n - 1)
        t[n - 1 :] = b
        t[: n - 1] = b[1:]
        return t

    blob = np.zeros((128, WTOT), np.float64)

    def put(name, sym):
        m, off = _BANDS[name]
        blob[:, off : off + 2 * m - 128] = _band_from_symbol(sym, m)

    # A = nega2048: T1024-level Gauss operators (scale 0.5 folded in)
    full = skew_sym(vm)
    tT = _sub_symbol(full, 2048, 0, 1024)
    tU = _sub_symbol(full, 2048, 1024, 1024)
    sP = 0.5 * tT
    sQ = 0.5 * (tT + tU)
    sR = 0.5 * (tU - tT)
    for nm, sym in (("P", sP), ("Q", sQ)):
        t0 = _sub_symbol(sym, 1024, 0, 512)
        tV = _sub_symbol(sym, 1024, -512, 512)
        tW = _sub_symbol(sym, 1024, 512, 512)
        put(nm + "T0", t0)
        put(nm + "V", tV - t0)
        put(nm + "W", tW - t0)
    put("AR", sR)

    for nm, bsrc, scale in [("B", vpm, 0.25), ("C", vppm, 0.125)]:
        n = len(bsrc)
        m = n // 2
        fl = skew_sym(bsrc)
        t1 = _sub_symbol(fl, n, 0, m)
        t2 = _sub_symbol(fl, n, m, m)
        put(nm + "P", scale * t1)
        put(nm + "Q", scale * (t1 + t2))
        put(nm + "R", scale * (t2 - t1))
    put("D", 0.125 * cyc_sym(vppp))
    return blob.astype(BF16)


def _fold_x(x_shard):
    """[1024, 4096] f32 -> [128, 8, 32, 128] bf16 chunk-partitioned CRT
    leaves, per-block contiguous."""
    xs = x_shard.astype(np.float32)
    xm = xs[:, :2048] - xs[:, 2048:]
    xp = xs[:, :2048] + xs[:, 2048:]
    xpm = xp[:, :1024] - xp[:, 1024:]
    xpp = xp[:, :1024] + xp[:, 1024:]
    xppm = xpp[:, :512] - xpp[:, 512:]
    xppp = xpp[:, :512] + xpp[:, 512:]
    L = np.concatenate([xm, xpm, xppm, xppp], axis=1)  # [1024, 4096]
    a = L.reshape(1024, 32, 128).transpose(2, 1, 0)  # [128, 32, 1024]
    a = a.reshape(128, 32, 8, 128).transpose(0, 2, 1, 3)
    return np.ascontiguousarray(a.astype(BF16))


# ---------------------------------------------------------------- device
def _build():
    import concourse.bacc as bacc
    import concourse.mybir as mybir
    import concourse.tile as tile

    f32 = mybir.dt.float32
    bf16 = mybir.dt.bfloat16
    ADD = mybir.AluOpType.add
    SUB = mybir.AluOpType.subtract

    nc = bacc.Bacc("TRN2", target_bir_lowering=False, debug=False)
    xl_d = nc.declare_dram_parameter("xl", [128, 32 * B_SHARD], bf16, isOutput=False)
    bands_d = nc.declare_dram_parameter("bands", [128, WTOT], bf16, isOutput=False)
    out_d = nc.declare_dram_parameter("out", [B_SHARD, N], bf16, isOutput=True)

    xl_t = xl_d[:].rearrange("p (k a b) -> p k a b", k=NB, a=32)  # [128, 8, 32, 128]

    with tile.TileContext(nc) as tc:
        with (
            tc.tile_pool(name="const", bufs=1) as constp,
            tc.tile_pool(name="xl", bufs=2) as xlp,
            tc.tile_pool(name="fold", bufs=3) as fp,
            tc.tile_pool(name="yn", bufs=4) as ynp,
            tc.tile_pool(name="mid", bufs=4) as midp,
            tc.tile_pool(name="outp", bufs=3) as op,
            tc.tile_pool(name="ps", bufs=1, space="PSUM") as pp,
        ):
            band = constp.tile([128, WTOT], bf16)
            warm = constp.tile([128, 512], bf16, name="warm")
            nc.gpsimd.memset(warm[:], 0.0)

            def bsl(name, lo, width):
                off = _BANDS[name][1]
                return band[:, off + lo : off + lo + width]

            # first-block DMAs in consumption order; xl(1) last
            xl0 = xlp.tile([128, 32, 128], bf16, tag="xl", name="xl0")
            nc.sync.dma_start(xl0[:, 8:16, :], xl_t[:, 0, 8:16, :])
            nc.sync.dma_start(band[:, 0:896], bands_d[:, 0:896])        # QT0 band
            nc.sync.dma_start(xl0[:, 0:8, :], xl_t[:, 0, 0:8, :])
            nc.sync.dma_start(band[:, 896:2688], bands_d[:, 896:2688])  # QV/QW
            nc.sync.dma_start(band[:, 2688:5376], bands_d[:, 2688:5376])  # P bands
            nc.sync.dma_start(band[:, 5376:7296], bands_d[:, 5376:7296])  # AR band
            nc.sync.dma_start(xl0[:, 16:32, :], xl_t[:, 0, 16:32, :])
            nc.sync.dma_start(band[:, 7296:WTOT], bands_d[:, 7296:WTOT])  # D/B/C
            xl1 = xlp.tile([128, 32, 128], bf16, tag="xl", name="xl1")
            nc.sync.dma_start(xl1[:], xl_t[:, 1, :, :])

            def t1024(psum, bname, src):
                for j in range(2):
                    for t in range(8):
                        nc.tensor.matmul(
                            psum[:, 512 * j : 512 * j + 512],
                            src(t),
                            bsl(bname, (7 - t) * 128 + 512 * j, 512),
                            start=(t == 0),
                            stop=(t == 7),
                        )

            def t512(psum, bname, src):
                for t in range(4):
                    nc.tensor.matmul(
                        psum[:],
                        src(t),
                        bsl(bname, (3 - t) * 128, 512),
                        start=(t == 0),
                        stop=(t == 3),
                    )

            def t256(psum, bname, src):
                for t in range(2):
                    nc.tensor.matmul(
                        psum[:],
                        src(t),
                        bsl(bname, (1 - t) * 128, 256),
                        start=(t == 0),
                        stop=(t == 1),
                    )

            def emit_folds(xl):
                vQ01 = fp.tile([128, 4, 128], bf16, tag="vQ01", name="vQ01")
                nc.vector.tensor_tensor(vQ01[:], xl[:, 8:12, :], xl[:, 12:16, :], ADD)
                fA = fp.tile([128, 8, 128], bf16, tag="fA", name="fA")
                nc.vector.tensor_tensor(fA[:], xl[:, 0:8, :], xl[:, 8:16, :], ADD)
                vP01 = fp.tile([128, 4, 128], bf16, tag="vP01", name="vP01")
                nc.vector.tensor_tensor(vP01[:], fA[:, 0:4, :], fA[:, 4:8, :], ADD)
                fC = fp.tile([128, 2, 128], bf16, tag="fC", name="fC")
                nc.vector.tensor_tensor(fC[:], xl[:, 24:26, :], xl[:, 26:28, :], ADD)
                fB = fp.tile([128, 4, 128], bf16, tag="fB", name="fB")
                nc.vector.tensor_tensor(fB[:], xl[:, 16:20, :], xl[:, 20:24, :], ADD)
                return vQ01, fA, vP01, fB, fC

            def emit_tree(bt, tag, bpfx, vp01, v0src, v1src, warmup=False):
                """Depth-2 Karatsuba Toeplitz-1024 tree -> tX [128,1024] bf16."""
                TL = pp.tile([128, 1536], f32, tag=tag, name=tag + "L")
                if warmup:
                    for _ in range(4):
                        nc.tensor.matmul(
                            TL[:, 1024:1536], warm[:, 0:128], warm[:],
                            start=True, stop=True,
                        )
                t512(TL[:, 0:512], bpfx + "T0", lambda t: vp01[:, t, :])
                sXp = midp.tile([128, 512], bf16, tag="s" + bpfx, name="s" + bpfx)
                nc.scalar.copy(sXp[:], TL[:, 0:512])
                t512(TL[:, 512:1024], bpfx + "V", v1src)
                t512(TL[:, 1024:1536], bpfx + "W", v0src)
                sXv = midp.tile([128, 1024], bf16, tag="sv" + bpfx, name="sv" + bpfx)
                nc.scalar.copy(sXv[:, 0:512], TL[:, 512:1024])
                nc.scalar.copy(sXv[:, 512:1024], TL[:, 1024:1536])
                tX = midp.tile([128, 1024], bf16, tag="t" + bpfx, name="t" + bpfx)
                nc.vector.tensor_tensor(tX[:, 0:512], sXp[:], sXv[:, 0:512], ADD)
                nc.vector.tensor_tensor(tX[:, 512:1024], sXp[:], sXv[:, 512:1024], ADD)
                return tX

            def emit_C(xl, fC):
                TCg = pp.tile([128, 768], f32, tag="TC", name="TC")
                t256(TCg[:, 0:256], "CQ", lambda t: xl[:, 26 + t, :])
                t256(TCg[:, 256:512], "CP", lambda t: fC[:, t, :])
                sCP = midp.tile([128, 256], bf16, tag="sCP", name="sCP")
                nc.scalar.copy(sCP[:], TCg[:, 256:512])
                ynC = ynp.tile([128, 512], bf16, tag="ynC", name="ynC")
                nc.vector.tensor_tensor(ynC[:, 0:256], sCP[:], TCg[:, 0:256], SUB)
                t256(TCg[:, 512:768], "CR", lambda t: xl[:, 24 + t, :])
                nc.vector.tensor_tensor(ynC[:, 256:512], sCP[:], TCg[:, 512:768], ADD)
                return ynC

            def emit_B(xl, fB):
                T2g = pp.tile([128, 1536], f32, tag="T2", name="T2B")
                t512(T2g[:, 0:512], "BQ", lambda t: xl[:, 20 + t, :])
                t512(T2g[:, 512:1024], "BP", lambda t: fB[:, t, :])
                sBP = midp.tile([128, 512], bf16, tag="sBP", name="sBP")
                nc.scalar.copy(sBP[:], T2g[:, 512:1024])
                ynB = ynp.tile([128, 1024], bf16, tag="ynB", name="ynB")
                nc.vector.tensor_tensor(ynB[:, 0:512], sBP[:], T2g[:, 0:512], SUB)
                t512(T2g[:, 1024:1536], "BR", lambda t: xl[:, 16 + t, :])
                nc.vector.tensor_tensor(ynB[:, 512:1024], sBP[:], T2g[:, 1024:1536], ADD)
                return ynB

            def emit_D(psum, xl):
                # D shares the R generation's T1 tile (banks are disjoint);
                # ACT evacuates so cpp is SBUF-only and nothing D-related
                # gates the next block's T1 reuse
                t512(psum, "D", lambda t: xl[:, 28 + t, :])
                sD = midp.tile([128, 512], bf16, tag="sD", name="sD")
                nc.scalar.copy(sD[:], psum)
                return sD

            def emit_unfold_mid(TD, ynB, ynC):
                cpp = midp.tile([128, 1024], bf16, tag="cpp", name="cpp")
                nc.vector.tensor_tensor(cpp[:, 0:512], TD[:], ynC[:], ADD)
                nc.vector.tensor_tensor(cpp[:, 512:1024], TD[:], ynC[:], SUB)
                u1 = midp.tile([128, 1024], bf16, tag="u1", name="u1")
                nc.vector.tensor_tensor(u1[:], cpp[:], ynB[:], ADD)
                u2 = midp.tile([128, 1024], bf16, tag="u2", name="u2")
                nc.vector.tensor_tensor(u2[:], cpp[:], ynB[:], SUB)
                return u1, u2

            def emit_finals(b0, u1, u2, ynA, dve_share=False):
                # Pool by default; DVE takes half in the drain-critical blocks,
                # which also get per-1024 DMAs for finer tail interleave
                if dve_share:
                    for seg, usrc, alu, eng in (
                        (0, u1, ADD, nc.gpsimd), (1, u2, ADD, nc.gpsimd),
                        (2, u1, SUB, nc.gpsimd), (3, u2, SUB, nc.gpsimd),
                    ):
                        o = op.tile([128, 1024], bf16, tag="o", name="o", bufs=4)
                        eng.tensor_tensor(
                            o[:], usrc[:], ynA[:, 1024 * (seg % 2) : 1024 * (seg % 2) + 1024], alu
                        )
                        nc.sync.dma_start(
                            out_d[b0 : b0 + 128, 1024 * seg : 1024 * seg + 1024], o[:]
                        )
                    return
                oL = op.tile([128, 2048], bf16, tag="oL", name="oL")
                nc.gpsimd.tensor_tensor(oL[:, 0:1024], u1[:], ynA[:, 0:1024], ADD)
                nc.gpsimd.tensor_tensor(oL[:, 1024:2048], u2[:], ynA[:, 1024:2048], ADD)
                nc.sync.dma_start(out_d[b0 : b0 + 128, 0:2048], oL[:])
                oR = op.tile([128, 2048], bf16, tag="oR", name="oR")
                nc.gpsimd.tensor_tensor(oR[:, 0:1024], u1[:], ynA[:, 0:1024], SUB)
                nc.gpsimd.tensor_tensor(oR[:, 1024:2048], u2[:], ynA[:, 1024:2048], SUB)
                nc.sync.dma_start(out_d[b0 : b0 + 128, 2048:4096], oR[:])

            pending = None
            xls = {0: xl0, 1: xl1}
            folds_cur = emit_folds(xl0)
            for bt in range(NB):
                b0 = 128 * bt
                xl = xls[bt]
                vQ01, fA, vP01, fB, fC = folds_cur
                last = bt == NB - 1
                if not last:
                    if pending is not None:
                        pending()
                    tQ = emit_tree(
                        bt, "T1", "Q", vQ01,
                        lambda t: xl[:, 8 + t, :], lambda t: xl[:, 12 + t, :],
                        warmup=(bt == 0),
                    )
                    tP = emit_tree(
                        bt, "T2", "P", vP01,
                        lambda t: fA[:, t, :], lambda t: fA[:, 4 + t, :],
                    )
                    ynA = ynp.tile([128, 2048], bf16, tag="ynA", name="ynA")
                    nc.vector.tensor_tensor(ynA[:, 0:1024], tP[:], tQ[:], SUB)
                    TRp = pp.tile([128, 1536], f32, tag="T1", name="T1RD")
                    t1024(TRp[:, 0:1024], "AR", lambda t: xl[:, t, :])
                    sR = midp.tile([128, 1024], bf16, tag="sRR", name="sRR")
                    for h in (0, 1):
                        nc.scalar.copy(
                            sR[:, 512 * h : 512 * h + 512],
                            TRp[:, 512 * h : 512 * h + 512],
                        )
                        nc.vector.tensor_tensor(
                            ynA[:, 1024 + 512 * h : 1536 + 512 * h],
                            tP[:, 512 * h : 512 * h + 512],
                            sR[:, 512 * h : 512 * h + 512],
                            ADD,
                        )
                    sD = emit_D(TRp[:, 1024:1536], xl)
                    # next block's input + folds right behind the hi-halves in
                    # the DVE queue so block bt+1's first group is never gated
                    if bt + 2 < NB:
                        xls[bt + 2] = xlp.tile([128, 32, 128], bf16, tag="xl", name="xl")
                        nc.sync.dma_start(xls[bt + 2][:], xl_t[:, bt + 2, :, :])
                    folds_cur = emit_folds(xls[bt + 1])
                    ynB = emit_B(xl, fB)
                    ynC = emit_C(xl, fC)
                    cpp = midp.tile([128, 1024], bf16, tag="cpp", name="cpp")
                    nc.vector.tensor_tensor(cpp[:, 0:512], sD[:], ynC[:], ADD)
                    nc.vector.tensor_tensor(cpp[:, 512:1024], sD[:], ynC[:], SUB)
                    u1 = midp.tile([128, 1024], bf16, tag="u1", name="u1")
                    nc.vector.tensor_tensor(u1[:], cpp[:], ynB[:], ADD)
                    u2 = midp.tile([128, 1024], bf16, tag="u2", name="u2")
                    nc.vector.tensor_tensor(u2[:], cpp[:], ynB[:], SUB)
                    dve_share = bt >= NB - 3
                    pending = (
                        lambda b0=b0, u1=u1, u2=u2, ynA=ynA, d=dve_share: emit_finals(
                            b0, u1, u2, ynA, dve_share=d
                        )
                    )
                else:
                    # last block: C/B/D first, then Q/P trees, R last; finals
                    # for the y_lo half stream during R, y_hi half-granular
                    if pending is not None:
                        pending()
                        pending = None
                    ynC = emit_C(xl, fC)
                    ynB = emit_B(xl, fB)
                    TDg = pp.tile([128, 1536], f32, tag="T1", name="T1D")
                    sD = emit_D(TDg[:, 0:512], xl)
                    cpp = midp.tile([128, 1024], bf16, tag="cpp", name="cpp")
                    nc.vector.tensor_tensor(cpp[:, 0:512], sD[:], ynC[:], ADD)
                    nc.vector.tensor_tensor(cpp[:, 512:1024], sD[:], ynC[:], SUB)
                    u1 = midp.tile([128, 1024], bf16, tag="u1", name="u1")
                    nc.vector.tensor_tensor(u1[:], cpp[:], ynB[:], ADD)
                    u2 = midp.tile([128, 1024], bf16, tag="u2", name="u2")
                    nc.vector.tensor_tensor(u2[:], cpp[:], ynB[:], SUB)
                    tQ = emit_tree(
                        bt, "T1", "Q", vQ01,
                        lambda t: xl[:, 8 + t, :], lambda t: xl[:, 12 + t, :],
                    )
                    tP = emit_tree(
                        bt, "T2", "P", vP01,
                        lambda t: fA[:, t, :], lambda t: fA[:, 4 + t, :],
                    )
                    ynA = ynp.tile([128, 2048], bf16, tag="ynA", name="ynA")
                    nc.vector.tensor_tensor(ynA[:, 0:1024], tP[:], tQ[:], SUB)
                    for seg, alu, eng in ((0, ADD, "v"), (2, SUB, "p")):
                        o = op.tile([128, 1024], bf16, tag="o", name="o", bufs=4)
                        if eng == "v":
                            nc.vector.tensor_tensor(o[:], u1[:], ynA[:, 0:1024], alu)
                        else:
                            nc.gpsimd.tensor_tensor(o[:], u1[:], ynA[:, 0:1024], alu)
                        nc.sync.dma_start(
                            out_d[b0 : b0 + 128, 1024 * seg : 1024 * seg + 1024], o[:]
                        )
                    TRp = pp.tile([128, 1536], f32, tag="T2", name="T2R")
                    t1024(TRp[:, 0:1024], "AR", lambda t: xl[:, t, :])
                    for h in (0, 1):
                        nc.vector.tensor_tensor(
                            ynA[:, 1024 + 512 * h : 1536 + 512 * h],
                            tP[:, 512 * h : 512 * h + 512],
                            TRp[:, 512 * h : 512 * h + 512],
                            ADD,
                        )
                        for seg, alu, eng in ((1, ADD, "v"), (3, SUB, "p")):
                            o = op.tile([128, 512], bf16, tag="oh", name="oh", bufs=4)
                            args = (
                                o[:],
                                u2[:, 512 * h : 512 * h + 512],
                                ynA[:, 1024 + 512 * h : 1536 + 512 * h],
                                alu,
                            )
                            if eng == "v":
                                nc.vector.tensor_tensor(*args)
                            else:
                                nc.gpsimd.tensor_tensor(*args)
                            nc.sync.dma_start(
                                out_d[
                                    b0 : b0 + 128,
                                    1024 * seg + 512 * h : 1024 * seg + 512 * h + 512,
                                ],
                                o[:],
                            )
            if pending is not None:
                pending()

    nc.compile()
    return nc


def _get_nc():
    if "nc" not in _STATE:
        _STATE["nc"] = _build()
    return _STATE["nc"]


def _prep_inputs(x, w):
    x = np.ascontiguousarray(x, dtype=np.float32)
    w = np.asarray(w, dtype=np.float32)
    bands = _make_bands(w)
    in_maps = []
    for i in range(N_CORES):
        xl = _fold_x(x[i * B_SHARD : (i + 1) * B_SHARD])
        in_maps.append({"xl": xl.reshape(128, 32 * B_SHARD), "bands": bands})
    return in_maps


def kernel(x, w, _trace=False):
    from concourse.bass_utils import run_bass_kernel_spmd

    nc = _get_nc()
    in_maps = _prep_inputs(x, w)
    res = run_bass_kernel_spmd(nc, in_maps, list(range(N_CORES)), trace=_trace)
    out = np.concatenate(
        [np.asarray(res.results[i]["out"]).astype(np.float32) for i in range(N_CORES)],
        axis=0,
    )
    if _trace:
        _STATE["last_result"] = res
    return out


# revision 11
# speedup vs baseline: 1.0372x; 1.0006x over previous
"""Circulant matmul for TRN2 v3: CRT + Gauss 3-mult negacyclic splits with
depth-2 Karatsuba on two of the three nega2048 trees, bf16 matmuls,
host-precomputed x-folds and pre-scaled band operators.

out[b, r] = sum_c x[b,c] w[(c-r) mod N] = cyclic_conv(x_row, v), v = roll(rev(w),1).

CRT: z^4096-1 -> leaves nega2048 (A), nega1024 (B), nega512 (C), cyc512 (D).
nega-n via Gauss 3-mult on its [[T,-U],[U,T]] structure:
  P = T(a0+a1), Q = (T+U)a1, R = (U-T)a0, y = [P-Q, P+R].
For A, the P and Q Toeplitz-1024 products are themselves Karatsuba'd into
3 Toeplitz-512 products each (p = T0(v0+v1), top = p + (V-T0)v1,
bot = p + (W-T0)v0); R stays dense -- engine balance: deeper splits save PE
but overload DVE/Pool with combines. PE: 30208 cyc/128-row block.

Host sends bf16: x CRT-leaf chunks [128, 8, 32, 128] (8 MB/core, per-block
contiguous) + pre-scaled band blob (3.1 MB). All CRT/Gauss scales folded
into bands. ACT evacuates shared Kara leaves + cyc512, DVE does PSUM
combines, Pool does folds fB/fC and most final output combines. PSUM map:
T1 (Q-leaves -> R||cyc512), T2 (P-leaves -> B-leaves), TC (nega512) --
ordered so no combine chain ever gates the next block's first matmul
group; per-block PE order Q, P, R, D, B, C. The last two blocks split
finals across DVE/Pool at fine granularity so the tail output DMAs
overlap the final A-tree matmul stream.
"""

import sys

sys.path.insert(0, "/opt/trn_rl_repo")

import numpy as np
import ml_dtypes

BF16 = ml_dtypes.bfloat16
N = 4096
B = 8192
N_CORES = 8
B_SHARD = B // N_CORES  # 1024
NB = B_SHARD // 128     # 8 row-tiles per core

# band blob layout: name -> (m, offset); width = 2m-128; consumption order
_BANDS = {
    "QT0": (512, 0), "QV": (512, 896), "QW": (512, 1792),
    "PT0": (512, 2688), "PV": (512, 3584), "PW": (512, 4480),
    "AR": (1024, 5376),
    "D": (512, 7296), "BQ": (512, 8192), "BP": (512, 9088),
    "CQ": (256, 9984), "CP": (256, 10368), "BR": (512, 10752),
    "CR": (256, 11648),
}
WTOT = 12032

_STATE = {}


# ---------------------------------------------------------------- host math
def _sub_symbol(t, m, shift, h):
    c = m - 1
    return t[c + shift - (h - 1) : c + shift + h].copy()


def _band_from_symbol(t, m):
    u = np.arange(2 * m - 128)[None, :]
    p = np.arange(128)[:, None]
    return t[u - p + 127]


def _make_bands(w):
    v = np.roll(w[::-1].astype(np.float64), 1)
    vm = v[:2048] - v[2048:]
    vp = v[:2048] + v[2048:]
    vpm = vp[:1024] - vp[1024:]
    vpp = vp[:1024] + vp[1024:]
    vppm = vpp[:512] - vpp[512:]
    vppp = vpp[:512] + vpp[512:]

    def skew_sym(b):
        n = len(b)
        t = np.empty(2 * n - 1)
        t[n - 1 :] = b
        t[: n - 1] = -b[1:]
        return t

    def cyc_sym(b):
        n = len(b)
        t = np.empty(2 * n - 1)
        t[n - 1 :] = b
        t[: n - 1] = b[1:]
        return t

    blob = np.zeros((128, WTOT), np.float64)

    def put(name, sym):
        m, off = _BANDS[name]
        blob[:, off : off + 2 * m - 128] = _band_from_symbol(sym, m)

    # A = nega2048: T1024-level Gauss operators (scale 0.5 folded in)
    full = skew_sym(vm)
    tT = _sub_symbol(full, 2048, 0, 1024)
    tU = _sub_symbol(full, 2048, 1024, 1024)
    sP = 0.5 * tT
    sQ = 0.5 * (tT + tU)
    sR = 0.5 * (tU - tT)
    for nm, sym in (("P", sP), ("Q", sQ)):
        t0 = _sub_symbol(sym, 1024, 0, 512)
        tV = _sub_symbol(sym, 1024, -512, 512)
        tW = _sub_symbol(sym, 1024, 512, 512)
        put(nm + "T0", t0)
        put(nm + "V", tV - t0)
        put(nm + "W", tW - t0)
    put("AR", sR)

    for nm, bsrc, scale in [("B", vpm, 0.25), ("C", vppm, 0.125)]:
        n = len(bsrc)
        m = n // 2
        fl = skew_sym(bsrc)
        t1 = _sub_symbol(fl, n, 0, m)
        t2 = _sub_symbol(fl, n, m, m)
        put(nm + "P", scale * t1)
        put(nm + "Q", scale * (t1 + t2))
        put(nm + "R", scale * (t2 - t1))
    put("D", 0.125 * cyc_sym(vppp))
    return blob.astype(BF16)


def _fold_x(x_shard):
    """[1024, 4096] f32 -> [128, 8, 32, 128] bf16 chunk-partitioned CRT
    leaves, per-block contiguous."""
    xs = x_shard.astype(np.float32)
    xm = xs[:, :2048] - xs[:, 2048:]
    xp = xs[:, :2048] + xs[:, 2048:]
    xpm = xp[:, :1024] - xp[:, 1024:]
    xpp = xp[:, :1024] + xp[:, 1024:]
    xppm = xpp[:, :512] - xpp[:, 512:]
    xppp = xpp[:, :512] + xpp[:, 512:]
    L = np.concatenate([xm, xpm, xppm, xppp], axis=1)  # [1024, 4096]
    a = L.reshape(1024, 32, 128).transpose(2, 1, 0)  # [128, 32, 1024]
    a = a.reshape(128, 32, 8, 128).transpose(0, 2, 1, 3)
    return np.ascontiguousarray(a.astype(BF16))


# ---------------------------------------------------------------- device
def _build():
    import concourse.bacc as bacc
    import concourse.mybir as mybir
    import concourse.tile as tile

    f32 = mybir.dt.float32
    bf16 = mybir.dt.bfloat16
    ADD = mybir.AluOpType.add
    SUB = mybir.AluOpType.subtract

    nc = bacc.Bacc("TRN2", target_bir_lowering=False, debug=False)
    xl_d = nc.declare_dram_parameter("xl", [128, 32 * B_SHARD], bf16, isOutput=False)
    bands_d = nc.declare_dram_parameter("bands", [128, WTOT], bf16, isOutput=False)
    out_d = nc.declare_dram_parameter("out", [B_SHARD, N], bf16, isOutput=True)

    xl_t = xl_d[:].rearrange("p (k a b) -> p k a b", k=NB, a=32)  # [128, 8, 32, 128]

    with tile.TileContext(nc) as tc:
        with (
            tc.tile_pool(name="const", bufs=1) as constp,
            tc.tile_pool(name="xl", bufs=2) as xlp,
            tc.tile_pool(name="fold", bufs=3) as fp,
            tc.tile_pool(name="yn", bufs=4) as ynp,
            tc.tile_pool(name="mid", bufs=4) as midp,
            tc.tile_pool(name="outp", bufs=3) as op,
            tc.tile_pool(name="ps", bufs=1, space="PSUM") as pp,
        ):
            band = constp.tile([128, WTOT], bf16)
            warm = constp.tile([128, 512], bf16, name="warm")
            nc.gpsimd.memset(warm[:], 0.0)

            def bsl(name, lo, width):
                off = _BANDS[name][1]
                return band[:, off + lo : off + lo + width]

            # first-block DMAs in consumption order; xl(1) last
            xl0 = xlp.tile([128, 32, 128], bf16, tag="xl", name="xl0")
            nc.sync.dma_start(xl0[:, 8:16, :], xl_t[:, 0, 8:16, :])
            nc.sync.dma_start(band[:, 0:896], bands_d[:, 0:896])        # QT0 band
            nc.sync.dma_start(xl0[:, 0:8, :], xl_t[:, 0, 0:8, :])
            nc.sync.dma_start(band[:, 896:2688], bands_d[:, 896:2688])  # QV/QW
            nc.sync.dma_start(band[:, 2688:5376], bands_d[:, 2688:5376])  # P bands
            nc.sync.dma_start(band[:, 5376:7296], bands_d[:, 5376:7296])  # AR band
            nc.sync.dma_start(xl0[:, 16:32, :], xl_t[:, 0, 16:32, :])
            nc.sync.dma_start(band[:, 7296:WTOT], bands_d[:, 7296:WTOT])  # D/B/C
            xl1 = xlp.tile([128, 32, 128], bf16, tag="xl", name="xl1")
            nc.sync.dma_start(xl1[:], xl_t[:, 1, :, :])

            def t1024(psum, bname, src):
                for j in range(2):
                    for t in range(8):
                        nc.tensor.matmul(
                            psum[:, 512 * j : 512 * j + 512],
                            src(t),
                            bsl(bname, (7 - t) * 128 + 512 * j, 512),
                            start=(t == 0),
                            stop=(t == 7),
                        )

            def t512(psum, bname, src):
                for t in range(4):
                    nc.tensor.matmul(
                        psum[:],
                        src(t),
                        bsl(bname, (3 - t) * 128, 512),
                        start=(t == 0),
                        stop=(t == 3),
                    )

            def t256(psum, bname, src):
                for t in range(2):
                    nc.tensor.matmul(
                        psum[:],
                        src(t),
                        bsl(bname, (1 - t) * 128, 256),
                        start=(t == 0),
                        stop=(t == 1),
                    )

            def emit_folds(xl):
                vQ01 = fp.tile([128, 4, 128], bf16, tag="vQ01", name="vQ01")
                nc.vector.tensor_tensor(vQ01[:], xl[:, 8:12, :], xl[:, 12:16, :], ADD)
                fA = fp.tile([128, 8, 128], bf16, tag="fA", name="fA")
                nc.vector.tensor_tensor(fA[:], xl[:, 0:8, :], xl[:, 8:16, :], ADD)
                vP01 = fp.tile([128, 4, 128], bf16, tag="vP01", name="vP01")
                nc.vector.tensor_tensor(vP01[:], fA[:, 0:4, :], fA[:, 4:8, :], ADD)
                fC = fp.tile([128, 2, 128], bf16, tag="fC", name="fC")
                nc.vector.tensor_tensor(fC[:], xl[:, 24:26, :], xl[:, 26:28, :], ADD)
                fB = fp.tile([128, 4, 128], bf16, tag="fB", name="fB")
                nc.vector.tensor_tensor(fB[:], xl[:, 16:20, :], xl[:, 20:24, :], ADD)
                return vQ01, fA, vP01, fB, fC

            def emit_tree(bt, tag, bpfx, vp01, v0src, v1src, warmup=False, evac=True):
                """Depth-2 Karatsuba Toeplitz-1024 tree -> tX [128,1024] bf16."""
                TL = pp.tile([128, 1536], f32, tag=tag, name=tag + "L")
                if warmup:
                    for _ in range(4):
                        nc.tensor.matmul(
                            TL[:, 1024:1536], warm[:, 0:128], warm[:],
                            start=True, stop=True,
                        )
                t512(TL[:, 0:512], bpfx + "T0", lambda t: vp01[:, t, :])
                sXp = midp.tile([128, 512], bf16, tag="s" + bpfx, name="s" + bpfx)
                nc.scalar.copy(sXp[:], TL[:, 0:512])
                t512(TL[:, 512:1024], bpfx + "V", v1src)
                t512(TL[:, 1024:1536], bpfx + "W", v0src)
                tX = midp.tile([128, 1024], bf16, tag="t" + bpfx, name="t" + bpfx)
                if evac:
                    sXv = midp.tile([128, 1024], bf16, tag="sv" + bpfx, name="sv" + bpfx)
                    nc.scalar.copy(sXv[:, 0:512], TL[:, 512:1024])
                    nc.scalar.copy(sXv[:, 512:1024], TL[:, 1024:1536])
                    nc.vector.tensor_tensor(tX[:, 0:512], sXp[:], sXv[:, 0:512], ADD)
                    nc.vector.tensor_tensor(tX[:, 512:1024], sXp[:], sXv[:, 512:1024], ADD)
                else:
                    nc.vector.tensor_tensor(tX[:, 0:512], sXp[:], TL[:, 512:1024], ADD)
                    nc.vector.tensor_tensor(tX[:, 512:1024], sXp[:], TL[:, 1024:1536], ADD)
                return tX

            def emit_C(xl, fC):
                TCg = pp.tile([128, 768], f32, tag="TC", name="TC")
                t256(TCg[:, 0:256], "CQ", lambda t: xl[:, 26 + t, :])
                t256(TCg[:, 256:512], "CP", lambda t: fC[:, t, :])
                sCP = midp.tile([128, 256], bf16, tag="sCP", name="sCP")
                nc.scalar.copy(sCP[:], TCg[:, 256:512])
                ynC = ynp.tile([128, 512], bf16, tag="ynC", name="ynC")
                nc.vector.tensor_tensor(ynC[:, 0:256], sCP[:], TCg[:, 0:256], SUB)
                t256(TCg[:, 512:768], "CR", lambda t: xl[:, 24 + t, :])
                nc.vector.tensor_tensor(ynC[:, 256:512], sCP[:], TCg[:, 512:768], ADD)
                return ynC

            def emit_B(xl, fB):
                T2g = pp.tile([128, 1536], f32, tag="T2", name="T2B")
                t512(T2g[:, 0:512], "BQ", lambda t: xl[:, 20 + t, :])
                t512(T2g[:, 512:1024], "BP", lambda t: fB[:, t, :])
                sBP = midp.tile([128, 512], bf16, tag="sBP", name="sBP")
                nc.scalar.copy(sBP[:], T2g[:, 512:1024])
                ynB = ynp.tile([128, 1024], bf16, tag="ynB", name="ynB")
                nc.vector.tensor_tensor(ynB[:, 0:512], sBP[:], T2g[:, 0:512], SUB)
                t512(T2g[:, 1024:1536], "BR", lambda t: xl[:, 16 + t, :])
                nc.vector.tensor_tensor(ynB[:, 512:1024], sBP[:], T2g[:, 1024:1536], ADD)
                return ynB

            def emit_D(psum, xl):
                # D shares the R generation's T1 tile (banks are disjoint);
                # ACT evacuates so cpp is SBUF-only and nothing D-related
                # gates the next block's T1 reuse
                t512(psum, "D", lambda t: xl[:, 28 + t, :])
                sD = midp.tile([128, 512], bf16, tag="sD", name="sD")
                nc.scalar.copy(sD[:], psum)
                return sD

            def emit_unfold_mid(TD, ynB, ynC):
                cpp = midp.tile([128, 1024], bf16, tag="cpp", name="cpp")
                nc.vector.tensor_tensor(cpp[:, 0:512], TD[:], ynC[:], ADD)
                nc.vector.tensor_tensor(cpp[:, 512:1024], TD[:], ynC[:], SUB)
                u1 = midp.tile([128, 1024], bf16, tag="u1", name="u1")
                nc.vector.tensor_tensor(u1[:], cpp[:], ynB[:], ADD)
                u2 = midp.tile([128, 1024], bf16, tag="u2", name="u2")
                nc.vector.tensor_tensor(u2[:], cpp[:], ynB[:], SUB)
                return u1, u2

            def emit_finals(b0, u1, u2, ynA, dve_share=False):
                # Pool by default; DVE takes half in the drain-critical blocks,
                # which also get per-1024 DMAs for finer tail interleave
                if dve_share:
                    for seg, usrc, alu, eng in (
                        (0, u1, ADD, nc.gpsimd), (1, u2, ADD, nc.gpsimd),
                        (2, u1, SUB, nc.gpsimd), (3, u2, SUB, nc.gpsimd),
                    ):
                        o = op.tile([128, 1024], bf16, tag="o", name="o", bufs=4)
                        eng.tensor_tensor(
                            o[:], usrc[:], ynA[:, 1024 * (seg % 2) : 1024 * (seg % 2) + 1024], alu
                        )
                        nc.sync.dma_start(
                            out_d[b0 : b0 + 128, 1024 * seg : 1024 * seg + 1024], o[:]
                        )
                    return
                oL = op.tile([128, 2048], bf16, tag="oL", name="oL")
                nc.gpsimd.tensor_tensor(oL[:, 0:1024], u1[:], ynA[:, 0:1024], ADD)
                nc.gpsimd.tensor_tensor(oL[:, 1024:2048], u2[:], ynA[:, 1024:2048], ADD)
                nc.sync.dma_start(out_d[b0 : b0 + 128, 0:2048], oL[:])
                oR = op.tile([128, 2048], bf16, tag="oR", name="oR")
                nc.gpsimd.tensor_tensor(oR[:, 0:1024], u1[:], ynA[:, 0:1024], SUB)
                nc.gpsimd.tensor_tensor(oR[:, 1024:2048], u2[:], ynA[:, 1024:2048], SUB)
                nc.sync.dma_start(out_d[b0 : b0 + 128, 2048:4096], oR[:])

            pending = None
            xls = {0: xl0, 1: xl1}
            folds_cur = emit_folds(xl0)
            for bt in range(NB):
                b0 = 128 * bt
                xl = xls[bt]
                vQ01, fA, vP01, fB, fC = folds_cur
                last = bt == NB - 1
                if not last:
                    if pending is not None:
                        pending()
                    tQ = emit_tree(
                        bt, "T1", "Q", vQ01,
                        lambda t: xl[:, 8 + t, :], lambda t: xl[:, 12 + t, :],
                        warmup=(bt == 0),
                    )
                    tP = emit_tree(
                        bt, "T2", "P", vP01,
                        lambda t: fA[:, t, :], lambda t: fA[:, 4 + t, :],
                    )
                    ynA = ynp.tile([128, 2048], bf16, tag="ynA", name="ynA")
                    nc.vector.tensor_tensor(ynA[:, 0:1024], tP[:], tQ[:], SUB)
                    TRp = pp.tile([128, 1536], f32, tag="T1", name="T1RD")
                    t1024(TRp[:, 0:1024], "AR", lambda t: xl[:, t, :])
                    sR = midp.tile([128, 1024], bf16, tag="sRR", name="sRR")
                    for h in (0, 1):
                        nc.scalar.copy(
                            sR[:, 512 * h : 512 * h + 512],
                            TRp[:, 512 * h : 512 * h + 512],
                        )
                        nc.vector.tensor_tensor(
                            ynA[:, 1024 + 512 * h : 1536 + 512 * h],
                            tP[:, 512 * h : 512 * h + 512],
                            sR[:, 512 * h : 512 * h + 512],
                            ADD,
                        )
                    sD = emit_D(TRp[:, 1024:1536], xl)
                    # next block's input + folds right behind the hi-halves in
                    # the DVE queue so block bt+1's first group is never gated
                    if bt + 2 < NB:
                        xls[bt + 2] = xlp.tile([128, 32, 128], bf16, tag="xl", name="xl")
                        nc.sync.dma_start(xls[bt + 2][:], xl_t[:, bt + 2, :, :])
                    folds_cur = emit_folds(xls[bt + 1])
                    ynB = emit_B(xl, fB)
                    ynC = emit_C(xl, fC)
                    cpp = midp.tile([128, 1024], bf16, tag="cpp", name="cpp")
                    nc.vector.tensor_tensor(cpp[:, 0:512], sD[:], ynC[:], ADD)
                    nc.vector.tensor_tensor(cpp[:, 512:1024], sD[:], ynC[:], SUB)
                    u1 = midp.tile([128, 1024], bf16, tag="u1", name="u1")
                    nc.vector.tensor_tensor(u1[:], cpp[:], ynB[:], ADD)
                    u2 = midp.tile([128, 1024], bf16, tag="u2", name="u2")
                    nc.vector.tensor_tensor(u2[:], cpp[:], ynB[:], SUB)
                    dve_share = bt >= NB - 3
                    pending = (
                        lambda b0=b0, u1=u1, u2=u2, ynA=ynA, d=dve_share: emit_finals(
                            b0, u1, u2, ynA, dve_share=d
                        )
                    )
                else:
                    # last block: C/B/D first, then Q/P trees, R last; finals
                    # for the y_lo half stream during R, y_hi half-granular
                    if pending is not None:
                        pending()
                        pending = None
                    ynC = emit_C(xl, fC)
                    ynB = emit_B(xl, fB)
                    TDg = pp.tile([128, 1536], f32, tag="T1", name="T1D")
                    sD = emit_D(TDg[:, 0:512], xl)
                    cpp = midp.tile([128, 1024], bf16, tag="cpp", name="cpp")
                    nc.vector.tensor_tensor(cpp[:, 0:512], sD[:], ynC[:], ADD)
                    nc.vector.tensor_tensor(cpp[:, 512:1024], sD[:], ynC[:], SUB)
                    u1 = midp.tile([128, 1024], bf16, tag="u1", name="u1")
                    nc.vector.tensor_tensor(u1[:], cpp[:], ynB[:], ADD)
                    u2 = midp.tile([128, 1024], bf16, tag="u2", name="u2")
                    nc.vector.tensor_tensor(u2[:], cpp[:], ynB[:], SUB)
                    tQ = emit_tree(
                        bt, "T1", "Q", vQ01,
                        lambda t: xl[:, 8 + t, :], lambda t: xl[:, 12 + t, :],
                    )
                    tP = emit_tree(
                        bt, "T2", "P", vP01,
                        lambda t: fA[:, t, :], lambda t: fA[:, 4 + t, :],
                    )
                    ynA = ynp.tile([128, 2048], bf16, tag="ynA", name="ynA")
                    nc.vector.tensor_tensor(ynA[:, 0:1024], tP[:], tQ[:], SUB)
                    for seg, alu, eng in ((0, ADD, "v"), (2, SUB, "p")):
                        o = op.tile([128, 1024], bf16, tag="o", name="o", bufs=4)
                        if eng == "v":
                            nc.vector.tensor_tensor(o[:], u1[:], ynA[:, 0:1024], alu)
                        else:
                            nc.gpsimd.tensor_tensor(o[:], u1[:], ynA[:, 0:1024], alu)
                        nc.sync.dma_start(
                            out_d[b0 : b0 + 128, 1024 * seg : 1024 * seg + 1024], o[:]
                        )
                    TRp = pp.tile([128, 1536], f32, tag="T2", name="T2R")
                    t1024(TRp[:, 0:1024], "AR", lambda t: xl[:, t, :])
                    for h in (0, 1):
                        nc.vector.tensor_tensor(
                            ynA[:, 1024 + 512 * h : 1536 + 512 * h],
                            tP[:, 512 * h : 512 * h + 512],
                            TRp[:, 512 * h : 512 * h + 512],
                            ADD,
                        )
                        for seg, alu, eng in ((1, ADD, "v"), (3, SUB, "p")):
                            o = op.tile([128, 512], bf16, tag="oh", name="oh", bufs=4)
                            args = (
                                o[:],
                                u2[:, 512 * h : 512 * h + 512],
                                ynA[:, 1024 + 512 * h : 1536 + 512 * h],
                                alu,
                            )
                            if eng == "v":
                                nc.vector.tensor_tensor(*args)
                            else:
                                nc.gpsimd.tensor_tensor(*args)
                            nc.sync.dma_start(
                                out_d[
                                    b0 : b0 + 128,
                                    1024 * seg + 512 * h : 1024 * seg + 512 * h + 512,
                                ],
                                o[:],
                            )
            if pending is not None:
                pending()

    nc.compile()
    return nc


def _get_nc():
    if "nc" not in _STATE:
        _STATE["nc"] = _build()
    return _STATE["nc"]


def _prep_inputs(x, w):
    x = np.ascontiguousarray(x, dtype=np.float32)
    w = np.asarray(w, dtype=np.float32)
    bands = _make_bands(w)
    in_maps = []
    for i in range(N_CORES):
        xl = _fold_x(x[i * B_SHARD : (i + 1) * B_SHARD])
        in_maps.append({"xl": xl.reshape(128, 32 * B_SHARD), "bands": bands})
    return in_maps


def kernel(x, w, _trace=False):
    from concourse.bass_utils import run_bass_kernel_spmd

    nc = _get_nc()
    in_maps = _prep_inputs(x, w)
    res = run_bass_kernel_spmd(nc, in_maps, list(range(N_CORES)), trace=_trace)
    out = np.concatenate(
        [np.asarray(res.results[i]["out"]).astype(np.float32) for i in range(N_CORES)],
        axis=0,
    )
    if _trace:
        _STATE["last_result"] = res
    return out


# revision 12
# speedup vs baseline: 1.0463x; 1.0088x over previous
"""Circulant matmul for TRN2 v3: CRT + Gauss 3-mult negacyclic splits with
depth-2 Karatsuba on two of the three nega2048 trees, bf16 matmuls,
host-precomputed x-folds and pre-scaled band operators.

out[b, r] = sum_c x[b,c] w[(c-r) mod N] = cyclic_conv(x_row, v), v = roll(rev(w),1).

CRT: z^4096-1 -> leaves nega2048 (A), nega1024 (B), nega512 (C), cyc512 (D).
nega-n via Gauss 3-mult on its [[T,-U],[U,T]] structure:
  P = T(a0+a1), Q = (T+U)a1, R = (U-T)a0, y = [P-Q, P+R].
For A, the P and Q Toeplitz-1024 products are themselves Karatsuba'd into
3 Toeplitz-512 products each (p = T0(v0+v1), top = p + (V-T0)v1,
bot = p + (W-T0)v0); R stays dense -- engine balance: deeper splits save PE
but overload DVE/Pool with combines. PE: 30208 cyc/128-row block.

Host sends bf16: x CRT-leaf chunks [128, 8, 32, 128] (8 MB/core, per-block
contiguous) + pre-scaled band blob (3.1 MB). All CRT/Gauss scales folded
into bands. ACT evacuates shared Kara leaves + cyc512, DVE does PSUM
combines, Pool does folds fB/fC and most final output combines. PSUM map:
T1 (Q-leaves -> R||cyc512), T2 (P-leaves -> B-leaves), TC (nega512) --
ordered so no combine chain ever gates the next block's first matmul
group; per-block PE order Q, P, R, D, B, C. The last two blocks split
finals across DVE/Pool at fine granularity so the tail output DMAs
overlap the final A-tree matmul stream.
"""

import sys

sys.path.insert(0, "/opt/trn_rl_repo")

import numpy as np
import ml_dtypes

BF16 = ml_dtypes.bfloat16
N = 4096
B = 8192
N_CORES = 8
B_SHARD = B // N_CORES  # 1024
NB = B_SHARD // 128     # 8 row-tiles per core

# band blob layout: name -> (m, offset); width = 2m-128; consumption order
_BANDS = {
    "QT0": (512, 0), "QV": (512, 896), "QW": (512, 1792),
    "PT0": (512, 2688), "PV": (512, 3584), "PW": (512, 4480),
    "AR": (1024, 5376),
    "D": (512, 7296), "BQ": (512, 8192), "BP": (512, 9088),
    "CQ": (256, 9984), "CP": (256, 10368), "BR": (512, 10752),
    "CR": (256, 11648),
}
WTOT = 12032

_STATE = {}


# ---------------------------------------------------------------- host math
def _sub_symbol(t, m, shift, h):
    c = m - 1
    return t[c + shift - (h - 1) : c + shift + h].copy()


def _band_from_symbol(t, m):
    u = np.arange(2 * m - 128)[None, :]
    p = np.arange(128)[:, None]
    return t[u - p + 127]


def _make_bands(w):
    v = np.roll(w[::-1].astype(np.float64), 1)
    vm = v[:2048] - v[2048:]
    vp = v[:2048] + v[2048:]
    vpm = vp[:1024] - vp[1024:]
    vpp = vp[:1024] + vp[1024:]
    vppm = vpp[:512] - vpp[512:]
    vppp = vpp[:512] + vpp[512:]

    def skew_sym(b):
        n = len(b)
        t = np.empty(2 * n - 1)
        t[n - 1 :] = b
        t[: n - 1] = -b[1:]
        return t

    def cyc_sym(b):
        n = len(b)
        t = np.empty(2 * n - 1)
        t[n - 1 :] = b
        t[: n - 1] = b[1:]
        return t

    blob = np.zeros((128, WTOT), np.float64)

    def put(name, sym):
        m, off = _BANDS[name]
        blob[:, off : off + 2 * m - 128] = _band_from_symbol(sym, m)

    # A = nega2048: T1024-level Gauss operators (scale 0.5 folded in)
    full = skew_sym(vm)
    tT = _sub_symbol(full, 2048, 0, 1024)
    tU = _sub_symbol(full, 2048, 1024, 1024)
    sP = 0.5 * tT
    sQ = 0.5 * (tT + tU)
    sR = 0.5 * (tU - tT)
    for nm, sym in (("P", sP), ("Q", sQ)):
        t0 = _sub_symbol(sym, 1024, 0, 512)
        tV = _sub_symbol(sym, 1024, -512, 512)
        tW = _sub_symbol(sym, 1024, 512, 512)
        put(nm + "T0", t0)
        put(nm + "V", tV - t0)
        put(nm + "W", tW - t0)
    put("AR", sR)

    for nm, bsrc, scale in [("B", vpm, 0.25), ("C", vppm, 0.125)]:
        n = len(bsrc)
        m = n // 2
        fl = skew_sym(bsrc)
        t1 = _sub_symbol(fl, n, 0, m)
        t2 = _sub_symbol(fl, n, m, m)
        put(nm + "P", scale * t1)
        put(nm + "Q", scale * (t1 + t2))
        put(nm + "R", scale * (t2 - t1))
    put("D", 0.125 * cyc_sym(vppp))
    return blob.astype(BF16)


def _fold_x(x_shard):
    """[1024, 4096] f32 -> [128, 8, 32, 128] bf16 chunk-partitioned CRT
    leaves, per-block contiguous."""
    xs = x_shard.astype(np.float32)
    xm = xs[:, :2048] - xs[:, 2048:]
    xp = xs[:, :2048] + xs[:, 2048:]
    xpm = xp[:, :1024] - xp[:, 1024:]
    xpp = xp[:, :1024] + xp[:, 1024:]
    xppm = xpp[:, :512] - xpp[:, 512:]
    xppp = xpp[:, :512] + xpp[:, 512:]
    L = np.concatenate([xm, xpm, xppm, xppp], axis=1)  # [1024, 4096]
    a = L.reshape(1024, 32, 128).transpose(2, 1, 0)  # [128, 32, 1024]
    a = a.reshape(128, 32, 8, 128).transpose(0, 2, 1, 3)
    return np.ascontiguousarray(a.astype(BF16))


# ---------------------------------------------------------------- device
def _build():
    import concourse.bacc as bacc
    import concourse.mybir as mybir
    import concourse.tile as tile

    f32 = mybir.dt.float32
    bf16 = mybir.dt.bfloat16
    ADD = mybir.AluOpType.add
    SUB = mybir.AluOpType.subtract

    nc = bacc.Bacc("TRN2", target_bir_lowering=False, debug=False)
    xl_d = nc.declare_dram_parameter("xl", [128, 32 * B_SHARD], bf16, isOutput=False)
    bands_d = nc.declare_dram_parameter("bands", [128, WTOT], bf16, isOutput=False)
    out_d = nc.declare_dram_parameter("out", [B_SHARD, N], bf16, isOutput=True)

    xl_t = xl_d[:].rearrange("p (k a b) -> p k a b", k=NB, a=32)  # [128, 8, 32, 128]

    with tile.TileContext(nc) as tc:
        with (
            tc.tile_pool(name="const", bufs=1) as constp,
            tc.tile_pool(name="xl", bufs=2) as xlp,
            tc.tile_pool(name="fold", bufs=3) as fp,
            tc.tile_pool(name="yn", bufs=4) as ynp,
            tc.tile_pool(name="mid", bufs=4) as midp,
            tc.tile_pool(name="outp", bufs=3) as op,
            tc.tile_pool(name="ps", bufs=1, space="PSUM") as pp,
        ):
            band = constp.tile([128, WTOT], bf16)
            warm = constp.tile([128, 512], bf16, name="warm")
            nc.gpsimd.memset(warm[:], 0.0)

            def bsl(name, lo, width):
                off = _BANDS[name][1]
                return band[:, off + lo : off + lo + width]

            # first-block DMAs in consumption order; xl(1) last
            xl0 = xlp.tile([128, 32, 128], bf16, tag="xl", name="xl0")
            nc.sync.dma_start(xl0[:, 8:16, :], xl_t[:, 0, 8:16, :])
            nc.sync.dma_start(band[:, 0:896], bands_d[:, 0:896])        # QT0 band
            nc.sync.dma_start(xl0[:, 0:8, :], xl_t[:, 0, 0:8, :])
            nc.sync.dma_start(band[:, 896:2688], bands_d[:, 896:2688])  # QV/QW
            nc.sync.dma_start(band[:, 2688:5376], bands_d[:, 2688:5376])  # P bands
            nc.sync.dma_start(band[:, 5376:7296], bands_d[:, 5376:7296])  # AR band
            nc.sync.dma_start(xl0[:, 16:32, :], xl_t[:, 0, 16:32, :])
            nc.sync.dma_start(band[:, 7296:WTOT], bands_d[:, 7296:WTOT])  # D/B/C
            xl1 = xlp.tile([128, 32, 128], bf16, tag="xl", name="xl1")
            nc.sync.dma_start(xl1[:], xl_t[:, 1, :, :])

            def t1024(psum, bname, src):
                for j in range(2):
                    for t in range(8):
                        nc.tensor.matmul(
                            psum[:, 512 * j : 512 * j + 512],
                            src(t),
                            bsl(bname, (7 - t) * 128 + 512 * j, 512),
                            start=(t == 0),
                            stop=(t == 7),
                        )

            def t512(psum, bname, src):
                for t in range(4):
                    nc.tensor.matmul(
                        psum[:],
                        src(t),
                        bsl(bname, (3 - t) * 128, 512),
                        start=(t == 0),
                        stop=(t == 3),
                    )

            def t256(psum, bname, src):
                for t in range(2):
                    nc.tensor.matmul(
                        psum[:],
                        src(t),
                        bsl(bname, (1 - t) * 128, 256),
                        start=(t == 0),
                        stop=(t == 1),
                    )

            def emit_folds(xl):
                vQ01 = fp.tile([128, 4, 128], bf16, tag="vQ01", name="vQ01")
                nc.vector.tensor_tensor(vQ01[:], xl[:, 8:12, :], xl[:, 12:16, :], ADD)
                fA = fp.tile([128, 8, 128], bf16, tag="fA", name="fA")
                nc.vector.tensor_tensor(fA[:], xl[:, 0:8, :], xl[:, 8:16, :], ADD)
                vP01 = fp.tile([128, 4, 128], bf16, tag="vP01", name="vP01")
                nc.vector.tensor_tensor(vP01[:], fA[:, 0:4, :], fA[:, 4:8, :], ADD)
                fC = fp.tile([128, 2, 128], bf16, tag="fC", name="fC")
                nc.vector.tensor_tensor(fC[:], xl[:, 24:26, :], xl[:, 26:28, :], ADD)
                fB = fp.tile([128, 4, 128], bf16, tag="fB", name="fB")
                nc.vector.tensor_tensor(fB[:], xl[:, 16:20, :], xl[:, 20:24, :], ADD)
                return vQ01, fA, vP01, fB, fC

            def emit_tree(bt, tag, bpfx, vp01, v0src, v1src, warmup=False, evac=True):
                """Depth-2 Karatsuba Toeplitz-1024 tree -> tX [128,1024] bf16."""
                TL = pp.tile([128, 1536], f32, tag=tag, name=tag + "L")
                if warmup:
                    for _ in range(4):
                        nc.tensor.matmul(
                            TL[:, 1024:1536], warm[:, 0:128], warm[:],
                            start=True, stop=True,
                        )
                t512(TL[:, 0:512], bpfx + "T0", lambda t: vp01[:, t, :])
                sXp = midp.tile([128, 512], bf16, tag="s" + bpfx, name="s" + bpfx)
                nc.scalar.copy(sXp[:], TL[:, 0:512])
                t512(TL[:, 512:1024], bpfx + "V", v1src)
                t512(TL[:, 1024:1536], bpfx + "W", v0src)
                tX = midp.tile([128, 1024], bf16, tag="t" + bpfx, name="t" + bpfx)
                if evac:
                    sXv = midp.tile([128, 1024], bf16, tag="sv" + bpfx, name="sv" + bpfx)
                    nc.scalar.copy(sXv[:, 0:512], TL[:, 512:1024])
                    nc.scalar.copy(sXv[:, 512:1024], TL[:, 1024:1536])
                    nc.vector.tensor_tensor(tX[:, 0:512], sXp[:], sXv[:, 0:512], ADD)
                    nc.vector.tensor_tensor(tX[:, 512:1024], sXp[:], sXv[:, 512:1024], ADD)
                else:
                    nc.vector.tensor_tensor(tX[:, 0:512], sXp[:], TL[:, 512:1024], ADD)
                    nc.vector.tensor_tensor(tX[:, 512:1024], sXp[:], TL[:, 1024:1536], ADD)
                return tX

            def emit_C(xl, fC):
                TCg = pp.tile([128, 768], f32, tag="TC", name="TC")
                t256(TCg[:, 0:256], "CQ", lambda t: xl[:, 26 + t, :])
                t256(TCg[:, 256:512], "CP", lambda t: fC[:, t, :])
                sCP = midp.tile([128, 256], bf16, tag="sCP", name="sCP")
                nc.vector.tensor_copy(sCP[:], TCg[:, 256:512])
                ynC = ynp.tile([128, 512], bf16, tag="ynC", name="ynC")
                nc.vector.tensor_tensor(ynC[:, 0:256], sCP[:], TCg[:, 0:256], SUB)
                t256(TCg[:, 512:768], "CR", lambda t: xl[:, 24 + t, :])
                nc.vector.tensor_tensor(ynC[:, 256:512], sCP[:], TCg[:, 512:768], ADD)
                return ynC

            def emit_B(xl, fB):
                T2g = pp.tile([128, 1536], f32, tag="T2", name="T2B")
                t512(T2g[:, 0:512], "BQ", lambda t: xl[:, 20 + t, :])
                t512(T2g[:, 512:1024], "BP", lambda t: fB[:, t, :])
                sBP = midp.tile([128, 512], bf16, tag="sBP", name="sBP")
                nc.scalar.copy(sBP[:], T2g[:, 512:1024])
                ynB = ynp.tile([128, 1024], bf16, tag="ynB", name="ynB")
                nc.vector.tensor_tensor(ynB[:, 0:512], sBP[:], T2g[:, 0:512], SUB)
                t512(T2g[:, 1024:1536], "BR", lambda t: xl[:, 16 + t, :])
                nc.vector.tensor_tensor(ynB[:, 512:1024], sBP[:], T2g[:, 1024:1536], ADD)
                return ynB

            def emit_D(psum, xl):
                # D shares the R generation's T1 tile (banks are disjoint);
                # ACT evacuates so cpp is SBUF-only and nothing D-related
                # gates the next block's T1 reuse
                t512(psum, "D", lambda t: xl[:, 28 + t, :])
                sD = midp.tile([128, 512], bf16, tag="sD", name="sD")
                nc.scalar.copy(sD[:], psum)
                return sD

            def emit_unfold_mid(TD, ynB, ynC):
                cpp = midp.tile([128, 1024], bf16, tag="cpp", name="cpp")
                nc.vector.tensor_tensor(cpp[:, 0:512], TD[:], ynC[:], ADD)
                nc.vector.tensor_tensor(cpp[:, 512:1024], TD[:], ynC[:], SUB)
                u1 = midp.tile([128, 1024], bf16, tag="u1", name="u1")
                nc.vector.tensor_tensor(u1[:], cpp[:], ynB[:], ADD)
                u2 = midp.tile([128, 1024], bf16, tag="u2", name="u2")
                nc.vector.tensor_tensor(u2[:], cpp[:], ynB[:], SUB)
                return u1, u2

            def emit_finals(b0, u1, u2, ynA, dve_share=False):
                # Pool by default; DVE takes half in the drain-critical blocks,
                # which also get per-1024 DMAs for finer tail interleave
                if dve_share:
                    for seg, usrc, alu, eng in (
                        (0, u1, ADD, nc.gpsimd), (1, u2, ADD, nc.gpsimd),
                        (2, u1, SUB, nc.gpsimd), (3, u2, SUB, nc.gpsimd),
                    ):
                        o = op.tile([128, 1024], bf16, tag="o", name="o", bufs=4)
                        eng.tensor_tensor(
                            o[:], usrc[:], ynA[:, 1024 * (seg % 2) : 1024 * (seg % 2) + 1024], alu
                        )
                        nc.sync.dma_start(
                            out_d[b0 : b0 + 128, 1024 * seg : 1024 * seg + 1024], o[:]
                        )
                    return
                oL = op.tile([128, 2048], bf16, tag="oL", name="oL")
                nc.gpsimd.tensor_tensor(oL[:, 0:1024], u1[:], ynA[:, 0:1024], ADD)
                nc.gpsimd.tensor_tensor(oL[:, 1024:2048], u2[:], ynA[:, 1024:2048], ADD)
                nc.sync.dma_start(out_d[b0 : b0 + 128, 0:2048], oL[:])
                oR = op.tile([128, 2048], bf16, tag="oR", name="oR")
                nc.gpsimd.tensor_tensor(oR[:, 0:1024], u1[:], ynA[:, 0:1024], SUB)
                nc.gpsimd.tensor_tensor(oR[:, 1024:2048], u2[:], ynA[:, 1024:2048], SUB)
                nc.sync.dma_start(out_d[b0 : b0 + 128, 2048:4096], oR[:])

            pending = None
            xls = {0: xl0, 1: xl1}
            folds_cur = emit_folds(xl0)
            for bt in range(NB):
                b0 = 128 * bt
                xl = xls[bt]
                vQ01, fA, vP01, fB, fC = folds_cur
                last = bt == NB - 1
                if not last:
                    if pending is not None:
                        pending()
                    tQ = emit_tree(
                        bt, "T1", "Q", vQ01,
                        lambda t: xl[:, 8 + t, :], lambda t: xl[:, 12 + t, :],
                        warmup=(bt == 0),
                    )
                    tP = emit_tree(
                        bt, "T2", "P", vP01,
                        lambda t: fA[:, t, :], lambda t: fA[:, 4 + t, :],
                    )
                    ynA = ynp.tile([128, 2048], bf16, tag="ynA", name="ynA")
                    nc.vector.tensor_tensor(ynA[:, 0:1024], tP[:], tQ[:], SUB)
                    TRp = pp.tile([128, 1536], f32, tag="T1", name="T1RD")
                    t1024(TRp[:, 0:1024], "AR", lambda t: xl[:, t, :])
                    sR = midp.tile([128, 1024], bf16, tag="sRR", name="sRR")
                    for h in (0, 1):
                        nc.scalar.copy(
                            sR[:, 512 * h : 512 * h + 512],
                            TRp[:, 512 * h : 512 * h + 512],
                        )
                        nc.vector.tensor_tensor(
                            ynA[:, 1024 + 512 * h : 1536 + 512 * h],
                            tP[:, 512 * h : 512 * h + 512],
                            sR[:, 512 * h : 512 * h + 512],
                            ADD,
                        )
                    sD = emit_D(TRp[:, 1024:1536], xl)
                    # next block's input + folds right behind the hi-halves in
                    # the DVE queue so block bt+1's first group is never gated
                    if bt + 2 < NB:
                        xls[bt + 2] = xlp.tile([128, 32, 128], bf16, tag="xl", name="xl")
                        nc.sync.dma_start(xls[bt + 2][:], xl_t[:, bt + 2, :, :])
                    folds_cur = emit_folds(xls[bt + 1])
                    ynB = emit_B(xl, fB)
                    ynC = emit_C(xl, fC)
                    cpp = midp.tile([128, 1024], bf16, tag="cpp", name="cpp")
                    nc.vector.tensor_tensor(cpp[:, 0:512], sD[:], ynC[:], ADD)
                    nc.vector.tensor_tensor(cpp[:, 512:1024], sD[:], ynC[:], SUB)
                    u1 = midp.tile([128, 1024], bf16, tag="u1", name="u1")
                    nc.vector.tensor_tensor(u1[:], cpp[:], ynB[:], ADD)
                    u2 = midp.tile([128, 1024], bf16, tag="u2", name="u2")
                    nc.vector.tensor_tensor(u2[:], cpp[:], ynB[:], SUB)
                    dve_share = bt >= NB - 3
                    pending = (
                        lambda b0=b0, u1=u1, u2=u2, ynA=ynA, d=dve_share: emit_finals(
                            b0, u1, u2, ynA, dve_share=d
                        )
                    )
                else:
                    # last block: C/B/D first, then Q/P trees, R last; finals
                    # for the y_lo half stream during R, y_hi half-granular
                    if pending is not None:
                        pending()
                        pending = None
                    ynC = emit_C(xl, fC)
                    ynB = emit_B(xl, fB)
                    TDg = pp.tile([128, 1536], f32, tag="T1", name="T1D")
                    sD = emit_D(TDg[:, 0:512], xl)
                    cpp = midp.tile([128, 1024], bf16, tag="cpp", name="cpp")
                    nc.vector.tensor_tensor(cpp[:, 0:512], sD[:], ynC[:], ADD)
                    nc.vector.tensor_tensor(cpp[:, 512:1024], sD[:], ynC[:], SUB)
                    u1 = midp.tile([128, 1024], bf16, tag="u1", name="u1")
                    nc.vector.tensor_tensor(u1[:], cpp[:], ynB[:], ADD)
                    u2 = midp.tile([128, 1024], bf16, tag="u2", name="u2")
                    nc.vector.tensor_tensor(u2[:], cpp[:], ynB[:], SUB)
                    tQ = emit_tree(
                        bt, "T1", "Q", vQ01,
                        lambda t: xl[:, 8 + t, :], lambda t: xl[:, 12 + t, :],
                    )
                    tP = emit_tree(
                        bt, "T2", "P", vP01,
                        lambda t: fA[:, t, :], lambda t: fA[:, 4 + t, :],
                    )
                    ynA = ynp.tile([128, 2048], bf16, tag="ynA", name="ynA")
                    nc.vector.tensor_tensor(ynA[:, 0:1024], tP[:], tQ[:], SUB)
                    for seg, alu, eng in ((0, ADD, "v"), (2, SUB, "p")):
                        o = op.tile([128, 1024], bf16, tag="o", name="o", bufs=4)
                        if eng == "v":
                            nc.vector.tensor_tensor(o[:], u1[:], ynA[:, 0:1024], alu)
                        else:
                            nc.gpsimd.tensor_tensor(o[:], u1[:], ynA[:, 0:1024], alu)
                        nc.sync.dma_start(
                            out_d[b0 : b0 + 128, 1024 * seg : 1024 * seg + 1024], o[:]
                        )
                    TRp = pp.tile([128, 1536], f32, tag="T2", name="T2R")
                    t1024(TRp[:, 0:1024], "AR", lambda t: xl[:, t, :])
                    for h in (0, 1):
                        nc.vector.tensor_tensor(
                            ynA[:, 1024 + 512 * h : 1536 + 512 * h],
                            tP[:, 512 * h : 512 * h + 512],
                            TRp[:, 512 * h : 512 * h + 512],
                            ADD,
                        )
                        for seg, alu, eng in ((1, ADD, "v"), (3, SUB, "p")):
                            o = op.tile([128, 512], bf16, tag="oh", name="oh", bufs=4)
                            args = (
                                o[:],
                                u2[:, 512 * h : 512 * h + 512],
                                ynA[:, 1024 + 512 * h : 1536 + 512 * h],
                                alu,
                            )
                            if eng == "v":
                                nc.vector.tensor_tensor(*args)
                            else:
                                nc.gpsimd.tensor_tensor(*args)
                            nc.sync.dma_start(
                                out_d[
                                    b0 : b0 + 128,
                                    1024 * seg + 512 * h : 1024 * seg + 512 * h + 512,
                                ],
                                o[:],
                            )
            if pending is not None:
                pending()

    nc.compile()
    return nc


def _get_nc():
    if "nc" not in _STATE:
        _STATE["nc"] = _build()
    return _STATE["nc"]


def _prep_inputs(x, w):
    x = np.ascontiguousarray(x, dtype=np.float32)
    w = np.asarray(w, dtype=np.float32)
    bands = _make_bands(w)
    in_maps = []
    for i in range(N_CORES):
        xl = _fold_x(x[i * B_SHARD : (i + 1) * B_SHARD])
        in_maps.append({"xl": xl.reshape(128, 32 * B_SHARD), "bands": bands})
    return in_maps


def kernel(x, w, _trace=False):
    from concourse.bass_utils import run_bass_kernel_spmd

    nc = _get_nc()
    in_maps = _prep_inputs(x, w)
    res = run_bass_kernel_spmd(nc, in_maps, list(range(N_CORES)), trace=_trace)
    out = np.concatenate(
        [np.asarray(res.results[i]["out"]).astype(np.float32) for i in range(N_CORES)],
        axis=0,
    )
    if _trace:
        _STATE["last_result"] = res
    return out
